# revision 1
# baseline (speedup 1.0000x reference)
"""Trainium2 Bass kernel for sparse (top-k) multi-head causal attention.

Problem (hardcoded shapes, from the reference):
  B=32, S=512, D=512, H=8, DK=64, k_index=5 (any k<=8 supported)
  out = TopKCausalAttention(q, k, v; w_q..w_o, b_q..b_o)

Sharding: data-parallel over batch across 8 NeuronCores (4 batches/core).

Per-core algorithm (all on one core, per batch b and head h):
  qhT[d, r] = (w_q/8)^T-projection of q (transposed layout, d on partitions)
  khT[d, c] likewise; vh[r, d] in natural layout.
  scores_psum[r-tile, :] = qhT.T @ khT  (+ bf16 identity-matmul adds the
      strictly-causal -1e32 mask on the diagonal tile; upper tiles skipped)
  e = exp(scores)                 (ACT, PSUM->SBUF)
  top8 = vector.max(e)            (top-8 per row, one DVE op)
  tau = top8[:, k-1]; rows < k get tau := 0 (keep everything valid)
  Z = sum(top8[:, :k]) per row    (rows < k: full-row sum; row 0: Z := 1)
  p = (e >= tau) * e * (1/Z)      (DVE scalar_tensor_tensor + GPSIMD
                                   tensor_scalar; exact top-k by value
                                   threshold, matching the reference
                                   `probs >= thresh` semantics)
  pT via PE transposes banked 4-wide into one PSUM tile, one wide
  evacuation per column-tile, then one wide attnT matmul per ci:
  attnT[d, r>=ci*128] += vh_ci_headslice.T @ pT_ci   (fp16)
  y[r, :] = sum_hp attnT_hp-slice.T @ w_o-slice (+ b_o)  -> DRAM out

  dtypes: q/k projections + QK^T in fp32 (top-k selection is
  discontinuous — lower precision flips selected indices; fp32r and
  fp16x2-split were measured to flip rows on the graded data). The v/p
  path runs in fp16 (same 1 cyc/row PE rate as bf16, 8x less rounding
  error; absmax ~5e-4 of scale).
"""

import math
import os

os.environ.setdefault("MYCRO_LOCAL_CACHE", "1")

from contextlib import ExitStack

import numpy as np

import concourse.bass as bass
import concourse.bacc as bacc
import concourse.mybir as mybir
import concourse.tile as tile
from concourse.bass_utils import run_bass_kernel_spmd

B, S, D, H = 32, 512, 512, 8
DK = D // H  # 64
NCORES = 8
BC = B // NCORES  # batches per core
RT = S // 128  # row tiles per sequence
FT = D // 128  # feature tiles
NEG = -1.0e32

F32 = mybir.dt.float32
BF16 = mybir.dt.bfloat16
F16 = mybir.dt.float16

_last_nc = None

# dtype config knobs (tweakable for perf iteration)
CFG = {
    "qk_dt": F32,    # q/k projection + QK^T matmuls (selection-critical: f32)
    "v_dt": F16,     # v projection / attnT / y matmuls (smooth path)
    "p_dt": F16,     # dtype of normalized probs (transpose + pV path)
    "trace": False,
}


def _build_program(k_index: int, has_bias: dict):
    """Builds the per-core Bass program. Returns (nc, input_names)."""
    nc = bacc.Bacc(
        "TRN2", target_bir_lowering=False, debug=False, num_devices=NCORES
    )

    QKDT = CFG["qk_dt"]
    VDT = CFG["v_dt"]
    PDT = CFG["p_dt"]

    # --- DRAM I/O -------------------------------------------------------
    qT = nc.dram_tensor("qT", (BC, D, S), QKDT, kind="ExternalInput").ap()
    kT = nc.dram_tensor("kT", (BC, D, S), QKDT, kind="ExternalInput").ap()
    vT = nc.dram_tensor("vT", (BC, D, S), VDT, kind="ExternalInput").ap()
    wq = nc.dram_tensor("wq", (D, D), QKDT, kind="ExternalInput").ap()
    wk = nc.dram_tensor("wk", (D, D), QKDT, kind="ExternalInput").ap()
    wv = nc.dram_tensor("wv", (D, D), VDT, kind="ExternalInput").ap()
    wo = nc.dram_tensor("wo", (D, D), VDT, kind="ExternalInput").ap()
    bias_aps = {}
    for name in ("bq", "bk", "bv", "bo"):
        if has_bias[name]:
            bias_aps[name] = nc.dram_tensor(
                name, (1, D), F32, kind="ExternalInput"
            ).ap()
    out = nc.dram_tensor("out", (BC, S, D), F32, kind="ExternalOutput").ap()

    # --- inline constants ----------------------------------------------
    ident_np = np.eye(128, dtype=np.float32)
    # additive strict-causal mask for a diagonal tile: M[r, c] = NEG if c >= r
    mask_np = np.where(
        np.arange(128)[None, :] >= np.arange(128)[:, None], NEG, 0.0
    ).astype(np.float32)
    ident_p = nc.inline_tensor(
        ident_np.astype(mybir.dt.np(PDT)), name="identp"
    ).ap()
    ident_b = nc.inline_tensor(
        ident_np.astype(mybir.dt.np(BF16)), name="identb"
    ).ap()
    maskT_b = nc.inline_tensor(
        mask_np.T.copy().astype(mybir.dt.np(BF16)), name="maskT"
    ).ap()
    ones_row = nc.inline_tensor(
        np.ones((1, S), dtype=np.float32), name="onesrow"
    ).ap()

    with tile.TileContext(nc) as tc, ExitStack() as ctx:
        # ---------------- pools ----------------
        consts = ctx.enter_context(tc.tile_pool(name="consts", bufs=1))
        xpool = ctx.enter_context(tc.tile_pool(name="xpool", bufs=2))
        projpool = ctx.enter_context(tc.tile_pool(name="projpool", bufs=2))
        epool = ctx.enter_context(tc.tile_pool(name="epool", bufs=20))
        ppool = ctx.enter_context(tc.tile_pool(name="ppool", bufs=8))
        pnpool = ctx.enter_context(tc.tile_pool(name="pnpool", bufs=12))
        ptpool = ctx.enter_context(tc.tile_pool(name="ptpool", bufs=12))
        smallpool = ctx.enter_context(tc.tile_pool(name="smallpool", bufs=4))
        atpool = ctx.enter_context(tc.tile_pool(name="atpool", bufs=3))
        ypool = ctx.enter_context(tc.tile_pool(name="ypool", bufs=3))

        ps_proj = ctx.enter_context(tc.tile_pool(name="ps_proj", bufs=2, space="PSUM"))
        ps_sc = ctx.enter_context(tc.tile_pool(name="ps_sc", bufs=2, space="PSUM"))
        ps_pt = ctx.enter_context(tc.tile_pool(name="ps_pt", bufs=1, space="PSUM"))
        ps_at = ctx.enter_context(tc.tile_pool(name="ps_at", bufs=2, space="PSUM"))
        ps_y = ctx.enter_context(tc.tile_pool(name="ps_y", bufs=1, space="PSUM"))

        # ---------------- resident constants ----------------
        # q/k weights first, then batch 0's activations, then the rest of
        # the weights: on the DMA queue this lets the first projection
        # matmuls start ~8us earlier instead of waiting for all 16 weight
        # tiles to land.
        wq_sb = [consts.tile_from(wq[ft * 128:(ft + 1) * 128, :], name=f"wq{ft}")
                 for ft in range(FT)]
        _xq0 = [xpool.tile_from(qT[0, ft * 128:(ft + 1) * 128, :],
                                name=f"xq{ft}") for ft in range(FT)]
        wk_sb = [consts.tile_from(wk[ft * 128:(ft + 1) * 128, :], name=f"wk{ft}")
                 for ft in range(FT)]
        preloaded = {}
        preloaded[0] = (
            _xq0,
            [xpool.tile_from(kT[0, ft * 128:(ft + 1) * 128, :],
                             name=f"xk{ft}") for ft in range(FT)],
            [xpool.tile_from(vT[0, ft * 128:(ft + 1) * 128, :],
                             name=f"xv{ft}") for ft in range(FT)],
        )
        wv_sb = [consts.tile_from(wv[ft * 128:(ft + 1) * 128, :], name=f"wv{ft}")
                 for ft in range(FT)]
        wo_sb = [consts.tile_from(wo[dt * 128:(dt + 1) * 128, :], name=f"wo{dt}")
                 for dt in range(FT)]
        identp_sb = consts.tile_from(ident_p, name="identp_sb")
        if PDT == mybir.dt.float32r:
            identp_sb = identp_sb.bitcast(PDT)  # same 4-byte bits as f32
        elif PDT == BF16:
            identp_sb = None  # use identb_sb at the call site
        identb_sb = consts.tile_from(ident_b, name="identb_sb")
        maskT_sb = consts.tile_from(maskT_b, name="maskT_sb")
        ones_sb = consts.tile_from(ones_row, name="ones_sb")
        bias_sb = {
            nm: consts.tile_from(ap, name=f"{nm}_sb") for nm, ap in bias_aps.items()
        }

        Exp = mybir.ActivationFunctionType.Exp
        AO = mybir.AluOpType

        def emit_proj(b, defer_v=False):
            """Loads + q/k/v projections for batch b."""
            if b in preloaded:
                xq, xk, xv = preloaded.pop(b)
            else:
                xq = [xpool.tile_from(qT[b, ft * 128:(ft + 1) * 128, :],
                                      name=f"xq{ft}") for ft in range(FT)]
                xk = [xpool.tile_from(kT[b, ft * 128:(ft + 1) * 128, :],
                                      name=f"xk{ft}") for ft in range(FT)]
                xv = [xpool.tile_from(vT[b, ft * 128:(ft + 1) * 128, :],
                                      name=f"xv{ft}") for ft in range(FT)]
            qhT, khT, vh = [], [], []
            # interleaved per dt so the first head-pair (dt=0) has both its
            # qhT and khT tiles after two projection groups, not five
            for dt in range(FT):
                for which, w_sb, xs, bkey, outl in (
                        ("q", wq_sb, xq, "bq", qhT), ("k", wk_sb, xk, "bk", khT)):
                    ps = ps_proj.tile([128, S], F32, name="psq", tag="psproj")
                    nbias = bkey in bias_sb
                    for ft in range(FT):
                        nc.tensor.matmul(
                            ps, w_sb[ft][:, dt * 128:(dt + 1) * 128], xs[ft],
                            start=(ft == 0), stop=(ft == FT - 1 and not nbias))
                    if nbias:
                        nc.tensor.matmul(
                            ps, bias_sb[bkey][0:1, dt * 128:(dt + 1) * 128],
                            ones_sb, start=False, stop=True)
                    t = projpool.tile([128, S], QKDT, name=f"{which}hT{dt}",
                                      tag=f"{which}hT{dt}")
                    nc.scalar.copy(t, ps)
                    outl.append(t)
            def do_vproj(rts=range(RT)):
                for rt in rts:
                    ps = ps_proj.tile([128, D], F32, name="psv", tag="psproj")
                    nbias = "bv" in bias_sb
                    for ft in range(FT):
                        nc.tensor.matmul(
                            ps, xv[ft][:, rt * 128:(rt + 1) * 128], wv_sb[ft],
                            start=(ft == 0), stop=(ft == FT - 1 and not nbias))
                    if nbias:
                        nc.tensor.matmul(
                            ps, ones_sb[0:1, 0:128], bias_sb["bv"],
                            start=False, stop=True)
                    t = projpool.tile([128, D], VDT, name=f"vh{rt}", tag=f"vh{rt}")
                    nc.scalar.copy(t, ps)
                    vh.append(t)
                return vh
            if defer_v:
                return qhT, khT, do_vproj
            return qhT, khT, do_vproj()

        def emit_headpair(hp, qhT, khT, vh):
            """Scores / top-k softmax / transposes / attnT for one head pair.

            The two heads occupy partition halves 0:64 / 64:128 of qhT/khT, so
            their K=64 QK matmuls land in different PE row groups; issuing
            them back-to-back lets them run concurrently. The same applies to
            the M=64 attnT matmuls (different column groups), interleaved at
            the end.
            """
            etiles = [[None] * RT, [None] * RT]
            zfulls = [None, None]
            top8s = []
            for hh in range(2):
                top8s.append(smallpool.tile(
                    [128, RT * 8], F32, name=f"top8{hh}", tag=f"top8{hh}"))
            for ri in range(RT):
                w = (ri + 1) * 128
                spss = []
                # both heads' K=64 QK matmuls first (disjoint PE row groups ->
                # array-level concurrency), then the full-K mask matmuls which
                # would otherwise serialize them
                for hh in range(2):
                    po = hh * 64
                    sps = ps_sc.tile([128, S], F32, name="sps", tag="sps")
                    nc.tensor.matmul(
                        sps[:, 0:w],
                        qhT[hp][po:po + 64, ri * 128:(ri + 1) * 128],
                        khT[hp][po:po + 64, 0:w],
                        start=True, stop=False)
                    spss.append(sps)
                for hh in range(2):
                    nc.tensor.matmul(
                        spss[hh][:, ri * 128:(ri + 1) * 128],
                        maskT_sb, identb_sb, start=False, stop=True)
                for hh in range(2):
                    e = epool.tile([128, S], F32, name="e", tag="e")
                    if ri == 0:
                        zf = smallpool.tile(
                            [128, 1], F32, name=f"zfull{hh}", tag=f"zfull{hh}")
                        zfulls[hh] = zf
                        nc.scalar.activation(
                            e[:, 0:w], spss[hh][:, 0:w], Exp, accum_out=zf)
                    else:
                        nc.scalar.activation(e[:, 0:w], spss[hh][:, 0:w], Exp)
                    nc.vector.max(
                        out=top8s[hh][:, ri * 8:(ri + 1) * 8], in_=e[:, 0:w])
                    etiles[hh][ri] = e
            ptrows = [[None] * RT, [None] * RT]
            for hh in range(2):
                top8 = top8s[hh]
                # thresholds + normalizers (batched across row-tiles)
                zk = smallpool.tile([128, RT], F32, name="zk", tag="zk")
                nc.vector.reduce_sum(
                    zk, top8.rearrange("p (r e) -> p r e", e=8)[:, :, 0:k_index],
                    axis=mybir.AxisListType.X)
                nc.vector.tensor_copy(zk[0:k_index, 0:1], zfulls[hh][0:k_index, :])
                nc.vector.memset(zk[0:1, 0:1], 1.0)
                # rows < k keep every valid entry: tau := 0
                nc.vector.memset(top8[0:k_index, k_index - 1:k_index], 0.0)
                rz = smallpool.tile([128, RT], F32, name="rz", tag="rz")
                nc.vector.reciprocal(rz, zk)

                # masked, normalized probs
                pns = []
                for ri in range(RT):
                    w = (ri + 1) * 128
                    e = etiles[hh][ri]
                    tau = top8[:, ri * 8 + k_index - 1: ri * 8 + k_index]
                    pu = ppool.tile([128, S], F32, name="pu", tag="pu")
                    nc.vector.scalar_tensor_tensor(
                        pu[:, 0:w], e[:, 0:w], tau, e[:, 0:w],
                        op0=AO.is_ge, op1=AO.mult)
                    pn = pnpool.tile([128, S], PDT, name="pn", tag="pn")
                    nc.gpsimd.tensor_scalar(
                        pn[:, 0:w], pu[:, 0:w], rz[:, ri:ri + 1], None,
                        op0=AO.mult)
                    pns.append(pn)
                # transpose p per column-tile: bank 4 PE transposes into one
                # PSUM tile, then one wide evacuation per ci
                for ci in range(RT):
                    wv_ = (RT - ci) * 128
                    ptb = ps_pt.tile([128, S], PDT, name="ptb", tag="ptb")
                    for ri in range(ci, RT):
                        nc.tensor.transpose(
                            ptb[:, (ri - ci) * 128:(ri - ci + 1) * 128],
                            pns[ri][:, ci * 128:(ci + 1) * 128],
                            identb_sb if PDT == BF16 else identp_sb)
                    ptrow = ptpool.tile([128, S], PDT, name="ptrow", tag="ptrow")
                    if ci % 2 == 0:
                        nc.vector.tensor_copy(ptrow[:, 0:wv_], ptb[:, 0:wv_])
                    else:
                        nc.scalar.copy(ptrow[:, 0:wv_], ptb[:, 0:wv_])
                    ptrows[hh][ci] = ptrow
            # attnT: one wide matmul per (ci, head); the two heads' M=64
            # matmuls hit different column groups -> interleave for concurrency
            def finish(vh):
                at_ps = ps_at.tile([128, S], F32, name="atps", tag="atps")
                for ci in range(RT):
                    wv_ = (RT - ci) * 128
                    for hh in range(2):
                        h = 2 * hp + hh
                        po = hh * 64
                        nc.tensor.matmul(
                            at_ps[po:po + 64, ci * 128:S],
                            vh[ci][:, h * DK:(h + 1) * DK],
                            ptrows[hh][ci][:, 0:wv_],
                            start=(ci == 0), stop=(ci == RT - 1),
                            skip_group_check=True)
                at = atpool.tile([128, S], VDT, name=f"at{hp}", tag=f"at{hp}")
                nc.scalar.copy(at, at_ps)
                return at
            if vh is None:
                return finish
            return finish(vh)

        def emit_y(b, attnT_sb):
            for ri in range(RT):
                yps = ps_y.tile([128, D], F32, name="yps", tag="yps")
                nbias = "bo" in bias_sb
                for hp in range(FT):
                    nc.tensor.matmul(
                        yps, attnT_sb[hp][:, ri * 128:(ri + 1) * 128], wo_sb[hp],
                        start=(hp == 0), stop=(hp == FT - 1 and not nbias))
                if nbias:
                    nc.tensor.matmul(
                        yps, ones_sb[0:1, 0:128], bias_sb["bo"],
                        start=False, stop=True)
                y = ypool.tile([128, D], F32, name="y", tag="y")
                nc.scalar.copy(y, yps)
                nc.scalar.dma_start(out[b, ri * 128:(ri + 1) * 128, :], y)

        for b in range(BC):
            last = b == BC - 1
            qhT, khT, vh = emit_proj(b, defer_v=last)
            attnT_sb = []
            if last:
                # cooldown filler: last batch's v-projection groups spread
                # one per head-pair scores phase, filling PE gaps that no
                # next-batch projections exist to fill; the deferred attnT
                # finishes are dependency-driven and emitted afterwards
                do_v = vh
                fins = []
                vh = None
                for hp in range(FT):
                    fins.append(emit_headpair(hp, qhT, khT, None))
                    vh = do_v(rts=[hp])
                attnT_sb = [fin(vh) for fin in fins]
            else:
                for hp in range(FT):
                    attnT_sb.append(emit_headpair(hp, qhT, khT, vh))
            emit_y(b, attnT_sb)

    nc.compile()
    return nc


def kernel(**inputs):
    q = np.asarray(inputs["q"], np.float32)
    k = np.asarray(inputs["k"], np.float32)
    v = np.asarray(inputs["v"], np.float32)
    w_q = np.asarray(inputs["w_q"], np.float32)
    w_k = np.asarray(inputs["w_k"], np.float32)
    w_v = np.asarray(inputs["w_v"], np.float32)
    w_o = np.asarray(inputs["w_o"], np.float32)
    b_q = np.asarray(inputs["b_q"], np.float32)
    b_k = np.asarray(inputs["b_k"], np.float32)
    b_v = np.asarray(inputs["b_v"], np.float32)
    b_o = np.asarray(inputs["b_o"], np.float32)
    k_index = int(np.asarray(inputs["k_index"]))
    assert 1 <= k_index <= 8, f"kernel supports k_index<=8, got {k_index}"

    # fold the 1/sqrt(DK) score scaling into the q projection (exact: 2^-3)
    scale = np.float32(1.0 / math.sqrt(DK))
    w_qs = (w_q * scale).astype(np.float32)
    b_qs = (b_q * scale).astype(np.float32)

    has_bias = {
        "bq": bool(np.any(b_qs)),
        "bk": bool(np.any(b_k)),
        "bv": bool(np.any(b_v)),
        "bo": bool(np.any(b_o)),
    }

    nc = _build_program(k_index, has_bias)
    global _last_nc
    _last_nc = nc

    npq = mybir.dt.np(CFG["qk_dt"])
    npv = mybir.dt.np(CFG["v_dt"])
    shared = {
        "wq": np.ascontiguousarray(w_qs.astype(npq)),
        "wk": np.ascontiguousarray(w_k.astype(npq)),
        "wv": np.ascontiguousarray(w_v.astype(npv)),
        "wo": np.ascontiguousarray(w_o.astype(npv)),
    }
    for nm, arr in (("bq", b_qs), ("bk", b_k), ("bv", b_v), ("bo", b_o)):
        if has_bias[nm]:
            shared[nm] = np.ascontiguousarray(arr.reshape(1, D).astype(np.float32))

    in_maps = []
    for c in range(NCORES):
        sl = slice(c * BC, (c + 1) * BC)
        in_maps.append(dict(
            shared,
            qT=np.ascontiguousarray(q[sl].transpose(0, 2, 1).astype(npq)),
            kT=np.ascontiguousarray(k[sl].transpose(0, 2, 1).astype(npq)),
            vT=np.ascontiguousarray(v[sl].transpose(0, 2, 1).astype(npv)),
        ))

    res = run_bass_kernel_spmd(
        nc, in_maps, core_ids=list(range(NCORES)), trace=CFG["trace"]
    )
    out = np.concatenate([r["out"] for r in res.results], axis=0)
    kernel.last_result = res
    return out



# revision 45
# speedup vs baseline: 1.1258x; 1.1258x over previous
"""Trainium2 Bass kernel for sparse (top-k) multi-head causal attention.

Problem (hardcoded shapes, from the reference):
  B=32, S=512, D=512, H=8, DK=64, k_index=5 (any k<=8 supported)
  out = TopKCausalAttention(q, k, v; w_q..w_o, b_q..b_o)

Sharding: data-parallel over batch across 8 NeuronCores (4 batches/core).

Numerics: the top-k selection is discontinuous, so scores need ~2^-16
relative accuracy vs the fp32 reference.  fp32 matmuls run at 4 cyc/row
on the PE; instead the q/k path uses f16 hi/lo PAIR arithmetic (3
matmuls at 1 cyc/row, ~2^-22 effective):
  q = qhi + qlo (host-split f16), w_q = whi + wlo (host-split f16)
  qh = qhi*whi + qhi*wlo + qlo*whi        (dropped qlo*wlo ~ 2^-22)
  qh -> (hi, lo) f16 evac split; scores = qhh*khh + qhh*khl + qhl*khh
Measured end-to-end rel err vs fp32 reference: ~2.7e-3 (gate 2e-2).
(float32r at 1 cyc/row was measured: its DMA/weight path quantizes to
11 mantissa bits -> rel err 1.6e-2, too close to the gate; and
engine-written f32r tiles load garbage as PE weights.)

Per-core algorithm (per batch b, head pair hp, heads hh=0,1):
  scores_psum[r-tile, 0:w] = 3 pair matmuls per head (+ bf16
      identity-matmul adds the strictly-causal -1e32 mask on the
      diagonal tile; upper tiles skipped)
  e = exp(scores)                 (ACT, PSUM->SBUF, accum Z at ri=0)
  top8 = vector.max(e)            (top-8 per row, one DVE op)
  tau = top8[:, k-1]; rows < k get tau := 0; Z = sum(top8[:, :k]) or
      full-row sum for rows < k; row 0: Z := 1
  pu = (e >= tau) * e             (DVE stt, f16 out; exact-by-value
                                   threshold, matching reference)
  R[ri] = diag(1/Z)               (f16, tensor_scalar identity * rz)
  ptb[c, r] = pu[r, c]^T @ R      (regular PE matmul: transpose AND
                                   1/Z normalization in one 1cyc/row op)
  attnT[d, r] += vh_ci^T @ ptrow_ci   (f16, triangular)
  y[r, :] = sum_hp attnT^T @ w_o (+ b_o) -> DRAM
"""

import math
import os

os.environ.setdefault("MYCRO_LOCAL_CACHE", "1")

from contextlib import ExitStack

import numpy as np

import concourse.bass as bass
import concourse.bacc as bacc
import concourse.mybir as mybir
import concourse.tile as tile
from concourse.bass_utils import run_bass_kernel_spmd

B, S, D, H = 32, 512, 512, 8
DK = D // H  # 64
NCORES = 8
BC = B // NCORES  # batches per core
RT = S // 128  # row tiles per sequence
FT = D // 128  # feature tiles
NEG = -1.0e32

F32 = mybir.dt.float32
BF16 = mybir.dt.bfloat16
F16 = mybir.dt.float16

_last_nc = None

CFG = {
    "trace": False,
    "mask_on_pe": True,   # bf16 identity-matmul mask vs DVE tensor add
}


def _build_program(k_index: int, has_bias: dict):
    """Builds the per-core Bass program."""
    nc = bacc.Bacc(
        "TRN2", target_bir_lowering=False, debug=False, num_devices=NCORES
    )

    # --- DRAM I/O -------------------------------------------------------
    # q/k in transposed layout, host-split into f16 hi/lo pairs and
    # host-pre-arranged as [128, FT*S] (ft-blocks side by side) so each
    # tensor loads with ONE wide DMA instead of FT strided ones.
    qhiT = nc.dram_tensor("qhiT", (BC, 128, FT * S), F16, kind="ExternalInput").ap()
    qloT = nc.dram_tensor("qloT", (BC, 128, FT * S), F16, kind="ExternalInput").ap()
    khiT = nc.dram_tensor("khiT", (BC, 128, FT * S), F16, kind="ExternalInput").ap()
    kloT = nc.dram_tensor("kloT", (BC, 128, FT * S), F16, kind="ExternalInput").ap()
    vT = nc.dram_tensor("vT", (BC, 128, FT * S), F16, kind="ExternalInput").ap()
    wqhi = nc.dram_tensor("wqhi", (128, FT * D), F16, kind="ExternalInput").ap()
    wqlo = nc.dram_tensor("wqlo", (128, FT * D), F16, kind="ExternalInput").ap()
    wkhi = nc.dram_tensor("wkhi", (128, FT * D), F16, kind="ExternalInput").ap()
    wklo = nc.dram_tensor("wklo", (128, FT * D), F16, kind="ExternalInput").ap()
    wv = nc.dram_tensor("wv", (128, FT * D), F16, kind="ExternalInput").ap()
    wo = nc.dram_tensor("wo", (128, FT * D), F16, kind="ExternalInput").ap()
    bias_aps = {}
    for name in ("bq", "bk", "bv", "bo"):
        if has_bias[name]:
            bias_aps[name] = nc.dram_tensor(
                name, (1, D), F32, kind="ExternalInput"
            ).ap()
    out = nc.dram_tensor("out", (BC, S, D), F32, kind="ExternalOutput").ap()

    # --- inline constants ----------------------------------------------
    ident_np = np.eye(128, dtype=np.float32)
    mask_np = np.where(
        np.arange(128)[None, :] >= np.arange(128)[:, None], NEG, 0.0
    ).astype(np.float32)
    ident_p = nc.inline_tensor(
        ident_np.astype(mybir.dt.np(F16)), name="identp"
    ).ap()
    ident_b = nc.inline_tensor(
        ident_np.astype(mybir.dt.np(BF16)), name="identb"
    ).ap()
    maskT_b = nc.inline_tensor(
        mask_np.T.copy().astype(mybir.dt.np(BF16)), name="maskT"
    ).ap()
    ones_row = nc.inline_tensor(
        np.ones((1, S), dtype=np.float32), name="onesrow"
    ).ap()

    with tile.TileContext(nc) as tc, ExitStack() as ctx:
        # ---------------- pools ----------------
        consts = ctx.enter_context(tc.tile_pool(name="consts", bufs=1))
        xpool = ctx.enter_context(tc.tile_pool(name="xpool", bufs=2))
        projpool = ctx.enter_context(tc.tile_pool(name="projpool", bufs=2))
        epool = ctx.enter_context(tc.tile_pool(name="epool", bufs=20))
        pnpool = ctx.enter_context(tc.tile_pool(name="pnpool", bufs=12))
        rpool = ctx.enter_context(tc.tile_pool(name="rpool", bufs=10))
        ptpool = ctx.enter_context(tc.tile_pool(name="ptpool", bufs=12))
        smallpool = ctx.enter_context(tc.tile_pool(name="smallpool", bufs=4))
        atpool = ctx.enter_context(tc.tile_pool(name="atpool", bufs=3))
        ypool = ctx.enter_context(tc.tile_pool(name="ypool", bufs=3))

        ps_proj = ctx.enter_context(tc.tile_pool(name="ps_proj", bufs=2, space="PSUM"))
        ps_sc = ctx.enter_context(tc.tile_pool(name="ps_sc", bufs=3, space="PSUM"))
        ps_pt = ctx.enter_context(tc.tile_pool(name="ps_pt", bufs=1, space="PSUM"))
        ps_at = ctx.enter_context(tc.tile_pool(name="ps_at", bufs=1, space="PSUM"))
        ps_y = ctx.enter_context(tc.tile_pool(name="ps_y", bufs=1, space="PSUM"))

        # ---------------- resident constants ----------------
        # combined [128, FT*S] tiles: one wide DMA per tensor; q weights +
        # batch 0's q first so the first projection matmuls start earliest.
        HW = FT * D // 2
        wqh_sb = consts.tile([128, FT * D], F16, name="wqh")
        nc.sync.dma_start(wqh_sb[:, 0:HW], wqhi[:, 0:HW])
        _xq0h = xpool.tile([128, FT * S], F16, name="xqh", tag="xqh")
        nc.sync.dma_start(_xq0h[:, 0:HW], qhiT[0, :, 0:HW])
        wql_sb = consts.tile([128, FT * D], F16, name="wql")
        nc.sync.dma_start(wql_sb[:, 0:HW], wqlo[:, 0:HW])
        _xq0l = xpool.tile([128, FT * S], F16, name="xql", tag="xql")
        nc.sync.dma_start(_xq0l[:, 0:HW], qloT[0, :, 0:HW])
        nc.sync.dma_start(wqh_sb[:, HW:], wqhi[:, HW:])
        nc.sync.dma_start(_xq0h[:, HW:], qhiT[0, :, HW:])
        nc.sync.dma_start(wql_sb[:, HW:], wqlo[:, HW:])
        nc.sync.dma_start(_xq0l[:, HW:], qloT[0, :, HW:])
        _xq0 = (_xq0h, _xq0l)
        wkh_sb = consts.tile_from(wkhi, name="wkh")
        _xk0h = xpool.tile_from(khiT[0], name="xkh")
        wkl_sb = consts.tile_from(wklo, name="wkl")
        preloaded = {}
        preloaded[0] = (
            _xq0,
            (_xk0h, xpool.tile_from(kloT[0], name="xkl")),
            xpool.tile_from(vT[0], name="xv"),
        )
        wv_sb = consts.tile_from(wv, name="wv")
        wo_sb = consts.tile_from(wo, name="wo")
        identp_sb = consts.tile_from(ident_p, name="identp_sb")
        identb_sb = consts.tile_from(ident_b, name="identb_sb")
        maskT_sb = consts.tile_from(maskT_b, name="maskT_sb")
        ones_sb = consts.tile_from(ones_row, name="ones_sb")
        bias_sb = {
            nm: consts.tile_from(ap, name=f"{nm}_sb") for nm, ap in bias_aps.items()
        }

        Exp = mybir.ActivationFunctionType.Exp
        AO = mybir.AluOpType

        def emit_proj(b, defer_v=False):
            """Loads + q/k/v projections for batch b.

            q/k: f16 pair-product accumulation (12 matmuls per output
            tile), evacuated as an f16 hi/lo split: hi via ACT copy,
            lo = psum - hi via DVE/Pool tensor_tensor subtract.
            """
            if b in preloaded:
                (xqh, xql), (xkh, xkl), xv = preloaded.pop(b)
            else:
                xqh = xpool.tile_from(qhiT[b], name="xqh")
                xql = xpool.tile_from(qloT[b], name="xql")
                xkh = xpool.tile_from(khiT[b], name="xkh")
                xkl = xpool.tile_from(kloT[b], name="xkl")
                xv = xpool.tile_from(vT[b], name="xv")
            qhT, khT, vh = [], [], []  # qhT/khT: list of (hi, lo) per dt
            for dt in range(FT):
                for which, whi_sb, wlo_sb, xh, xl, bkey, outl in (
                        ("q", wqh_sb, wql_sb, xqh, xql, "bq", qhT),
                        ("k", wkh_sb, wkl_sb, xkh, xkl, "bk", khT)):
                    ps = ps_proj.tile([128, S], F32, name="psq", tag="psproj")
                    nbias = bkey in bias_sb
                    nmm = 3 * FT
                    i = 0
                    for ft in range(FT):
                        wsl = slice(ft * D + dt * 128, ft * D + (dt + 1) * 128)
                        xsl = slice(ft * S, (ft + 1) * S)
                        for w_sb, xs in ((whi_sb, xh), (wlo_sb, xh),
                                         (whi_sb, xl)):
                            i += 1
                            nc.tensor.matmul(
                                ps, w_sb[:, wsl], xs[:, xsl],
                                start=(i == 1),
                                stop=(i == nmm and not nbias))
                    if nbias:
                        nc.tensor.matmul(
                            ps, bias_sb[bkey][0:1, dt * 128:(dt + 1) * 128],
                            ones_sb, start=False, stop=True)
                    thi = projpool.tile([128, S], F16, name=f"{which}hT{dt}h",
                                        tag=f"{which}hT{dt}h")
                    nc.scalar.copy(thi, ps)
                    tlo = projpool.tile([128, S], F16, name=f"{which}hT{dt}l",
                                        tag=f"{which}hT{dt}l")
                    # GPSIMD cannot access PSUM (walrus constraint): the
                    # latency-critical lo evac goes on DVE
                    nc.vector.tensor_tensor(tlo, ps, thi, op=AO.subtract)
                    outl.append((thi, tlo))

            def do_vproj(rts=range(RT)):
                for rt in rts:
                    ps = ps_proj.tile([128, D], F32, name="psv", tag="psproj")
                    nbias = "bv" in bias_sb
                    for ft in range(FT):
                        nc.tensor.matmul(
                            ps, xv[:, ft * S + rt * 128:ft * S + (rt + 1) * 128],
                            wv_sb[:, ft * D:(ft + 1) * D],
                            start=(ft == 0), stop=(ft == FT - 1 and not nbias))
                    if nbias:
                        nc.tensor.matmul(
                            ps, ones_sb[0:1, 0:128], bias_sb["bv"],
                            start=False, stop=True)
                    t = projpool.tile([128, D], F16, name=f"vh{rt}", tag=f"vh{rt}")
                    nc.scalar.copy(t, ps)
                    vh.append(t)
                return vh
            if defer_v:
                return qhT, khT, do_vproj
            return qhT, khT, do_vproj()

        def emit_headpair(hp, qhT, khT, vh):
            """Scores / top-k softmax / normalized transpose / attnT for one
            head pair (partition halves 0:64 / 64:128 of the proj tiles)."""
            etiles = [[None] * RT, [None] * RT]
            zfulls = [None, None]
            top8s = []
            for hh in range(2):
                top8s.append(smallpool.tile(
                    [128, RT * 8], F32, name=f"top8{hh}", tag=f"top8{hh}"))
            qh_hi, qh_lo = qhT[hp]
            kh_hi, kh_lo = khT[hp]
            for ri in range(RT):
                w = (ri + 1) * 128
                spss = []
                for hh in range(2):
                    po = hh * 64
                    sps = ps_sc.tile([128, S], F32, name="sps", tag="sps")
                    rsl = slice(ri * 128, (ri + 1) * 128)
                    for i, (qt, kt) in enumerate((
                            (qh_hi, kh_hi), (qh_hi, kh_lo), (qh_lo, kh_hi))):
                        nc.tensor.matmul(
                            sps[:, 0:w],
                            qt[po:po + 64, rsl],
                            kt[po:po + 64, 0:w],
                            start=(i == 0), stop=False)
                    spss.append(sps)
                for hh in range(2):
                    nc.tensor.matmul(
                        spss[hh][:, ri * 128:(ri + 1) * 128],
                        maskT_sb, identb_sb, start=False, stop=True)
                for hh in range(2):
                    e = epool.tile([128, S], F32, name="e", tag="e")
                    if ri == 0:
                        zf = smallpool.tile(
                            [128, 1], F32, name=f"zfull{hh}", tag=f"zfull{hh}")
                        zfulls[hh] = zf
                        nc.scalar.activation(
                            e[:, 0:w], spss[hh][:, 0:w], Exp, accum_out=zf)
                    else:
                        nc.scalar.activation(e[:, 0:w], spss[hh][:, 0:w], Exp)
                    nc.vector.max(
                        out=top8s[hh][:, ri * 8:(ri + 1) * 8], in_=e[:, 0:w])
                    etiles[hh][ri] = e
            ptrows = [[None] * RT, [None] * RT]
            for hh in range(2):
                top8 = top8s[hh]
                zk = smallpool.tile([128, RT], F32, name="zk", tag="zk")
                nc.vector.reduce_sum(
                    zk, top8.rearrange("p (r e) -> p r e", e=8)[:, :, 0:k_index],
                    axis=mybir.AxisListType.X)
                nc.vector.tensor_copy(zk[0:k_index, 0:1], zfulls[hh][0:k_index, :])
                nc.vector.memset(zk[0:1, 0:1], 1.0)
                # rows < k keep every valid entry: tau := 0
                nc.vector.memset(top8[0:k_index, k_index - 1:k_index], 0.0)
                rz = smallpool.tile([128, RT], F32, name="rz", tag="rz")
                nc.vector.reciprocal(rz, zk)

                # R[ri] = diag(rz[:, ri]) in f16: ACT copy-with-scale of
                # the identity
                rtiles = []
                for ri in range(RT):
                    R = rpool.tile([128, 128], F16, name="rdiag", tag="rdiag")
                    nc.gpsimd.tensor_scalar(
                        R, identp_sb, rz[:, ri:ri + 1], None, op0=AO.mult)
                    rtiles.append(R)

                # masked (unnormalized) probs, f16
                pns = []
                for ri in range(RT):
                    w = (ri + 1) * 128
                    e = etiles[hh][ri]
                    tau = top8[:, ri * 8 + k_index - 1: ri * 8 + k_index]
                    pn = pnpool.tile([128, S], F16, name="pn", tag="pn")
                    nc.vector.scalar_tensor_tensor(
                        pn[:, 0:w], e[:, 0:w], tau, e[:, 0:w],
                        op0=AO.is_ge, op1=AO.mult)
                    pns.append(pn)
                # normalized transpose: ptb[c, r-block] = pn[r-block, c]^T
                # @ diag(rz) -- regular matmul, transpose + 1/Z in one op
                for ci in range(RT):
                    wv_ = (RT - ci) * 128
                    ptb = ps_pt.tile([128, S], F32, name="ptb", tag="ptb")
                    for ri in range(ci, RT):
                        nc.tensor.matmul(
                            ptb[:, (ri - ci) * 128:(ri - ci + 1) * 128],
                            pns[ri][:, ci * 128:(ci + 1) * 128],
                            rtiles[ri], start=True, stop=True)
                    ptrow = ptpool.tile([128, S], F16, name="ptrow", tag="ptrow")
                    nc.scalar.copy(ptrow[:, 0:wv_], ptb[:, 0:wv_])
                    ptrows[hh][ci] = ptrow

            def finish(vh):
                at_ps = ps_at.tile([128, S], F32, name="atps", tag="atps")
                for ci in range(RT):
                    wv_ = (RT - ci) * 128
                    for hh in range(2):
                        h = 2 * hp + hh
                        po = hh * 64
                        nc.tensor.matmul(
                            at_ps[po:po + 64, ci * 128:S],
                            vh[ci][:, h * DK:(h + 1) * DK],
                            ptrows[hh][ci][:, 0:wv_],
                            start=(ci == 0), stop=(ci == RT - 1),
                            skip_group_check=True)
                at = atpool.tile([128, S], F16, name=f"at{hp}", tag=f"at{hp}")
                nc.scalar.copy(at, at_ps)
                return at
            if vh is None:
                return finish
            return finish(vh)

        def emit_y(b, attnT_sb):
            for ri in range(RT):
                yps = ps_y.tile([128, D], F32, name="yps", tag="yps")
                nbias = "bo" in bias_sb
                for hp in range(FT):
                    nc.tensor.matmul(
                        yps, attnT_sb[hp][:, ri * 128:(ri + 1) * 128],
                        wo_sb[:, hp * D:(hp + 1) * D],
                        start=(hp == 0), stop=(hp == FT - 1 and not nbias))
                if nbias:
                    nc.tensor.matmul(
                        yps, ones_sb[0:1, 0:128], bias_sb["bo"],
                        start=False, stop=True)
                y = ypool.tile([128, D], F32, name="y", tag="y")
                nc.scalar.copy(y, yps)
                nc.scalar.dma_start(out[b, ri * 128:(ri + 1) * 128, :], y)

        # proj for batch b+1 is emitted between hp1 and hp2 of batch b
        # (latency-critical DVE lo-subtracts enqueue ahead of later head
        # pairs' DVE chains).  The LAST batch's head pairs are interleaved
        # into batch BC-2's stream so only two chains drain at the tail.
        projs = {0: emit_proj(0)}
        ats = {b: [] for b in range(BC)}
        for b in range(BC - 1):
            qhT, khT, vh = projs.pop(b)
            if b < BC - 2:
                for hp in range(FT):
                    ats[b].append(emit_headpair(hp, qhT, khT, vh))
                    if hp == 1:
                        projs[b + 1] = emit_proj(b + 1)
                emit_y(b, ats[b])
            else:
                # interleave tail: b2.hp0 b2.hp1 [proj3] b2.hp2 b3.hp0
                # b2.hp3 b3.hp1 y2 b3.hp2 b3.hp3 y3
                ats[b].append(emit_headpair(0, qhT, khT, vh))
                ats[b].append(emit_headpair(1, qhT, khT, vh))
                projs[b + 1] = emit_proj(b + 1)
                qhT3, khT3, vh3 = projs.pop(b + 1)
                ats[b].append(emit_headpair(2, qhT, khT, vh))
                ats[b + 1].append(emit_headpair(0, qhT3, khT3, vh3))
                ats[b].append(emit_headpair(3, qhT, khT, vh))
                ats[b + 1].append(emit_headpair(1, qhT3, khT3, vh3))
                emit_y(b, ats[b])
                ats[b + 1].append(emit_headpair(2, qhT3, khT3, vh3))
                ats[b + 1].append(emit_headpair(3, qhT3, khT3, vh3))
                emit_y(b + 1, ats[b + 1])

    nc.compile()
    return nc


def _split16(x):
    """Split fp32 array into (hi, lo) f16 pair with hi + lo ~= x."""
    hi = x.astype(np.float16)
    lo = (x - hi.astype(np.float32)).astype(np.float16)
    return hi, lo


def kernel(**inputs):
    q = np.asarray(inputs["q"], np.float32)
    k = np.asarray(inputs["k"], np.float32)
    v = np.asarray(inputs["v"], np.float32)
    w_q = np.asarray(inputs["w_q"], np.float32)
    w_k = np.asarray(inputs["w_k"], np.float32)
    w_v = np.asarray(inputs["w_v"], np.float32)
    w_o = np.asarray(inputs["w_o"], np.float32)
    b_q = np.asarray(inputs["b_q"], np.float32)
    b_k = np.asarray(inputs["b_k"], np.float32)
    b_v = np.asarray(inputs["b_v"], np.float32)
    b_o = np.asarray(inputs["b_o"], np.float32)
    k_index = int(np.asarray(inputs["k_index"]))
    assert 1 <= k_index <= 8, f"kernel supports k_index<=8, got {k_index}"

    # fold the 1/sqrt(DK) score scaling into the q projection (exact: 2^-3)
    scale = np.float32(1.0 / math.sqrt(DK))
    w_qs = (w_q * scale).astype(np.float32)
    b_qs = (b_q * scale).astype(np.float32)

    has_bias = {
        "bq": bool(np.any(b_qs)),
        "bk": bool(np.any(b_k)),
        "bv": bool(np.any(b_v)),
        "bo": bool(np.any(b_o)),
    }

    nc = _build_program(k_index, has_bias)
    global _last_nc
    _last_nc = nc

    def _wide_w(w16):
        # [D, D] -> [128, FT*D]: ft-blocks of 128 rows laid side by side
        return np.ascontiguousarray(
            w16.reshape(FT, 128, D).transpose(1, 0, 2).reshape(128, FT * D))

    def _wide_x(x16):
        # [B', D, S] -> [B', 128, FT*S]
        bb = x16.shape[0]
        return np.ascontiguousarray(
            x16.reshape(bb, FT, 128, S).transpose(0, 2, 1, 3)
            .reshape(bb, 128, FT * S))

    wqhi, wqlo = _split16(w_qs)
    wkhi, wklo = _split16(w_k)
    shared = {
        "wqhi": _wide_w(wqhi),
        "wqlo": _wide_w(wqlo),
        "wkhi": _wide_w(wkhi),
        "wklo": _wide_w(wklo),
        "wv": _wide_w(w_v.astype(np.float16)),
        "wo": _wide_w(w_o.astype(np.float16)),
    }
    for nm, arr in (("bq", b_qs), ("bk", b_k), ("bv", b_v), ("bo", b_o)):
        if has_bias[nm]:
            shared[nm] = np.ascontiguousarray(arr.reshape(1, D).astype(np.float32))

    qT = q.transpose(0, 2, 1)
    kT = k.transpose(0, 2, 1)
    vTf = v.transpose(0, 2, 1).astype(np.float16)
    qhiT, qloT = _split16(qT)
    khiT, kloT = _split16(kT)

    in_maps = []
    for c in range(NCORES):
        sl = slice(c * BC, (c + 1) * BC)
        in_maps.append(dict(
            shared,
            qhiT=_wide_x(qhiT[sl]),
            qloT=_wide_x(qloT[sl]),
            khiT=_wide_x(khiT[sl]),
            kloT=_wide_x(kloT[sl]),
            vT=_wide_x(vTf[sl]),
        ))

    res = run_bass_kernel_spmd(
        nc, in_maps, core_ids=list(range(NCORES)), trace=CFG["trace"]
    )
    out = np.concatenate([r["out"] for r in res.results], axis=0)
    kernel.last_result = res
    return out


# revision 56
# speedup vs baseline: 1.1385x; 1.0112x over previous
"""Trainium2 Bass kernel for sparse (top-k) multi-head causal attention.

Problem (hardcoded shapes, from the reference):
  B=32, S=512, D=512, H=8, DK=64, k_index=5 (any k<=8 supported)
  out = TopKCausalAttention(q, k, v; w_q..w_o, b_q..b_o)

Sharding: data-parallel over batch across 8 NeuronCores (4 batches/core).

Numerics: the top-k selection is discontinuous, so scores need ~2^-16
relative accuracy vs the fp32 reference.  fp32 matmuls run at 4 cyc/row
on the PE; instead the q/k path uses f16 hi/lo PAIR arithmetic (3
matmuls at 1 cyc/row, ~2^-22 effective):
  q = qhi + qlo (host-split f16), w_q = whi + wlo (host-split f16)
  qh = qhi*whi + qhi*wlo + qlo*whi        (dropped qlo*wlo ~ 2^-22)
  qh -> (hi, lo) f16 evac split; scores = qhh*khh + qhh*khl + qhl*khh
Measured end-to-end rel err vs fp32 reference: ~2.7e-3 (gate 2e-2).
(float32r at 1 cyc/row was measured: its DMA/weight path quantizes to
11 mantissa bits -> rel err 1.6e-2, too close to the gate; and
engine-written f32r tiles load garbage as PE weights.)

Per-core algorithm (per batch b, head pair hp, heads hh=0,1):
  scores_psum[r-tile, 0:w] = 3 pair matmuls per head (+ bf16
      identity-matmul adds the strictly-causal -1e32 mask on the
      diagonal tile; upper tiles skipped)
  e = exp(scores)                 (ACT, PSUM->SBUF, accum Z at ri=0)
  top8 = vector.max(e)            (top-8 per row, one DVE op)
  tau = top8[:, k-1]; rows < k get tau := 0; Z = sum(top8[:, :k]) or
      full-row sum for rows < k; row 0: Z := 1
  pu = (e >= tau) * e             (DVE stt, f16 out; exact-by-value
                                   threshold, matching reference)
  R[ri] = diag(1/Z)               (f16, tensor_scalar identity * rz)
  ptb[c, r] = pu[r, c]^T @ R      (regular PE matmul: transpose AND
                                   1/Z normalization in one 1cyc/row op)
  attnT[d, r] += vh_ci^T @ ptrow_ci   (f16, triangular)
  y[r, :] = sum_hp attnT^T @ w_o (+ b_o) -> DRAM

Scheduling (vs the per-instruction cost model): PE is the bottleneck
(~202.5us busy of ~249us total).  Batch b+1's projections are emitted
between hp1/hp2 of batch b so their PSUM evacuations (ACT hi-copy +
DVE lo-subtract; GPSIMD cannot touch PSUM) never queue behind head-pair
DVE chains.  The last batch's head pairs interleave into batch BC-2's
stream, and its pt/y evacuations move ACT->DVE, to shorten the
pipeline-drain tail.  Batch-0 q/wq loads are issued in halves so the
first projection starts before the full 2MB lands.
"""

import math
import os

os.environ.setdefault("MYCRO_LOCAL_CACHE", "1")

from contextlib import ExitStack

import numpy as np

import concourse.bass as bass
import concourse.bacc as bacc
import concourse.mybir as mybir
import concourse.tile as tile
from concourse.bass_utils import run_bass_kernel_spmd

B, S, D, H = 32, 512, 512, 8
DK = D // H  # 64
NCORES = 8
BC = B // NCORES  # batches per core
RT = S // 128  # row tiles per sequence
FT = D // 128  # feature tiles
NEG = -1.0e32

F32 = mybir.dt.float32
BF16 = mybir.dt.bfloat16
F16 = mybir.dt.float16

_last_nc = None

CFG = {
    "trace": False,
    "mask_on_pe": True,   # bf16 identity-matmul mask vs DVE tensor add
}


def _build_program(k_index: int, has_bias: dict):
    """Builds the per-core Bass program."""
    nc = bacc.Bacc(
        "TRN2", target_bir_lowering=False, debug=False, num_devices=NCORES
    )

    # --- DRAM I/O -------------------------------------------------------
    # q/k in transposed layout, host-split into f16 hi/lo pairs and
    # host-pre-arranged as [128, FT*S] (ft-blocks side by side) so each
    # tensor loads with ONE wide DMA instead of FT strided ones.
    qhiT = nc.dram_tensor("qhiT", (BC, 128, FT * S), F16, kind="ExternalInput").ap()
    qloT = nc.dram_tensor("qloT", (BC, 128, FT * S), F16, kind="ExternalInput").ap()
    khiT = nc.dram_tensor("khiT", (BC, 128, FT * S), F16, kind="ExternalInput").ap()
    kloT = nc.dram_tensor("kloT", (BC, 128, FT * S), F16, kind="ExternalInput").ap()
    vT = nc.dram_tensor("vT", (BC, 128, FT * S), F16, kind="ExternalInput").ap()
    wqhi = nc.dram_tensor("wqhi", (128, FT * D), F16, kind="ExternalInput").ap()
    wqlo = nc.dram_tensor("wqlo", (128, FT * D), F16, kind="ExternalInput").ap()
    wkhi = nc.dram_tensor("wkhi", (128, FT * D), F16, kind="ExternalInput").ap()
    wklo = nc.dram_tensor("wklo", (128, FT * D), F16, kind="ExternalInput").ap()
    wv = nc.dram_tensor("wv", (128, FT * D), F16, kind="ExternalInput").ap()
    wo = nc.dram_tensor("wo", (128, FT * D), F16, kind="ExternalInput").ap()
    bias_aps = {}
    for name in ("bq", "bk", "bv", "bo"):
        if has_bias[name]:
            bias_aps[name] = nc.dram_tensor(
                name, (1, D), F32, kind="ExternalInput"
            ).ap()
    out = nc.dram_tensor("out", (BC, S, D), F32, kind="ExternalOutput").ap()

    # --- inline constants ----------------------------------------------
    ident_np = np.eye(128, dtype=np.float32)
    mask_np = np.where(
        np.arange(128)[None, :] >= np.arange(128)[:, None], NEG, 0.0
    ).astype(np.float32)
    ident_p = nc.inline_tensor(
        ident_np.astype(mybir.dt.np(F16)), name="identp"
    ).ap()
    ident_b = nc.inline_tensor(
        ident_np.astype(mybir.dt.np(BF16)), name="identb"
    ).ap()
    maskT_b = nc.inline_tensor(
        mask_np.T.copy().astype(mybir.dt.np(BF16)), name="maskT"
    ).ap()
    ones_row = nc.inline_tensor(
        np.ones((1, S), dtype=np.float32), name="onesrow"
    ).ap()

    with tile.TileContext(nc) as tc, ExitStack() as ctx:
        # ---------------- pools ----------------
        consts = ctx.enter_context(tc.tile_pool(name="consts", bufs=1))
        xpool = ctx.enter_context(tc.tile_pool(name="xpool", bufs=2))
        projpool = ctx.enter_context(tc.tile_pool(name="projpool", bufs=2))
        epool = ctx.enter_context(tc.tile_pool(name="epool", bufs=20))
        pnpool = ctx.enter_context(tc.tile_pool(name="pnpool", bufs=12))
        rpool = ctx.enter_context(tc.tile_pool(name="rpool", bufs=10))
        ptpool = ctx.enter_context(tc.tile_pool(name="ptpool", bufs=12))
        smallpool = ctx.enter_context(tc.tile_pool(name="smallpool", bufs=4))
        atpool = ctx.enter_context(tc.tile_pool(name="atpool", bufs=3))
        ypool = ctx.enter_context(tc.tile_pool(name="ypool", bufs=3))

        ps_proj = ctx.enter_context(tc.tile_pool(name="ps_proj", bufs=2, space="PSUM"))
        ps_sc = ctx.enter_context(tc.tile_pool(name="ps_sc", bufs=3, space="PSUM"))
        ps_pt = ctx.enter_context(tc.tile_pool(name="ps_pt", bufs=1, space="PSUM"))
        ps_at = ctx.enter_context(tc.tile_pool(name="ps_at", bufs=1, space="PSUM"))
        ps_y = ctx.enter_context(tc.tile_pool(name="ps_y", bufs=1, space="PSUM"))

        # ---------------- resident constants ----------------
        # combined [128, FT*S] tiles: one wide DMA per tensor; q weights +
        # batch 0's q first so the first projection matmuls start earliest.
        HW = FT * D // 2
        wqh_sb = consts.tile([128, FT * D], F16, name="wqh")
        nc.sync.dma_start(wqh_sb[:, 0:HW], wqhi[:, 0:HW])
        _xq0h = xpool.tile([128, FT * S], F16, name="xqh", tag="xqh")
        nc.sync.dma_start(_xq0h[:, 0:HW], qhiT[0, :, 0:HW])
        wql_sb = consts.tile([128, FT * D], F16, name="wql")
        nc.sync.dma_start(wql_sb[:, 0:HW], wqlo[:, 0:HW])
        _xq0l = xpool.tile([128, FT * S], F16, name="xql", tag="xql")
        nc.sync.dma_start(_xq0l[:, 0:HW], qloT[0, :, 0:HW])
        nc.sync.dma_start(wqh_sb[:, HW:], wqhi[:, HW:])
        nc.sync.dma_start(_xq0h[:, HW:], qhiT[0, :, HW:])
        nc.sync.dma_start(wql_sb[:, HW:], wqlo[:, HW:])
        nc.sync.dma_start(_xq0l[:, HW:], qloT[0, :, HW:])
        _xq0 = (_xq0h, _xq0l)
        wkh_sb = consts.tile_from(wkhi, name="wkh")
        _xk0h = xpool.tile_from(khiT[0], name="xkh")
        wkl_sb = consts.tile_from(wklo, name="wkl")
        preloaded = {}
        preloaded[0] = (
            _xq0,
            (_xk0h, xpool.tile_from(kloT[0], name="xkl")),
            xpool.tile_from(vT[0], name="xv"),
        )
        wv_sb = consts.tile_from(wv, name="wv")
        wo_sb = consts.tile_from(wo, name="wo")
        identp_sb = consts.tile_from(ident_p, name="identp_sb")
        identb_sb = consts.tile_from(ident_b, name="identb_sb")
        maskT_sb = consts.tile_from(maskT_b, name="maskT_sb")
        ones_sb = consts.tile_from(ones_row, name="ones_sb")
        bias_sb = {
            nm: consts.tile_from(ap, name=f"{nm}_sb") for nm, ap in bias_aps.items()
        }

        Exp = mybir.ActivationFunctionType.Exp
        AO = mybir.AluOpType

        def emit_proj(b, defer_v=False):
            """Loads + q/k/v projections for batch b.

            q/k: f16 pair-product accumulation (12 matmuls per output
            tile), evacuated as an f16 hi/lo split: hi via ACT copy,
            lo = psum - hi via DVE/Pool tensor_tensor subtract.
            """
            if b in preloaded:
                (xqh, xql), (xkh, xkl), xv = preloaded.pop(b)
            else:
                xqh = xpool.tile_from(qhiT[b], name="xqh")
                xql = xpool.tile_from(qloT[b], name="xql")
                xkh = xpool.tile_from(khiT[b], name="xkh")
                xkl = xpool.tile_from(kloT[b], name="xkl")
                xv = xpool.tile_from(vT[b], name="xv")
            qhT, khT, vh = [], [], []  # qhT/khT: list of (hi, lo) per dt
            for dt in range(FT):
                for which, whi_sb, wlo_sb, xh, xl, bkey, outl in (
                        ("q", wqh_sb, wql_sb, xqh, xql, "bq", qhT),
                        ("k", wkh_sb, wkl_sb, xkh, xkl, "bk", khT)):
                    ps = ps_proj.tile([128, S], F32, name="psq", tag="psproj")
                    nbias = bkey in bias_sb
                    nmm = 3 * FT
                    i = 0
                    for ft in range(FT):
                        wsl = slice(ft * D + dt * 128, ft * D + (dt + 1) * 128)
                        xsl = slice(ft * S, (ft + 1) * S)
                        for w_sb, xs in ((whi_sb, xh), (wlo_sb, xh),
                                         (whi_sb, xl)):
                            i += 1
                            nc.tensor.matmul(
                                ps, w_sb[:, wsl], xs[:, xsl],
                                start=(i == 1),
                                stop=(i == nmm and not nbias))
                    if nbias:
                        nc.tensor.matmul(
                            ps, bias_sb[bkey][0:1, dt * 128:(dt + 1) * 128],
                            ones_sb, start=False, stop=True)
                    thi = projpool.tile([128, S], F16, name=f"{which}hT{dt}h",
                                        tag=f"{which}hT{dt}h")
                    nc.scalar.copy(thi, ps)
                    tlo = projpool.tile([128, S], F16, name=f"{which}hT{dt}l",
                                        tag=f"{which}hT{dt}l")
                    # GPSIMD cannot access PSUM (walrus constraint): the
                    # latency-critical lo evac goes on DVE
                    nc.vector.tensor_tensor(tlo, ps, thi, op=AO.subtract)
                    outl.append((thi, tlo))

            def do_vproj(rts=range(RT)):
                for rt in rts:
                    ps = ps_proj.tile([128, D], F32, name="psv", tag="psproj")
                    nbias = "bv" in bias_sb
                    for ft in range(FT):
                        nc.tensor.matmul(
                            ps, xv[:, ft * S + rt * 128:ft * S + (rt + 1) * 128],
                            wv_sb[:, ft * D:(ft + 1) * D],
                            start=(ft == 0), stop=(ft == FT - 1 and not nbias))
                    if nbias:
                        nc.tensor.matmul(
                            ps, ones_sb[0:1, 0:128], bias_sb["bv"],
                            start=False, stop=True)
                    t = projpool.tile([128, D], F16, name=f"vh{rt}", tag=f"vh{rt}")
                    nc.scalar.copy(t, ps)
                    vh.append(t)
                return vh
            if defer_v:
                return qhT, khT, do_vproj
            return qhT, khT, do_vproj()

        def emit_headpair(hp, qhT, khT, vh, pt_dve=False, at_dve=False):
            """Scores / top-k softmax / normalized transpose / attnT for one
            head pair (partition halves 0:64 / 64:128 of the proj tiles)."""
            etiles = [[None] * RT, [None] * RT]
            zfulls = [None, None]
            top8s = []
            for hh in range(2):
                top8s.append(smallpool.tile(
                    [128, RT * 8], F32, name=f"top8{hh}", tag=f"top8{hh}"))
            qh_hi, qh_lo = qhT[hp]
            kh_hi, kh_lo = khT[hp]
            for ri in range(RT):
                w = (ri + 1) * 128
                spss = []
                for hh in range(2):
                    po = hh * 64
                    sps = ps_sc.tile([128, S], F32, name="sps", tag="sps")
                    rsl = slice(ri * 128, (ri + 1) * 128)
                    for i, (qt, kt) in enumerate((
                            (qh_hi, kh_hi), (qh_hi, kh_lo), (qh_lo, kh_hi))):
                        nc.tensor.matmul(
                            sps[:, 0:w],
                            qt[po:po + 64, rsl],
                            kt[po:po + 64, 0:w],
                            start=(i == 0), stop=False)
                    spss.append(sps)
                for hh in range(2):
                    nc.tensor.matmul(
                        spss[hh][:, ri * 128:(ri + 1) * 128],
                        maskT_sb, identb_sb, start=False, stop=True)
                for hh in range(2):
                    e = epool.tile([128, S], F32, name="e", tag="e")
                    if ri == 0:
                        zf = smallpool.tile(
                            [128, 1], F32, name=f"zfull{hh}", tag=f"zfull{hh}")
                        zfulls[hh] = zf
                        nc.scalar.activation(
                            e[:, 0:w], spss[hh][:, 0:w], Exp, accum_out=zf)
                    else:
                        nc.scalar.activation(e[:, 0:w], spss[hh][:, 0:w], Exp)
                    nc.vector.max(
                        out=top8s[hh][:, ri * 8:(ri + 1) * 8], in_=e[:, 0:w])
                    etiles[hh][ri] = e
            ptrows = [[None] * RT, [None] * RT]
            for hh in range(2):
                top8 = top8s[hh]
                zk = smallpool.tile([128, RT], F32, name="zk", tag="zk")
                nc.vector.reduce_sum(
                    zk, top8.rearrange("p (r e) -> p r e", e=8)[:, :, 0:k_index],
                    axis=mybir.AxisListType.X)
                nc.vector.tensor_copy(zk[0:k_index, 0:1], zfulls[hh][0:k_index, :])
                nc.vector.memset(zk[0:1, 0:1], 1.0)
                # rows < k keep every valid entry: tau := 0
                nc.vector.memset(top8[0:k_index, k_index - 1:k_index], 0.0)
                rz = smallpool.tile([128, RT], F32, name="rz", tag="rz")
                nc.vector.reciprocal(rz, zk)

                # R[ri] = diag(rz[:, ri]) in f16: ACT copy-with-scale of
                # the identity
                rtiles = []
                for ri in range(RT):
                    R = rpool.tile([128, 128], F16, name="rdiag", tag="rdiag")
                    nc.gpsimd.tensor_scalar(
                        R, identp_sb, rz[:, ri:ri + 1], None, op0=AO.mult)
                    rtiles.append(R)

                # masked (unnormalized) probs, f16
                pns = []
                for ri in range(RT):
                    w = (ri + 1) * 128
                    e = etiles[hh][ri]
                    tau = top8[:, ri * 8 + k_index - 1: ri * 8 + k_index]
                    pn = pnpool.tile([128, S], F16, name="pn", tag="pn")
                    nc.vector.scalar_tensor_tensor(
                        pn[:, 0:w], e[:, 0:w], tau, e[:, 0:w],
                        op0=AO.is_ge, op1=AO.mult)
                    pns.append(pn)
                # normalized transpose: ptb[c, r-block] = pn[r-block, c]^T
                # @ diag(rz) -- regular matmul, transpose + 1/Z in one op
                for ci in range(RT):
                    wv_ = (RT - ci) * 128
                    ptb = ps_pt.tile([128, S], F32, name="ptb", tag="ptb")
                    for ri in range(ci, RT):
                        nc.tensor.matmul(
                            ptb[:, (ri - ci) * 128:(ri - ci + 1) * 128],
                            pns[ri][:, ci * 128:(ci + 1) * 128],
                            rtiles[ri], start=True, stop=True)
                    ptrow = ptpool.tile([128, S], F16, name="ptrow", tag="ptrow")
                    if pt_dve:
                        nc.vector.tensor_copy(ptrow[:, 0:wv_], ptb[:, 0:wv_])
                    else:
                        nc.scalar.copy(ptrow[:, 0:wv_], ptb[:, 0:wv_])
                    ptrows[hh][ci] = ptrow

            def finish(vh):
                at_ps = ps_at.tile([128, S], F32, name="atps", tag="atps")
                for ci in range(RT):
                    wv_ = (RT - ci) * 128
                    for hh in range(2):
                        h = 2 * hp + hh
                        po = hh * 64
                        nc.tensor.matmul(
                            at_ps[po:po + 64, ci * 128:S],
                            vh[ci][:, h * DK:(h + 1) * DK],
                            ptrows[hh][ci][:, 0:wv_],
                            start=(ci == 0), stop=(ci == RT - 1),
                            skip_group_check=True)
                at = atpool.tile([128, S], F16, name=f"at{hp}", tag=f"at{hp}")
                if at_dve:
                    nc.vector.tensor_copy(at, at_ps)
                else:
                    nc.scalar.copy(at, at_ps)
                return at
            if vh is None:
                return finish
            return finish(vh)

        def emit_y(b, attnT_sb, y_dve=False):
            for ri in range(RT):
                yps = ps_y.tile([128, D], F32, name="yps", tag="yps")
                nbias = "bo" in bias_sb
                for hp in range(FT):
                    nc.tensor.matmul(
                        yps, attnT_sb[hp][:, ri * 128:(ri + 1) * 128],
                        wo_sb[:, hp * D:(hp + 1) * D],
                        start=(hp == 0), stop=(hp == FT - 1 and not nbias))
                if nbias:
                    nc.tensor.matmul(
                        yps, ones_sb[0:1, 0:128], bias_sb["bo"],
                        start=False, stop=True)
                y = ypool.tile([128, D], F32, name="y", tag="y")
                if y_dve:
                    nc.vector.tensor_copy(y, yps)
                else:
                    nc.scalar.copy(y, yps)
                nc.scalar.dma_start(out[b, ri * 128:(ri + 1) * 128, :], y)

        # proj for batch b+1 is emitted between hp1 and hp2 of batch b
        # (latency-critical DVE lo-subtracts enqueue ahead of later head
        # pairs' DVE chains).  The LAST batch's head pairs are interleaved
        # into batch BC-2's stream so only two chains drain at the tail.
        projs = {0: emit_proj(0)}
        ats = {b: [] for b in range(BC)}
        for b in range(BC - 1):
            qhT, khT, vh = projs.pop(b)
            if b < BC - 2:
                for hp in range(FT):
                    ats[b].append(emit_headpair(hp, qhT, khT, vh))
                    if hp == 1:
                        projs[b + 1] = emit_proj(b + 1)
                emit_y(b, ats[b])
            else:
                # interleave tail: b2.hp0 b2.hp1 [proj3] b2.hp2 b3.hp0
                # b2.hp3 b3.hp1 y2 b3.hp2 b3.hp3 y3
                ats[b].append(emit_headpair(0, qhT, khT, vh))
                ats[b].append(emit_headpair(1, qhT, khT, vh))
                projs[b + 1] = emit_proj(b + 1)
                qhT3, khT3, vh3 = projs.pop(b + 1)
                ats[b].append(emit_headpair(2, qhT, khT, vh))
                ats[b + 1].append(emit_headpair(0, qhT3, khT3, vh3))
                ats[b].append(emit_headpair(3, qhT, khT, vh))
                ats[b + 1].append(emit_headpair(1, qhT3, khT3, vh3, pt_dve=True))
                emit_y(b, ats[b])
                ats[b + 1].append(emit_headpair(2, qhT3, khT3, vh3,
                                                 pt_dve=True, at_dve=True))
                ats[b + 1].append(emit_headpair(3, qhT3, khT3, vh3,
                                                 pt_dve=True, at_dve=True))
                emit_y(b + 1, ats[b + 1], y_dve=True)

    nc.compile()
    return nc


def _split16(x):
    """Split fp32 array into (hi, lo) f16 pair with hi + lo ~= x."""
    hi = x.astype(np.float16)
    lo = (x - hi.astype(np.float32)).astype(np.float16)
    return hi, lo


def kernel(**inputs):
    q = np.asarray(inputs["q"], np.float32)
    k = np.asarray(inputs["k"], np.float32)
    v = np.asarray(inputs["v"], np.float32)
    w_q = np.asarray(inputs["w_q"], np.float32)
    w_k = np.asarray(inputs["w_k"], np.float32)
    w_v = np.asarray(inputs["w_v"], np.float32)
    w_o = np.asarray(inputs["w_o"], np.float32)
    b_q = np.asarray(inputs["b_q"], np.float32)
    b_k = np.asarray(inputs["b_k"], np.float32)
    b_v = np.asarray(inputs["b_v"], np.float32)
    b_o = np.asarray(inputs["b_o"], np.float32)
    k_index = int(np.asarray(inputs["k_index"]))
    assert 1 <= k_index <= 8, f"kernel supports k_index<=8, got {k_index}"

    # fold the 1/sqrt(DK) score scaling into the q projection (exact: 2^-3)
    scale = np.float32(1.0 / math.sqrt(DK))
    w_qs = (w_q * scale).astype(np.float32)
    b_qs = (b_q * scale).astype(np.float32)

    has_bias = {
        "bq": bool(np.any(b_qs)),
        "bk": bool(np.any(b_k)),
        "bv": bool(np.any(b_v)),
        "bo": bool(np.any(b_o)),
    }

    nc = _build_program(k_index, has_bias)
    global _last_nc
    _last_nc = nc

    def _wide_w(w16):
        # [D, D] -> [128, FT*D]: ft-blocks of 128 rows laid side by side
        return np.ascontiguousarray(
            w16.reshape(FT, 128, D).transpose(1, 0, 2).reshape(128, FT * D))

    def _wide_x(x16):
        # [B', D, S] -> [B', 128, FT*S]
        bb = x16.shape[0]
        return np.ascontiguousarray(
            x16.reshape(bb, FT, 128, S).transpose(0, 2, 1, 3)
            .reshape(bb, 128, FT * S))

    wqhi, wqlo = _split16(w_qs)
    wkhi, wklo = _split16(w_k)
    shared = {
        "wqhi": _wide_w(wqhi),
        "wqlo": _wide_w(wqlo),
        "wkhi": _wide_w(wkhi),
        "wklo": _wide_w(wklo),
        "wv": _wide_w(w_v.astype(np.float16)),
        "wo": _wide_w(w_o.astype(np.float16)),
    }
    for nm, arr in (("bq", b_qs), ("bk", b_k), ("bv", b_v), ("bo", b_o)):
        if has_bias[nm]:
            shared[nm] = np.ascontiguousarray(arr.reshape(1, D).astype(np.float32))

    qT = q.transpose(0, 2, 1)
    kT = k.transpose(0, 2, 1)
    vTf = v.transpose(0, 2, 1).astype(np.float16)
    qhiT, qloT = _split16(qT)
    khiT, kloT = _split16(kT)

    in_maps = []
    for c in range(NCORES):
        sl = slice(c * BC, (c + 1) * BC)
        in_maps.append(dict(
            shared,
            qhiT=_wide_x(qhiT[sl]),
            qloT=_wide_x(qloT[sl]),
            khiT=_wide_x(khiT[sl]),
            kloT=_wide_x(kloT[sl]),
            vT=_wide_x(vTf[sl]),
        ))

    res = run_bass_kernel_spmd(
        nc, in_maps, core_ids=list(range(NCORES)), trace=CFG["trace"]
    )
    out = np.concatenate([r["out"] for r in res.results], axis=0)
    kernel.last_result = res
    return out


# revision 61
# speedup vs baseline: 1.1409x; 1.0021x over previous
"""Trainium2 Bass kernel for sparse (top-k) multi-head causal attention.

Problem (hardcoded shapes, from the reference):
  B=32, S=512, D=512, H=8, DK=64, k_index=5 (any k<=8 supported)
  out = TopKCausalAttention(q, k, v; w_q..w_o, b_q..b_o)

Sharding: data-parallel over batch across 8 NeuronCores (4 batches/core).

Numerics: the top-k selection is discontinuous, so scores need ~2^-16
relative accuracy vs the fp32 reference.  fp32 matmuls run at 4 cyc/row
on the PE; instead the q/k path uses f16 hi/lo PAIR arithmetic (3
matmuls at 1 cyc/row, ~2^-22 effective):
  q = qhi + qlo (host-split f16), w_q = whi + wlo (host-split f16)
  qh = qhi*whi + qhi*wlo + qlo*whi        (dropped qlo*wlo ~ 2^-22)
  qh -> (hi, lo) f16 evac split; scores = qhh*khh + qhh*khl + qhl*khh
Measured end-to-end rel err vs fp32 reference: ~2.7e-3 (gate 2e-2).
(float32r at 1 cyc/row was measured: its DMA/weight path quantizes to
11 mantissa bits -> rel err 1.6e-2, too close to the gate; and
engine-written f32r tiles load garbage as PE weights.)

Per-core algorithm (per batch b, head pair hp, heads hh=0,1):
  scores_psum[r-tile, 0:w] = 3 pair matmuls per head (+ bf16
      identity-matmul adds the strictly-causal -1e32 mask on the
      diagonal tile; upper tiles skipped)
  e = exp(scores)                 (ACT, PSUM->SBUF, accum Z at ri=0)
  top8 = vector.max(e)            (top-8 per row, one DVE op)
  tau = top8[:, k-1]; rows < k get tau := 0; Z = sum(top8[:, :k]) or
      full-row sum for rows < k; row 0: Z := 1
  pu = (e >= tau) * e             (DVE stt, f16 out; exact-by-value
                                   threshold, matching reference)
  R[ri] = diag(1/Z)               (f16, tensor_scalar identity * rz)
  ptb[c, r] = pu[r, c]^T @ R      (regular PE matmul: transpose AND
                                   1/Z normalization in one 1cyc/row op)
  attnT[d, r] += vh_ci^T @ ptrow_ci   (f16, triangular)
  y[r, :] = sum_hp attnT^T @ w_o (+ b_o) -> DRAM

Scheduling (vs the per-instruction cost model): PE is the bottleneck
(~202.5us busy of ~249us total).  Batch b+1's projections are emitted
between hp1/hp2 of batch b so their PSUM evacuations (ACT hi-copy +
DVE lo-subtract; GPSIMD cannot touch PSUM) never queue behind head-pair
DVE chains.  The last batch's head pairs interleave into batch BC-2's
stream, and its pt/y evacuations move ACT->DVE, to shorten the
pipeline-drain tail.  Batch-0 q/wq loads are issued in halves so the
first projection starts before the full 2MB lands.
"""

import math
import os

os.environ.setdefault("MYCRO_LOCAL_CACHE", "1")

from contextlib import ExitStack

import numpy as np

import concourse.bass as bass
import concourse.bacc as bacc
import concourse.mybir as mybir
import concourse.tile as tile
from concourse.bass_utils import run_bass_kernel_spmd

B, S, D, H = 32, 512, 512, 8
DK = D // H  # 64
NCORES = 8
BC = B // NCORES  # batches per core
RT = S // 128  # row tiles per sequence
FT = D // 128  # feature tiles
NEG = -1.0e32

F32 = mybir.dt.float32
BF16 = mybir.dt.bfloat16
F16 = mybir.dt.float16

_last_nc = None

CFG = {
    "trace": False,
    "mask_on_pe": True,   # bf16 identity-matmul mask vs DVE tensor add
}


def _build_program(k_index: int, has_bias: dict):
    """Builds the per-core Bass program."""
    nc = bacc.Bacc(
        "TRN2", target_bir_lowering=False, debug=False, num_devices=NCORES
    )

    # --- DRAM I/O -------------------------------------------------------
    # q/k in transposed layout, host-split into f16 hi/lo pairs and
    # host-pre-arranged as [128, FT*S] (ft-blocks side by side) so each
    # tensor loads with ONE wide DMA instead of FT strided ones.
    qhiT = nc.dram_tensor("qhiT", (BC, 128, FT * S), F16, kind="ExternalInput").ap()
    qloT = nc.dram_tensor("qloT", (BC, 128, FT * S), F16, kind="ExternalInput").ap()
    khiT = nc.dram_tensor("khiT", (BC, 128, FT * S), F16, kind="ExternalInput").ap()
    kloT = nc.dram_tensor("kloT", (BC, 128, FT * S), F16, kind="ExternalInput").ap()
    vT = nc.dram_tensor("vT", (BC, 128, FT * S), F16, kind="ExternalInput").ap()
    wqhi = nc.dram_tensor("wqhi", (128, FT * D), F16, kind="ExternalInput").ap()
    wqlo = nc.dram_tensor("wqlo", (128, FT * D), F16, kind="ExternalInput").ap()
    wkhi = nc.dram_tensor("wkhi", (128, FT * D), F16, kind="ExternalInput").ap()
    wklo = nc.dram_tensor("wklo", (128, FT * D), F16, kind="ExternalInput").ap()
    wv = nc.dram_tensor("wv", (128, FT * D), F16, kind="ExternalInput").ap()
    wo = nc.dram_tensor("wo", (128, FT * D), F16, kind="ExternalInput").ap()
    bias_aps = {}
    for name in ("bq", "bk", "bv", "bo"):
        if has_bias[name]:
            bias_aps[name] = nc.dram_tensor(
                name, (1, D), F32, kind="ExternalInput"
            ).ap()
    out = nc.dram_tensor("out", (BC, S, D), F32, kind="ExternalOutput").ap()

    # --- inline constants ----------------------------------------------
    ident_np = np.eye(128, dtype=np.float32)
    ident_p = nc.inline_tensor(
        ident_np.astype(mybir.dt.np(F16)), name="identp"
    ).ap()
    lt_np = (np.arange(128)[None, :] < np.arange(128)[:, None]).astype(
        mybir.dt.np(F16))
    lt_tri = nc.inline_tensor(lt_np, name="lttri").ap()
    ones_row = nc.inline_tensor(
        np.ones((1, S), dtype=np.float32), name="onesrow"
    ).ap()

    with tile.TileContext(nc) as tc, ExitStack() as ctx:
        # ---------------- pools ----------------
        consts = ctx.enter_context(tc.tile_pool(name="consts", bufs=1))
        xpool = ctx.enter_context(tc.tile_pool(name="xpool", bufs=2))
        projpool = ctx.enter_context(tc.tile_pool(name="projpool", bufs=2))
        epool = ctx.enter_context(tc.tile_pool(name="epool", bufs=20))
        pnpool = ctx.enter_context(tc.tile_pool(name="pnpool", bufs=12))
        rpool = ctx.enter_context(tc.tile_pool(name="rpool", bufs=10))
        ptpool = ctx.enter_context(tc.tile_pool(name="ptpool", bufs=12))
        smallpool = ctx.enter_context(tc.tile_pool(name="smallpool", bufs=4))
        atpool = ctx.enter_context(tc.tile_pool(name="atpool", bufs=3))
        ypool = ctx.enter_context(tc.tile_pool(name="ypool", bufs=3))

        ps_proj = ctx.enter_context(tc.tile_pool(name="ps_proj", bufs=2, space="PSUM"))
        ps_sc = ctx.enter_context(tc.tile_pool(name="ps_sc", bufs=3, space="PSUM"))
        ps_pt = ctx.enter_context(tc.tile_pool(name="ps_pt", bufs=1, space="PSUM"))
        ps_at = ctx.enter_context(tc.tile_pool(name="ps_at", bufs=1, space="PSUM"))
        ps_y = ctx.enter_context(tc.tile_pool(name="ps_y", bufs=1, space="PSUM"))

        # ---------------- resident constants ----------------
        # combined [128, FT*S] tiles: one wide DMA per tensor; q weights +
        # batch 0's q first so the first projection matmuls start earliest.
        HW = FT * D // 2
        wqh_sb = consts.tile([128, FT * D], F16, name="wqh")
        nc.sync.dma_start(wqh_sb[:, 0:HW], wqhi[:, 0:HW])
        _xq0h = xpool.tile([128, FT * S], F16, name="xqh", tag="xqh")
        nc.sync.dma_start(_xq0h[:, 0:HW], qhiT[0, :, 0:HW])
        wql_sb = consts.tile([128, FT * D], F16, name="wql")
        nc.sync.dma_start(wql_sb[:, 0:HW], wqlo[:, 0:HW])
        _xq0l = xpool.tile([128, FT * S], F16, name="xql", tag="xql")
        nc.sync.dma_start(_xq0l[:, 0:HW], qloT[0, :, 0:HW])
        nc.sync.dma_start(wqh_sb[:, HW:], wqhi[:, HW:])
        nc.sync.dma_start(_xq0h[:, HW:], qhiT[0, :, HW:])
        nc.sync.dma_start(wql_sb[:, HW:], wqlo[:, HW:])
        nc.sync.dma_start(_xq0l[:, HW:], qloT[0, :, HW:])
        _xq0 = (_xq0h, _xq0l)
        wkh_sb = consts.tile_from(wkhi, name="wkh")
        _xk0h = xpool.tile_from(khiT[0], name="xkh")
        wkl_sb = consts.tile_from(wklo, name="wkl")
        preloaded = {}
        preloaded[0] = (
            _xq0,
            (_xk0h, xpool.tile_from(kloT[0], name="xkl")),
            xpool.tile_from(vT[0], name="xv"),
        )
        wv_sb = consts.tile_from(wv, name="wv")
        wo_sb = consts.tile_from(wo, name="wo")
        identp_sb = consts.tile_from(ident_p, name="identp_sb")
        lt_sb = consts.tile_from(lt_tri, name="lt_sb")
        ones_sb = consts.tile_from(ones_row, name="ones_sb")
        bias_sb = {
            nm: consts.tile_from(ap, name=f"{nm}_sb") for nm, ap in bias_aps.items()
        }

        Exp = mybir.ActivationFunctionType.Exp
        AO = mybir.AluOpType

        def emit_proj(b, defer_v=False):
            """Loads + q/k/v projections for batch b.

            q/k: f16 pair-product accumulation (12 matmuls per output
            tile), evacuated as an f16 hi/lo split: hi via ACT copy,
            lo = psum - hi via DVE/Pool tensor_tensor subtract.
            """
            if b in preloaded:
                (xqh, xql), (xkh, xkl), xv = preloaded.pop(b)
            else:
                xqh = xpool.tile_from(qhiT[b], name="xqh")
                xql = xpool.tile_from(qloT[b], name="xql")
                xkh = xpool.tile_from(khiT[b], name="xkh")
                xkl = xpool.tile_from(kloT[b], name="xkl")
                xv = xpool.tile_from(vT[b], name="xv")
            qhT, khT, vh = [], [], []  # qhT/khT: list of (hi, lo) per dt
            for dt in range(FT):
                for which, whi_sb, wlo_sb, xh, xl, bkey, outl in (
                        ("q", wqh_sb, wql_sb, xqh, xql, "bq", qhT),
                        ("k", wkh_sb, wkl_sb, xkh, xkl, "bk", khT)):
                    ps = ps_proj.tile([128, S], F32, name="psq", tag="psproj")
                    nbias = bkey in bias_sb
                    nmm = 3 * FT
                    i = 0
                    for ft in range(FT):
                        wsl = slice(ft * D + dt * 128, ft * D + (dt + 1) * 128)
                        xsl = slice(ft * S, (ft + 1) * S)
                        for w_sb, xs in ((whi_sb, xh), (wlo_sb, xh),
                                         (whi_sb, xl)):
                            i += 1
                            nc.tensor.matmul(
                                ps, w_sb[:, wsl], xs[:, xsl],
                                start=(i == 1),
                                stop=(i == nmm and not nbias))
                    if nbias:
                        nc.tensor.matmul(
                            ps, bias_sb[bkey][0:1, dt * 128:(dt + 1) * 128],
                            ones_sb, start=False, stop=True)
                    thi = projpool.tile([128, S], F16, name=f"{which}hT{dt}h",
                                        tag=f"{which}hT{dt}h")
                    nc.scalar.copy(thi, ps)
                    tlo = projpool.tile([128, S], F16, name=f"{which}hT{dt}l",
                                        tag=f"{which}hT{dt}l")
                    # GPSIMD cannot access PSUM (walrus constraint): the
                    # latency-critical lo evac goes on DVE
                    nc.vector.tensor_tensor(tlo, ps, thi, op=AO.subtract)
                    outl.append((thi, tlo))

            def do_vproj(rts=range(RT)):
                for rt in rts:
                    ps = ps_proj.tile([128, D], F32, name="psv", tag="psproj")
                    nbias = "bv" in bias_sb
                    for ft in range(FT):
                        nc.tensor.matmul(
                            ps, xv[:, ft * S + rt * 128:ft * S + (rt + 1) * 128],
                            wv_sb[:, ft * D:(ft + 1) * D],
                            start=(ft == 0), stop=(ft == FT - 1 and not nbias))
                    if nbias:
                        nc.tensor.matmul(
                            ps, ones_sb[0:1, 0:128], bias_sb["bv"],
                            start=False, stop=True)
                    t = projpool.tile([128, D], F16, name=f"vh{rt}", tag=f"vh{rt}")
                    nc.scalar.copy(t, ps)
                    vh.append(t)
                return vh
            if defer_v:
                return qhT, khT, do_vproj
            return qhT, khT, do_vproj()

        def emit_headpair(hp, qhT, khT, vh, pt_dve=False, at_dve=False):
            """Scores / top-k softmax / normalized transpose / attnT for one
            head pair (partition halves 0:64 / 64:128 of the proj tiles)."""
            etiles = [[None] * RT, [None] * RT]
            zfulls = [None, None]
            top8s = []
            for hh in range(2):
                top8s.append(smallpool.tile(
                    [128, RT * 8], F32, name=f"top8{hh}", tag=f"top8{hh}"))
            qh_hi, qh_lo = qhT[hp]
            kh_hi, kh_lo = khT[hp]
            for ri in range(RT):
                w = (ri + 1) * 128
                spss = []
                for hh in range(2):
                    po = hh * 64
                    sps = ps_sc.tile([128, S], F32, name="sps", tag="sps")
                    rsl = slice(ri * 128, (ri + 1) * 128)
                    for i, (qt, kt) in enumerate((
                            (qh_hi, kh_hi), (qh_hi, kh_lo), (qh_lo, kh_hi))):
                        nc.tensor.matmul(
                            sps[:, 0:w],
                            qt[po:po + 64, rsl],
                            kt[po:po + 64, 0:w],
                            start=(i == 0), stop=(i == 2))
                    spss.append(sps)
                for hh in range(2):
                    e = epool.tile([128, S], F32, name="e", tag="e")
                    nc.scalar.activation(e[:, 0:w], spss[hh][:, 0:w], Exp)
                    # strict-causal mask applied post-exp on the diagonal
                    # block: e *= LT (0/1) on the otherwise-idle Pool engine
                    # (exp of unmasked scores is finite; x*0 == 0 exactly)
                    nc.gpsimd.tensor_tensor(
                        e[:, ri * 128:(ri + 1) * 128],
                        e[:, ri * 128:(ri + 1) * 128], lt_sb, op=AO.mult)
                    if ri == 0:
                        zf = smallpool.tile(
                            [128, 1], F32, name=f"zfull{hh}", tag=f"zfull{hh}")
                        zfulls[hh] = zf
                        nc.vector.reduce_sum(
                            zf, e[:, 0:w], axis=mybir.AxisListType.X)
                    nc.vector.max(
                        out=top8s[hh][:, ri * 8:(ri + 1) * 8], in_=e[:, 0:w])
                    etiles[hh][ri] = e
            ptrows = [[None] * RT, [None] * RT]
            for hh in range(2):
                top8 = top8s[hh]
                zk = smallpool.tile([128, RT], F32, name="zk", tag="zk")
                nc.vector.reduce_sum(
                    zk, top8.rearrange("p (r e) -> p r e", e=8)[:, :, 0:k_index],
                    axis=mybir.AxisListType.X)
                nc.vector.tensor_copy(zk[0:k_index, 0:1], zfulls[hh][0:k_index, :])
                nc.vector.memset(zk[0:1, 0:1], 1.0)
                # rows < k keep every valid entry: tau := 0
                nc.vector.memset(top8[0:k_index, k_index - 1:k_index], 0.0)
                rz = smallpool.tile([128, RT], F32, name="rz", tag="rz")
                nc.vector.reciprocal(rz, zk)

                # R[ri] = diag(rz[:, ri]) in f16: ACT copy-with-scale of
                # the identity
                rtiles = []
                for ri in range(RT):
                    R = rpool.tile([128, 128], F16, name="rdiag", tag="rdiag")
                    nc.gpsimd.tensor_scalar(
                        R, identp_sb, rz[:, ri:ri + 1], None, op0=AO.mult)
                    rtiles.append(R)

                # masked (unnormalized) probs, f16
                pns = []
                for ri in range(RT):
                    w = (ri + 1) * 128
                    e = etiles[hh][ri]
                    tau = top8[:, ri * 8 + k_index - 1: ri * 8 + k_index]
                    pn = pnpool.tile([128, S], F16, name="pn", tag="pn")
                    nc.vector.scalar_tensor_tensor(
                        pn[:, 0:w], e[:, 0:w], tau, e[:, 0:w],
                        op0=AO.is_ge, op1=AO.mult)
                    pns.append(pn)
                # normalized transpose: ptb[c, r-block] = pn[r-block, c]^T
                # @ diag(rz) -- regular matmul, transpose + 1/Z in one op
                for ci in range(RT):
                    wv_ = (RT - ci) * 128
                    ptb = ps_pt.tile([128, S], F32, name="ptb", tag="ptb")
                    for ri in range(ci, RT):
                        nc.tensor.matmul(
                            ptb[:, (ri - ci) * 128:(ri - ci + 1) * 128],
                            pns[ri][:, ci * 128:(ci + 1) * 128],
                            rtiles[ri], start=True, stop=True)
                    ptrow = ptpool.tile([128, S], F16, name="ptrow", tag="ptrow")
                    if pt_dve:
                        nc.vector.tensor_copy(ptrow[:, 0:wv_], ptb[:, 0:wv_])
                    else:
                        nc.scalar.copy(ptrow[:, 0:wv_], ptb[:, 0:wv_])
                    ptrows[hh][ci] = ptrow

            def finish(vh):
                at_ps = ps_at.tile([128, S], F32, name="atps", tag="atps")
                for ci in range(RT):
                    wv_ = (RT - ci) * 128
                    for hh in range(2):
                        h = 2 * hp + hh
                        po = hh * 64
                        nc.tensor.matmul(
                            at_ps[po:po + 64, ci * 128:S],
                            vh[ci][:, h * DK:(h + 1) * DK],
                            ptrows[hh][ci][:, 0:wv_],
                            start=(ci == 0), stop=(ci == RT - 1),
                            skip_group_check=True)
                at = atpool.tile([128, S], F16, name=f"at{hp}", tag=f"at{hp}")
                if at_dve:
                    nc.vector.tensor_copy(at, at_ps)
                else:
                    nc.scalar.copy(at, at_ps)
                return at
            if vh is None:
                return finish
            return finish(vh)

        def emit_y(b, attnT_sb, y_dve=False):
            for ri in range(RT):
                yps = ps_y.tile([128, D], F32, name="yps", tag="yps")
                nbias = "bo" in bias_sb
                for hp in range(FT):
                    nc.tensor.matmul(
                        yps, attnT_sb[hp][:, ri * 128:(ri + 1) * 128],
                        wo_sb[:, hp * D:(hp + 1) * D],
                        start=(hp == 0), stop=(hp == FT - 1 and not nbias))
                if nbias:
                    nc.tensor.matmul(
                        yps, ones_sb[0:1, 0:128], bias_sb["bo"],
                        start=False, stop=True)
                y = ypool.tile([128, D], F32, name="y", tag="y")
                if y_dve:
                    nc.vector.tensor_copy(y, yps)
                else:
                    nc.scalar.copy(y, yps)
                nc.scalar.dma_start(out[b, ri * 128:(ri + 1) * 128, :], y)

        # proj for batch b+1 is emitted between hp1 and hp2 of batch b
        # (latency-critical DVE lo-subtracts enqueue ahead of later head
        # pairs' DVE chains).  The LAST batch's head pairs are interleaved
        # into batch BC-2's stream so only two chains drain at the tail.
        projs = {0: emit_proj(0)}
        ats = {b: [] for b in range(BC)}
        for b in range(BC - 1):
            qhT, khT, vh = projs.pop(b)
            if b < BC - 2:
                for hp in range(FT):
                    ats[b].append(emit_headpair(hp, qhT, khT, vh))
                    if hp == 1:
                        projs[b + 1] = emit_proj(b + 1)
                emit_y(b, ats[b])
            else:
                # interleave tail: b2.hp0 b2.hp1 [proj3] b2.hp2 b3.hp0
                # b2.hp3 b3.hp1 y2 b3.hp2 b3.hp3 y3
                ats[b].append(emit_headpair(0, qhT, khT, vh))
                ats[b].append(emit_headpair(1, qhT, khT, vh))
                projs[b + 1] = emit_proj(b + 1)
                qhT3, khT3, vh3 = projs.pop(b + 1)
                ats[b].append(emit_headpair(2, qhT, khT, vh))
                ats[b + 1].append(emit_headpair(0, qhT3, khT3, vh3))
                ats[b].append(emit_headpair(3, qhT, khT, vh))
                ats[b + 1].append(emit_headpair(1, qhT3, khT3, vh3, pt_dve=True))
                emit_y(b, ats[b])
                ats[b + 1].append(emit_headpair(2, qhT3, khT3, vh3,
                                                 pt_dve=True, at_dve=True))
                ats[b + 1].append(emit_headpair(3, qhT3, khT3, vh3,
                                                 pt_dve=True, at_dve=True))
                emit_y(b + 1, ats[b + 1], y_dve=True)

    nc.compile()
    return nc


def _split16(x):
    """Split fp32 array into (hi, lo) f16 pair with hi + lo ~= x."""
    hi = x.astype(np.float16)
    lo = (x - hi.astype(np.float32)).astype(np.float16)
    return hi, lo


def kernel(**inputs):
    q = np.asarray(inputs["q"], np.float32)
    k = np.asarray(inputs["k"], np.float32)
    v = np.asarray(inputs["v"], np.float32)
    w_q = np.asarray(inputs["w_q"], np.float32)
    w_k = np.asarray(inputs["w_k"], np.float32)
    w_v = np.asarray(inputs["w_v"], np.float32)
    w_o = np.asarray(inputs["w_o"], np.float32)
    b_q = np.asarray(inputs["b_q"], np.float32)
    b_k = np.asarray(inputs["b_k"], np.float32)
    b_v = np.asarray(inputs["b_v"], np.float32)
    b_o = np.asarray(inputs["b_o"], np.float32)
    k_index = int(np.asarray(inputs["k_index"]))
    assert 1 <= k_index <= 8, f"kernel supports k_index<=8, got {k_index}"

    # fold the 1/sqrt(DK) score scaling into the q projection (exact: 2^-3)
    scale = np.float32(1.0 / math.sqrt(DK))
    w_qs = (w_q * scale).astype(np.float32)
    b_qs = (b_q * scale).astype(np.float32)

    has_bias = {
        "bq": bool(np.any(b_qs)),
        "bk": bool(np.any(b_k)),
        "bv": bool(np.any(b_v)),
        "bo": bool(np.any(b_o)),
    }

    nc = _build_program(k_index, has_bias)
    global _last_nc
    _last_nc = nc

    def _wide_w(w16):
        # [D, D] -> [128, FT*D]: ft-blocks of 128 rows laid side by side
        return np.ascontiguousarray(
            w16.reshape(FT, 128, D).transpose(1, 0, 2).reshape(128, FT * D))

    def _wide_x(x16):
        # [B', D, S] -> [B', 128, FT*S]
        bb = x16.shape[0]
        return np.ascontiguousarray(
            x16.reshape(bb, FT, 128, S).transpose(0, 2, 1, 3)
            .reshape(bb, 128, FT * S))

    wqhi, wqlo = _split16(w_qs)
    wkhi, wklo = _split16(w_k)
    shared = {
        "wqhi": _wide_w(wqhi),
        "wqlo": _wide_w(wqlo),
        "wkhi": _wide_w(wkhi),
        "wklo": _wide_w(wklo),
        "wv": _wide_w(w_v.astype(np.float16)),
        "wo": _wide_w(w_o.astype(np.float16)),
    }
    for nm, arr in (("bq", b_qs), ("bk", b_k), ("bv", b_v), ("bo", b_o)):
        if has_bias[nm]:
            shared[nm] = np.ascontiguousarray(arr.reshape(1, D).astype(np.float32))

    qT = q.transpose(0, 2, 1)
    kT = k.transpose(0, 2, 1)
    vTf = v.transpose(0, 2, 1).astype(np.float16)
    qhiT, qloT = _split16(qT)
    khiT, kloT = _split16(kT)

    in_maps = []
    for c in range(NCORES):
        sl = slice(c * BC, (c + 1) * BC)
        in_maps.append(dict(
            shared,
            qhiT=_wide_x(qhiT[sl]),
            qloT=_wide_x(qloT[sl]),
            khiT=_wide_x(khiT[sl]),
            kloT=_wide_x(kloT[sl]),
            vT=_wide_x(vTf[sl]),
        ))

    res = run_bass_kernel_spmd(
        nc, in_maps, core_ids=list(range(NCORES)), trace=CFG["trace"]
    )
    out = np.concatenate([r["out"] for r in res.results], axis=0)
    kernel.last_result = res
    return out


# revision 65
# speedup vs baseline: 1.1441x; 1.0028x over previous
"""Trainium2 Bass kernel for sparse (top-k) multi-head causal attention.

Problem (hardcoded shapes, from the reference):
  B=32, S=512, D=512, H=8, DK=64, k_index=5 (any k<=8 supported)
  out = TopKCausalAttention(q, k, v; w_q..w_o, b_q..b_o)

Sharding: data-parallel over batch across 8 NeuronCores (4 batches/core).

Numerics: the top-k selection is discontinuous, so scores need ~2^-16
relative accuracy vs the fp32 reference.  fp32 matmuls run at 4 cyc/row
on the PE; instead the q/k path uses f16 hi/lo PAIR arithmetic (3
matmuls at 1 cyc/row, ~2^-22 effective):
  q = qhi + qlo (host-split f16), w_q = whi + wlo (host-split f16)
  qh = qhi*whi + qhi*wlo + qlo*whi        (dropped qlo*wlo ~ 2^-22)
  qh -> (hi, lo) f16 evac split; scores = qhh*khh + qhh*khl + qhl*khh
Measured end-to-end rel err vs fp32 reference: ~2.7e-3 (gate 2e-2).
(float32r at 1 cyc/row was measured: its DMA/weight path quantizes to
11 mantissa bits -> rel err 1.6e-2, too close to the gate; and
engine-written f32r tiles load garbage as PE weights.)

Per-core algorithm (per batch b, head pair hp, heads hh=0,1):
  scores_psum[r-tile, 0:w] = 3 pair matmuls per head (upper tiles
      skipped; no mask matmul -- see below)
  e = exp(scores)                 (ACT, PSUM->SBUF)
  e[diag block] *= LT             (strict-causal mask applied POST-exp
                                   as a 0/1 lower-triangular multiply on
                                   the otherwise-idle Pool engine; exp of
                                   unmasked scores is finite and x*0 == 0,
                                   so the math is exact; frees 6.8us of
                                   PE identity-matmul mask work)
  zfull = row-sum of masked e at ri=0 (DVE reduce, for rows < k)
  top8 = vector.max(e)            (top-8 per row, one DVE op)
  tau = top8[:, k-1]; rows < k get tau := 0; Z = sum(top8[:, :k]) or
      full-row sum for rows < k; row 0: Z := 1
  pu = (e >= tau) * e             (DVE stt, f16 out; exact-by-value
                                   threshold, matching reference)
  R[ri] = diag(1/Z)               (f16, tensor_scalar identity * rz)
  ptb[c, r] = pu[r, c]^T @ R      (regular PE matmul: transpose AND
                                   1/Z normalization in one 1cyc/row op)
  attnT[d, r] += vh_ci^T @ ptrow_ci   (f16, triangular)
  y[r, :] = sum_hp attnT^T @ w_o (+ b_o) -> DRAM

Scheduling (vs the per-instruction cost model): PE is the bottleneck
(~202.5us busy of ~249us total).  Batch b+1's projections are emitted
between hp1/hp2 of batch b so their PSUM evacuations (ACT hi-copy +
DVE lo-subtract; GPSIMD cannot touch PSUM) never queue behind head-pair
DVE chains.  The last batch's head pairs interleave into batch BC-2's
stream, and its pt/y evacuations move ACT->DVE, to shorten the
pipeline-drain tail.  Batch-0 q/wq loads are issued in halves so the
first projection starts before the full 2MB lands.
"""

import math
import os

os.environ.setdefault("MYCRO_LOCAL_CACHE", "1")

from contextlib import ExitStack

import numpy as np

import concourse.bass as bass
import concourse.bacc as bacc
import concourse.mybir as mybir
import concourse.tile as tile
from concourse.bass_utils import run_bass_kernel_spmd

B, S, D, H = 32, 512, 512, 8
DK = D // H  # 64
NCORES = 8
BC = B // NCORES  # batches per core
RT = S // 128  # row tiles per sequence
FT = D // 128  # feature tiles
NEG = -1.0e32

F32 = mybir.dt.float32
BF16 = mybir.dt.bfloat16
F16 = mybir.dt.float16

_last_nc = None

CFG = {
    "trace": False,
    "mask_on_pe": True,   # bf16 identity-matmul mask vs DVE tensor add
}


def _build_program(k_index: int, has_bias: dict):
    """Builds the per-core Bass program."""
    nc = bacc.Bacc(
        "TRN2", target_bir_lowering=False, debug=False, num_devices=NCORES
    )

    # --- DRAM I/O -------------------------------------------------------
    # q/k in transposed layout, host-split into f16 hi/lo pairs and
    # host-pre-arranged as [128, FT*S] (ft-blocks side by side) so each
    # tensor loads with ONE wide DMA instead of FT strided ones.
    qhiT = nc.dram_tensor("qhiT", (BC, 128, FT * S), F16, kind="ExternalInput").ap()
    qloT = nc.dram_tensor("qloT", (BC, 128, FT * S), F16, kind="ExternalInput").ap()
    khiT = nc.dram_tensor("khiT", (BC, 128, FT * S), F16, kind="ExternalInput").ap()
    kloT = nc.dram_tensor("kloT", (BC, 128, FT * S), F16, kind="ExternalInput").ap()
    vT = nc.dram_tensor("vT", (BC, 128, FT * S), F16, kind="ExternalInput").ap()
    wqhi = nc.dram_tensor("wqhi", (128, FT * D), F16, kind="ExternalInput").ap()
    wqlo = nc.dram_tensor("wqlo", (128, FT * D), F16, kind="ExternalInput").ap()
    wkhi = nc.dram_tensor("wkhi", (128, FT * D), F16, kind="ExternalInput").ap()
    wklo = nc.dram_tensor("wklo", (128, FT * D), F16, kind="ExternalInput").ap()
    wv = nc.dram_tensor("wv", (128, FT * D), F16, kind="ExternalInput").ap()
    wo = nc.dram_tensor("wo", (128, FT * D), F16, kind="ExternalInput").ap()
    bias_aps = {}
    for name in ("bq", "bk", "bv", "bo"):
        if has_bias[name]:
            bias_aps[name] = nc.dram_tensor(
                name, (1, D), F32, kind="ExternalInput"
            ).ap()
    out = nc.dram_tensor("out", (BC, S, D), F32, kind="ExternalOutput").ap()

    # --- inline constants ----------------------------------------------
    ident_np = np.eye(128, dtype=np.float32)
    ident_p = nc.inline_tensor(
        ident_np.astype(mybir.dt.np(F16)), name="identp"
    ).ap()
    lt_np = (np.arange(128)[None, :] < np.arange(128)[:, None]).astype(
        mybir.dt.np(F16))
    lt_tri = nc.inline_tensor(lt_np, name="lttri").ap()
    ones_row = nc.inline_tensor(
        np.ones((1, S), dtype=np.float32), name="onesrow"
    ).ap()

    with tile.TileContext(nc) as tc, ExitStack() as ctx:
        # ---------------- pools ----------------
        consts = ctx.enter_context(tc.tile_pool(name="consts", bufs=1))
        xpool = ctx.enter_context(tc.tile_pool(name="xpool", bufs=2))
        projpool = ctx.enter_context(tc.tile_pool(name="projpool", bufs=2))
        epool = ctx.enter_context(tc.tile_pool(name="epool", bufs=20))
        pnpool = ctx.enter_context(tc.tile_pool(name="pnpool", bufs=12))
        rpool = ctx.enter_context(tc.tile_pool(name="rpool", bufs=10))
        ptpool = ctx.enter_context(tc.tile_pool(name="ptpool", bufs=12))
        smallpool = ctx.enter_context(tc.tile_pool(name="smallpool", bufs=4))
        atpool = ctx.enter_context(tc.tile_pool(name="atpool", bufs=3))
        ypool = ctx.enter_context(tc.tile_pool(name="ypool", bufs=3))

        ps_proj = ctx.enter_context(tc.tile_pool(name="ps_proj", bufs=2, space="PSUM"))
        ps_sc = ctx.enter_context(tc.tile_pool(name="ps_sc", bufs=3, space="PSUM"))
        ps_pt = ctx.enter_context(tc.tile_pool(name="ps_pt", bufs=1, space="PSUM"))
        ps_at = ctx.enter_context(tc.tile_pool(name="ps_at", bufs=1, space="PSUM"))
        ps_y = ctx.enter_context(tc.tile_pool(name="ps_y", bufs=1, space="PSUM"))

        # ---------------- resident constants ----------------
        # combined [128, FT*S] tiles: one wide DMA per tensor; q weights +
        # batch 0's q first so the first projection matmuls start earliest.
        HW = FT * D // 2
        wqh_sb = consts.tile([128, FT * D], F16, name="wqh")
        nc.sync.dma_start(wqh_sb[:, 0:HW], wqhi[:, 0:HW])
        _xq0h = xpool.tile([128, FT * S], F16, name="xqh", tag="xqh")
        nc.sync.dma_start(_xq0h[:, 0:HW], qhiT[0, :, 0:HW])
        wql_sb = consts.tile([128, FT * D], F16, name="wql")
        nc.sync.dma_start(wql_sb[:, 0:HW], wqlo[:, 0:HW])
        _xq0l = xpool.tile([128, FT * S], F16, name="xql", tag="xql")
        nc.sync.dma_start(_xq0l[:, 0:HW], qloT[0, :, 0:HW])
        nc.sync.dma_start(wqh_sb[:, HW:], wqhi[:, HW:])
        nc.sync.dma_start(_xq0h[:, HW:], qhiT[0, :, HW:])
        nc.sync.dma_start(wql_sb[:, HW:], wqlo[:, HW:])
        nc.sync.dma_start(_xq0l[:, HW:], qloT[0, :, HW:])
        _xq0 = (_xq0h, _xq0l)
        wkh_sb = consts.tile_from(wkhi, name="wkh")
        _xk0h = xpool.tile_from(khiT[0], name="xkh")
        wkl_sb = consts.tile_from(wklo, name="wkl")
        preloaded = {}
        preloaded[0] = (
            _xq0,
            (_xk0h, xpool.tile_from(kloT[0], name="xkl")),
            xpool.tile_from(vT[0], name="xv"),
        )
        wv_sb = consts.tile_from(wv, name="wv")
        wo_sb = consts.tile_from(wo, name="wo")
        identp_sb = consts.tile_from(ident_p, name="identp_sb")
        lt_sb = consts.tile_from(lt_tri, name="lt_sb")
        ones_sb = consts.tile_from(ones_row, name="ones_sb")
        bias_sb = {
            nm: consts.tile_from(ap, name=f"{nm}_sb") for nm, ap in bias_aps.items()
        }

        Exp = mybir.ActivationFunctionType.Exp
        AO = mybir.AluOpType

        def emit_proj(b, defer_v=False):
            """Loads + q/k/v projections for batch b.

            q/k: f16 pair-product accumulation (12 matmuls per output
            tile), evacuated as an f16 hi/lo split: hi via ACT copy,
            lo = psum - hi via DVE/Pool tensor_tensor subtract.
            """
            if b in preloaded:
                (xqh, xql), (xkh, xkl), xv = preloaded.pop(b)
            else:
                xqh = xpool.tile_from(qhiT[b], name="xqh")
                xql = xpool.tile_from(qloT[b], name="xql")
                xkh = xpool.tile_from(khiT[b], name="xkh")
                xkl = xpool.tile_from(kloT[b], name="xkl")
                xv = xpool.tile_from(vT[b], name="xv")
            qhT, khT, vh = [], [], []  # qhT/khT: list of (hi, lo) per dt
            for dt in range(FT):
                for which, whi_sb, wlo_sb, xh, xl, bkey, outl in (
                        ("q", wqh_sb, wql_sb, xqh, xql, "bq", qhT),
                        ("k", wkh_sb, wkl_sb, xkh, xkl, "bk", khT)):
                    ps = ps_proj.tile([128, S], F32, name="psq", tag="psproj")
                    nbias = bkey in bias_sb
                    nmm = 3 * FT
                    i = 0
                    for ft in range(FT):
                        wsl = slice(ft * D + dt * 128, ft * D + (dt + 1) * 128)
                        xsl = slice(ft * S, (ft + 1) * S)
                        for w_sb, xs in ((whi_sb, xh), (wlo_sb, xh),
                                         (whi_sb, xl)):
                            i += 1
                            nc.tensor.matmul(
                                ps, w_sb[:, wsl], xs[:, xsl],
                                start=(i == 1),
                                stop=(i == nmm and not nbias))
                    if nbias:
                        nc.tensor.matmul(
                            ps, bias_sb[bkey][0:1, dt * 128:(dt + 1) * 128],
                            ones_sb, start=False, stop=True)
                    thi = projpool.tile([128, S], F16, name=f"{which}hT{dt}h",
                                        tag=f"{which}hT{dt}h")
                    nc.scalar.copy(thi, ps)
                    tlo = projpool.tile([128, S], F16, name=f"{which}hT{dt}l",
                                        tag=f"{which}hT{dt}l")
                    # GPSIMD cannot access PSUM (walrus constraint): the
                    # latency-critical lo evac goes on DVE
                    nc.vector.tensor_tensor(tlo, ps, thi, op=AO.subtract)
                    outl.append((thi, tlo))

            def do_vproj(rts=range(RT)):
                for rt in rts:
                    ps = ps_proj.tile([128, D], F32, name="psv", tag="psproj")
                    nbias = "bv" in bias_sb
                    for ft in range(FT):
                        nc.tensor.matmul(
                            ps, xv[:, ft * S + rt * 128:ft * S + (rt + 1) * 128],
                            wv_sb[:, ft * D:(ft + 1) * D],
                            start=(ft == 0), stop=(ft == FT - 1 and not nbias))
                    if nbias:
                        nc.tensor.matmul(
                            ps, ones_sb[0:1, 0:128], bias_sb["bv"],
                            start=False, stop=True)
                    t = projpool.tile([128, D], F16, name=f"vh{rt}", tag=f"vh{rt}")
                    nc.scalar.copy(t, ps)
                    vh.append(t)
                return vh
            if defer_v:
                return qhT, khT, do_vproj
            return qhT, khT, do_vproj()

        def emit_headpair(hp, qhT, khT, vh, pt_dve=False, at_dve=False):
            """Scores / top-k softmax / normalized transpose / attnT for one
            head pair (partition halves 0:64 / 64:128 of the proj tiles)."""
            etiles = [[None] * RT, [None] * RT]
            zfulls = [None, None]
            top8s = []
            for hh in range(2):
                top8s.append(smallpool.tile(
                    [128, RT * 8], F32, name=f"top8{hh}", tag=f"top8{hh}"))
            qh_hi, qh_lo = qhT[hp]
            kh_hi, kh_lo = khT[hp]
            for ri in range(RT):
                w = (ri + 1) * 128
                spss = []
                for hh in range(2):
                    po = hh * 64
                    sps = ps_sc.tile([128, S], F32, name="sps", tag="sps")
                    rsl = slice(ri * 128, (ri + 1) * 128)
                    for i, (qt, kt) in enumerate((
                            (qh_hi, kh_hi), (qh_hi, kh_lo), (qh_lo, kh_hi))):
                        nc.tensor.matmul(
                            sps[:, 0:w],
                            qt[po:po + 64, rsl],
                            kt[po:po + 64, 0:w],
                            start=(i == 0), stop=(i == 2))
                    spss.append(sps)
                for hh in range(2):
                    e = epool.tile([128, S], F32, name="e", tag="e")
                    nc.scalar.activation(e[:, 0:w], spss[hh][:, 0:w], Exp)
                    # strict-causal mask applied post-exp on the diagonal
                    # block: e *= LT (0/1) on the otherwise-idle Pool engine
                    # (exp of unmasked scores is finite; x*0 == 0 exactly)
                    nc.gpsimd.tensor_tensor(
                        e[:, ri * 128:(ri + 1) * 128],
                        e[:, ri * 128:(ri + 1) * 128], lt_sb, op=AO.mult)
                    if ri == 0:
                        zf = smallpool.tile(
                            [128, 1], F32, name=f"zfull{hh}", tag=f"zfull{hh}")
                        zfulls[hh] = zf
                        nc.vector.reduce_sum(
                            zf, e[:, 0:w], axis=mybir.AxisListType.X)
                    nc.vector.max(
                        out=top8s[hh][:, ri * 8:(ri + 1) * 8], in_=e[:, 0:w])
                    etiles[hh][ri] = e
            ptrows = [[None] * RT, [None] * RT]
            rtiless, pnss = [], []
            for hh in range(2):
                top8 = top8s[hh]
                zk = smallpool.tile([128, RT], F32, name="zk", tag="zk")
                nc.vector.reduce_sum(
                    zk, top8.rearrange("p (r e) -> p r e", e=8)[:, :, 0:k_index],
                    axis=mybir.AxisListType.X)
                nc.vector.tensor_copy(zk[0:k_index, 0:1], zfulls[hh][0:k_index, :])
                nc.vector.memset(zk[0:1, 0:1], 1.0)
                # rows < k keep every valid entry: tau := 0
                nc.vector.memset(top8[0:k_index, k_index - 1:k_index], 0.0)
                rz = smallpool.tile([128, RT], F32, name="rz", tag="rz")
                nc.vector.reciprocal(rz, zk)

                # R[ri] = diag(rz[:, ri]) in f16: ACT copy-with-scale of
                # the identity
                rtiles = []
                for ri in range(RT):
                    R = rpool.tile([128, 128], F16, name="rdiag", tag="rdiag")
                    nc.gpsimd.tensor_scalar(
                        R, identp_sb, rz[:, ri:ri + 1], None, op0=AO.mult)
                    rtiles.append(R)
                if True:
                    rtiless.append(rtiles)

                # masked (unnormalized) probs, f16
                pns = []
                for ri in range(RT):
                    w = (ri + 1) * 128
                    e = etiles[hh][ri]
                    tau = top8[:, ri * 8 + k_index - 1: ri * 8 + k_index]
                    pn = pnpool.tile([128, S], F16, name="pn", tag="pn")
                    nc.vector.scalar_tensor_tensor(
                        pn[:, 0:w], e[:, 0:w], tau, e[:, 0:w],
                        op0=AO.is_ge, op1=AO.mult)
                    pns.append(pn)
                pnss.append(pns)

            # normalized transpose: ptb[c, r-block] = pn[r-block, c]^T
            # @ diag(rz) -- regular matmul, transpose + 1/Z in one op.
            # ci-major with heads alternating so attnT's ci-ordered
            # accumulation can start after the first two groups, not five.
            for ci in range(RT):
                for hh in range(2):
                    rtiles = rtiless[hh]
                    pns = pnss[hh]
                    wv_ = (RT - ci) * 128
                    ptb = ps_pt.tile([128, S], F32, name="ptb", tag="ptb")
                    for ri in range(ci, RT):
                        nc.tensor.matmul(
                            ptb[:, (ri - ci) * 128:(ri - ci + 1) * 128],
                            pns[ri][:, ci * 128:(ci + 1) * 128],
                            rtiles[ri], start=True, stop=True)
                    ptrow = ptpool.tile([128, S], F16, name="ptrow", tag="ptrow")
                    if pt_dve:
                        nc.vector.tensor_copy(ptrow[:, 0:wv_], ptb[:, 0:wv_])
                    else:
                        nc.scalar.copy(ptrow[:, 0:wv_], ptb[:, 0:wv_])
                    ptrows[hh][ci] = ptrow

            def finish(vh):
                at_ps = ps_at.tile([128, S], F32, name="atps", tag="atps")
                for ci in range(RT):
                    wv_ = (RT - ci) * 128
                    for hh in range(2):
                        h = 2 * hp + hh
                        po = hh * 64
                        nc.tensor.matmul(
                            at_ps[po:po + 64, ci * 128:S],
                            vh[ci][:, h * DK:(h + 1) * DK],
                            ptrows[hh][ci][:, 0:wv_],
                            start=(ci == 0), stop=(ci == RT - 1),
                            skip_group_check=True)
                at = atpool.tile([128, S], F16, name=f"at{hp}", tag=f"at{hp}")
                if at_dve:
                    nc.vector.tensor_copy(at, at_ps)
                else:
                    nc.scalar.copy(at, at_ps)
                return at
            if vh is None:
                return finish
            return finish(vh)

        def emit_y(b, attnT_sb, y_dve=False):
            for ri in range(RT):
                yps = ps_y.tile([128, D], F32, name="yps", tag="yps")
                nbias = "bo" in bias_sb
                for hp in range(FT):
                    nc.tensor.matmul(
                        yps, attnT_sb[hp][:, ri * 128:(ri + 1) * 128],
                        wo_sb[:, hp * D:(hp + 1) * D],
                        start=(hp == 0), stop=(hp == FT - 1 and not nbias))
                if nbias:
                    nc.tensor.matmul(
                        yps, ones_sb[0:1, 0:128], bias_sb["bo"],
                        start=False, stop=True)
                y = ypool.tile([128, D], F32, name="y", tag="y")
                if y_dve:
                    nc.vector.tensor_copy(y, yps)
                else:
                    nc.scalar.copy(y, yps)
                nc.scalar.dma_start(out[b, ri * 128:(ri + 1) * 128, :], y)

        # proj for batch b+1 is emitted between hp1 and hp2 of batch b
        # (latency-critical DVE lo-subtracts enqueue ahead of later head
        # pairs' DVE chains).  The LAST batch's head pairs are interleaved
        # into batch BC-2's stream so only two chains drain at the tail.
        projs = {0: emit_proj(0)}
        ats = {b: [] for b in range(BC)}
        for b in range(BC - 1):
            qhT, khT, vh = projs.pop(b)
            if b < BC - 2:
                for hp in range(FT):
                    ats[b].append(emit_headpair(hp, qhT, khT, vh))
                    if hp == 1:
                        projs[b + 1] = emit_proj(b + 1)
                emit_y(b, ats[b])
            else:
                # interleave tail: b2.hp0 b2.hp1 [proj3] b2.hp2 b3.hp0
                # b2.hp3 b3.hp1 y2 b3.hp2 b3.hp3 y3
                ats[b].append(emit_headpair(0, qhT, khT, vh))
                ats[b].append(emit_headpair(1, qhT, khT, vh))
                projs[b + 1] = emit_proj(b + 1)
                qhT3, khT3, vh3 = projs.pop(b + 1)
                ats[b].append(emit_headpair(2, qhT, khT, vh))
                ats[b + 1].append(emit_headpair(0, qhT3, khT3, vh3))
                ats[b].append(emit_headpair(3, qhT, khT, vh))
                ats[b + 1].append(emit_headpair(1, qhT3, khT3, vh3, pt_dve=True))
                emit_y(b, ats[b])
                ats[b + 1].append(emit_headpair(2, qhT3, khT3, vh3,
                                                 pt_dve=True, at_dve=True))
                ats[b + 1].append(emit_headpair(3, qhT3, khT3, vh3,
                                                 pt_dve=True, at_dve=True))
                emit_y(b + 1, ats[b + 1], y_dve=True)

    nc.compile()
    return nc


def _split16(x):
    """Split fp32 array into (hi, lo) f16 pair with hi + lo ~= x."""
    hi = x.astype(np.float16)
    lo = (x - hi.astype(np.float32)).astype(np.float16)
    return hi, lo


def kernel(**inputs):
    q = np.asarray(inputs["q"], np.float32)
    k = np.asarray(inputs["k"], np.float32)
    v = np.asarray(inputs["v"], np.float32)
    w_q = np.asarray(inputs["w_q"], np.float32)
    w_k = np.asarray(inputs["w_k"], np.float32)
    w_v = np.asarray(inputs["w_v"], np.float32)
    w_o = np.asarray(inputs["w_o"], np.float32)
    b_q = np.asarray(inputs["b_q"], np.float32)
    b_k = np.asarray(inputs["b_k"], np.float32)
    b_v = np.asarray(inputs["b_v"], np.float32)
    b_o = np.asarray(inputs["b_o"], np.float32)
    k_index = int(np.asarray(inputs["k_index"]))
    assert 1 <= k_index <= 8, f"kernel supports k_index<=8, got {k_index}"

    # fold the 1/sqrt(DK) score scaling into the q projection (exact: 2^-3)
    scale = np.float32(1.0 / math.sqrt(DK))
    w_qs = (w_q * scale).astype(np.float32)
    b_qs = (b_q * scale).astype(np.float32)

    has_bias = {
        "bq": bool(np.any(b_qs)),
        "bk": bool(np.any(b_k)),
        "bv": bool(np.any(b_v)),
        "bo": bool(np.any(b_o)),
    }

    nc = _build_program(k_index, has_bias)
    global _last_nc
    _last_nc = nc

    def _wide_w(w16):
        # [D, D] -> [128, FT*D]: ft-blocks of 128 rows laid side by side
        return np.ascontiguousarray(
            w16.reshape(FT, 128, D).transpose(1, 0, 2).reshape(128, FT * D))

    def _wide_x(x16):
        # [B', D, S] -> [B', 128, FT*S]
        bb = x16.shape[0]
        return np.ascontiguousarray(
            x16.reshape(bb, FT, 128, S).transpose(0, 2, 1, 3)
            .reshape(bb, 128, FT * S))

    wqhi, wqlo = _split16(w_qs)
    wkhi, wklo = _split16(w_k)
    shared = {
        "wqhi": _wide_w(wqhi),
        "wqlo": _wide_w(wqlo),
        "wkhi": _wide_w(wkhi),
        "wklo": _wide_w(wklo),
        "wv": _wide_w(w_v.astype(np.float16)),
        "wo": _wide_w(w_o.astype(np.float16)),
    }
    for nm, arr in (("bq", b_qs), ("bk", b_k), ("bv", b_v), ("bo", b_o)):
        if has_bias[nm]:
            shared[nm] = np.ascontiguousarray(arr.reshape(1, D).astype(np.float32))

    qT = q.transpose(0, 2, 1)
    kT = k.transpose(0, 2, 1)
    vTf = v.transpose(0, 2, 1).astype(np.float16)
    qhiT, qloT = _split16(qT)
    khiT, kloT = _split16(kT)

    in_maps = []
    for c in range(NCORES):
        sl = slice(c * BC, (c + 1) * BC)
        in_maps.append(dict(
            shared,
            qhiT=_wide_x(qhiT[sl]),
            qloT=_wide_x(qloT[sl]),
            khiT=_wide_x(khiT[sl]),
            kloT=_wide_x(kloT[sl]),
            vT=_wide_x(vTf[sl]),
        ))

    res = run_bass_kernel_spmd(
        nc, in_maps, core_ids=list(range(NCORES)), trace=CFG["trace"]
    )
    out = np.concatenate([r["out"] for r in res.results], axis=0)
    kernel.last_result = res
    return out


# revision 70
# speedup vs baseline: 1.1489x; 1.0042x over previous
"""Trainium2 Bass kernel for sparse (top-k) multi-head causal attention.

Problem (hardcoded shapes, from the reference):
  B=32, S=512, D=512, H=8, DK=64, k_index=5 (any k<=8 supported)
  out = TopKCausalAttention(q, k, v; w_q..w_o, b_q..b_o)

Sharding: data-parallel over batch across 8 NeuronCores (4 batches/core).

Numerics: the top-k selection is discontinuous, so scores need ~2^-16
relative accuracy vs the fp32 reference.  fp32 matmuls run at 4 cyc/row
on the PE; instead the q/k path uses f16 hi/lo PAIR arithmetic (3
matmuls at 1 cyc/row, ~2^-22 effective):
  q = qhi + qlo (host-split f16), w_q = whi + wlo (host-split f16)
  qh = qhi*whi + qhi*wlo + qlo*whi        (dropped qlo*wlo ~ 2^-22)
  qh -> (hi, lo) f16 evac split; scores = qhh*khh + qhh*khl + qhl*khh
Measured end-to-end rel err vs fp32 reference: ~2.7e-3 (gate 2e-2).
(float32r at 1 cyc/row was measured: its DMA/weight path quantizes to
11 mantissa bits -> rel err 1.6e-2, too close to the gate; and
engine-written f32r tiles load garbage as PE weights.)

Per-core algorithm (per batch b, head pair hp, heads hh=0,1):
  scores_psum[r-tile, 0:w] = 3 pair matmuls per head (upper tiles
      skipped; no mask matmul -- see below)
  e = exp(scores)                 (ACT, PSUM->SBUF)
  e[diag block] *= LT             (strict-causal mask applied POST-exp
                                   as a 0/1 lower-triangular multiply on
                                   the otherwise-idle Pool engine; exp of
                                   unmasked scores is finite and x*0 == 0,
                                   so the math is exact; frees 6.8us of
                                   PE identity-matmul mask work)
  zfull = row-sum of masked e at ri=0 (DVE reduce, for rows < k)
  top8 = vector.max(e)            (top-8 per row, one DVE op)
  tau = top8[:, k-1]; rows < k get tau := 0; Z = sum(top8[:, :k]) or
      full-row sum for rows < k; row 0: Z := 1
  pu = (e >= tau) * e             (DVE stt, f16 out; exact-by-value
                                   threshold, matching reference)
  R[ri] = diag(1/Z)               (f16, tensor_scalar identity * rz)
  ptb[c, r] = pu[r, c]^T @ R      (regular PE matmul: transpose AND
                                   1/Z normalization in one 1cyc/row op)
  attnT[d, r] += vh_ci^T @ ptrow_ci   (f16, triangular)
  y[r, :] = sum_hp attnT^T @ w_o (+ b_o) -> DRAM

Scheduling (vs the per-instruction cost model): PE is the bottleneck
(~202.5us busy of ~249us total).  Batch b+1's projections are emitted
between hp1/hp2 of batch b so their PSUM evacuations (ACT hi-copy +
DVE lo-subtract; GPSIMD cannot touch PSUM) never queue behind head-pair
DVE chains.  The last batch's head pairs interleave into batch BC-2's
stream, and its pt/y evacuations move ACT->DVE, to shorten the
pipeline-drain tail.  Batch-0 q/wq loads are issued in halves so the
first projection starts before the full 2MB lands.
"""

import math
import os

os.environ.setdefault("MYCRO_LOCAL_CACHE", "1")

from contextlib import ExitStack

import numpy as np

import concourse.bass as bass
import concourse.bacc as bacc
import concourse.mybir as mybir
import concourse.tile as tile
from concourse.bass_utils import run_bass_kernel_spmd

B, S, D, H = 32, 512, 512, 8
DK = D // H  # 64
NCORES = 8
BC = B // NCORES  # batches per core
RT = S // 128  # row tiles per sequence
FT = D // 128  # feature tiles
NEG = -1.0e32

F32 = mybir.dt.float32
BF16 = mybir.dt.bfloat16
F16 = mybir.dt.float16

_last_nc = None

CFG = {
    "trace": False,
    "mask_on_pe": True,   # bf16 identity-matmul mask vs DVE tensor add
}


def _build_program(k_index: int, has_bias: dict):
    """Builds the per-core Bass program."""
    nc = bacc.Bacc(
        "TRN2", target_bir_lowering=False, debug=False, num_devices=NCORES
    )

    # --- DRAM I/O -------------------------------------------------------
    # q/k in transposed layout, host-split into f16 hi/lo pairs and
    # host-pre-arranged as [128, FT*S] (ft-blocks side by side) so each
    # tensor loads with ONE wide DMA instead of FT strided ones.
    qhiT = nc.dram_tensor("qhiT", (BC, 128, FT * S), F16, kind="ExternalInput").ap()
    qloT = nc.dram_tensor("qloT", (BC, 128, FT * S), F16, kind="ExternalInput").ap()
    khiT = nc.dram_tensor("khiT", (BC, 128, FT * S), F16, kind="ExternalInput").ap()
    kloT = nc.dram_tensor("kloT", (BC, 128, FT * S), F16, kind="ExternalInput").ap()
    vT = nc.dram_tensor("vT", (BC, 128, FT * S), F16, kind="ExternalInput").ap()
    wqhi = nc.dram_tensor("wqhi", (128, FT * D), F16, kind="ExternalInput").ap()
    wqlo = nc.dram_tensor("wqlo", (128, FT * D), F16, kind="ExternalInput").ap()
    wkhi = nc.dram_tensor("wkhi", (128, FT * D), F16, kind="ExternalInput").ap()
    wklo = nc.dram_tensor("wklo", (128, FT * D), F16, kind="ExternalInput").ap()
    wv = nc.dram_tensor("wv", (128, FT * D), F16, kind="ExternalInput").ap()
    wo = nc.dram_tensor("wo", (128, FT * D), F16, kind="ExternalInput").ap()
    bias_aps = {}
    for name in ("bq", "bk", "bv", "bo"):
        if has_bias[name]:
            bias_aps[name] = nc.dram_tensor(
                name, (1, D), F32, kind="ExternalInput"
            ).ap()
    out = nc.dram_tensor("out", (BC, S, D), F32, kind="ExternalOutput").ap()

    # --- inline constants ----------------------------------------------
    ident_np = np.eye(128, dtype=np.float32)
    ident_p = nc.inline_tensor(
        ident_np.astype(mybir.dt.np(F16)), name="identp"
    ).ap()
    lt_np = (np.arange(128)[None, :] < np.arange(128)[:, None]).astype(
        mybir.dt.np(F16))
    lt_tri = nc.inline_tensor(lt_np, name="lttri").ap()
    ones_row = nc.inline_tensor(
        np.ones((1, S), dtype=np.float32), name="onesrow"
    ).ap()

    with tile.TileContext(nc) as tc, ExitStack() as ctx:
        # ---------------- pools ----------------
        consts = ctx.enter_context(tc.tile_pool(name="consts", bufs=1))
        xpool = ctx.enter_context(tc.tile_pool(name="xpool", bufs=2))
        projpool = ctx.enter_context(tc.tile_pool(name="projpool", bufs=2))
        epool = ctx.enter_context(tc.tile_pool(name="epool", bufs=20))
        pnpool = ctx.enter_context(tc.tile_pool(name="pnpool", bufs=12))
        rpool = ctx.enter_context(tc.tile_pool(name="rpool", bufs=10))
        ptpool = ctx.enter_context(tc.tile_pool(name="ptpool", bufs=12))
        smallpool = ctx.enter_context(tc.tile_pool(name="smallpool", bufs=4))
        atpool = ctx.enter_context(tc.tile_pool(name="atpool", bufs=3))
        ypool = ctx.enter_context(tc.tile_pool(name="ypool", bufs=3))

        ps_proj = ctx.enter_context(tc.tile_pool(name="ps_proj", bufs=2, space="PSUM"))
        ps_sc = ctx.enter_context(tc.tile_pool(name="ps_sc", bufs=3, space="PSUM"))
        ps_pt = ctx.enter_context(tc.tile_pool(name="ps_pt", bufs=1, space="PSUM"))
        ps_at = ctx.enter_context(tc.tile_pool(name="ps_at", bufs=1, space="PSUM"))
        ps_y = ctx.enter_context(tc.tile_pool(name="ps_y", bufs=1, space="PSUM"))

        # ---------------- resident constants ----------------
        # combined [128, FT*S] tiles: one wide DMA per tensor; q weights +
        # batch 0's q first so the first projection matmuls start earliest.
        HW = FT * D // 2
        wqh_sb = consts.tile([128, FT * D], F16, name="wqh")
        nc.sync.dma_start(wqh_sb[:, 0:HW], wqhi[:, 0:HW])
        _xq0h = xpool.tile([128, FT * S], F16, name="xqh", tag="xqh")
        nc.sync.dma_start(_xq0h[:, 0:HW], qhiT[0, :, 0:HW])
        wql_sb = consts.tile([128, FT * D], F16, name="wql")
        nc.sync.dma_start(wql_sb[:, 0:HW], wqlo[:, 0:HW])
        _xq0l = xpool.tile([128, FT * S], F16, name="xql", tag="xql")
        nc.sync.dma_start(_xq0l[:, 0:HW], qloT[0, :, 0:HW])
        nc.sync.dma_start(wqh_sb[:, HW:], wqhi[:, HW:])
        nc.sync.dma_start(_xq0h[:, HW:], qhiT[0, :, HW:])
        nc.sync.dma_start(wql_sb[:, HW:], wqlo[:, HW:])
        nc.sync.dma_start(_xq0l[:, HW:], qloT[0, :, HW:])
        _xq0 = (_xq0h, _xq0l)
        wkh_sb = consts.tile_from(wkhi, name="wkh")
        _xk0h = xpool.tile_from(khiT[0], name="xkh")
        wkl_sb = consts.tile_from(wklo, name="wkl")
        preloaded = {}
        preloaded[0] = (
            _xq0,
            (_xk0h, xpool.tile_from(kloT[0], name="xkl")),
            xpool.tile_from(vT[0], name="xv"),
        )
        wv_sb = consts.tile_from(wv, name="wv")
        wo_sb = consts.tile_from(wo, name="wo")
        identp_sb = consts.tile_from(ident_p, name="identp_sb")
        lt_sb = consts.tile_from(lt_tri, name="lt_sb")
        ones_sb = consts.tile_from(ones_row, name="ones_sb")
        bias_sb = {
            nm: consts.tile_from(ap, name=f"{nm}_sb") for nm, ap in bias_aps.items()
        }

        Exp = mybir.ActivationFunctionType.Exp
        AO = mybir.AluOpType

        def emit_proj(b, defer_v=False):
            """Loads + q/k/v projections for batch b.

            q/k: f16 pair-product accumulation (12 matmuls per output
            tile), evacuated as an f16 hi/lo split: hi via ACT copy,
            lo = psum - hi via DVE/Pool tensor_tensor subtract.
            """
            if b in preloaded:
                (xqh, xql), (xkh, xkl), xv = preloaded.pop(b)
            else:
                xqh = xpool.tile_from(qhiT[b], name="xqh")
                xql = xpool.tile_from(qloT[b], name="xql")
                xkh = xpool.tile_from(khiT[b], name="xkh")
                xkl = xpool.tile_from(kloT[b], name="xkl")
                xv = xpool.tile_from(vT[b], name="xv")
            qhT, khT, vh = [], [], []  # qhT/khT: list of (hi, lo) per dt
            for dt in range(FT):
                for which, whi_sb, wlo_sb, xh, xl, bkey, outl in (
                        ("q", wqh_sb, wql_sb, xqh, xql, "bq", qhT),
                        ("k", wkh_sb, wkl_sb, xkh, xkl, "bk", khT)):
                    ps = ps_proj.tile([128, S], F32, name="psq", tag="psproj")
                    nbias = bkey in bias_sb
                    nmm = 3 * FT
                    i = 0
                    # term-major: the 4 hi*hi matmuls come first and only
                    # need the hi loads (startup: lo tensors still in flight)
                    for w_sb, xs in ((whi_sb, xh), (wlo_sb, xh),
                                     (whi_sb, xl)):
                        for ft in range(FT):
                            wsl = slice(ft * D + dt * 128,
                                        ft * D + (dt + 1) * 128)
                            xsl = slice(ft * S, (ft + 1) * S)
                            i += 1
                            nc.tensor.matmul(
                                ps, w_sb[:, wsl], xs[:, xsl],
                                start=(i == 1),
                                stop=(i == nmm and not nbias))
                    if nbias:
                        nc.tensor.matmul(
                            ps, bias_sb[bkey][0:1, dt * 128:(dt + 1) * 128],
                            ones_sb, start=False, stop=True)
                    thi = projpool.tile([128, S], F16, name=f"{which}hT{dt}h",
                                        tag=f"{which}hT{dt}h")
                    nc.scalar.copy(thi, ps)
                    tlo = projpool.tile([128, S], F16, name=f"{which}hT{dt}l",
                                        tag=f"{which}hT{dt}l")
                    # GPSIMD cannot access PSUM (walrus constraint): the
                    # latency-critical lo evac goes on DVE
                    nc.vector.tensor_tensor(tlo, ps, thi, op=AO.subtract)
                    outl.append((thi, tlo))

            def do_vproj(rts=range(RT)):
                for rt in rts:
                    ps = ps_proj.tile([128, D], F32, name="psv", tag="psproj")
                    nbias = "bv" in bias_sb
                    for ft in range(FT):
                        nc.tensor.matmul(
                            ps, xv[:, ft * S + rt * 128:ft * S + (rt + 1) * 128],
                            wv_sb[:, ft * D:(ft + 1) * D],
                            start=(ft == 0), stop=(ft == FT - 1 and not nbias))
                    if nbias:
                        nc.tensor.matmul(
                            ps, ones_sb[0:1, 0:128], bias_sb["bv"],
                            start=False, stop=True)
                    t = projpool.tile([128, D], F16, name=f"vh{rt}", tag=f"vh{rt}")
                    nc.scalar.copy(t, ps)
                    vh.append(t)
                return vh
            if defer_v:
                return qhT, khT, do_vproj
            return qhT, khT, do_vproj()

        def emit_headpair(hp, qhT, khT, vh, pt_dve=False, at_dve=False):
            """Scores / top-k softmax / normalized transpose / attnT for one
            head pair (partition halves 0:64 / 64:128 of the proj tiles)."""
            etiles = [[None] * RT, [None] * RT]
            zfulls = [None, None]
            top8s = []
            for hh in range(2):
                top8s.append(smallpool.tile(
                    [128, RT * 8], F32, name=f"top8{hh}", tag=f"top8{hh}"))
            qh_hi, qh_lo = qhT[hp]
            kh_hi, kh_lo = khT[hp]
            for ri in range(RT):
                w = (ri + 1) * 128
                spss = []
                for hh in range(2):
                    po = hh * 64
                    sps = ps_sc.tile([128, S], F32, name="sps", tag="sps")
                    rsl = slice(ri * 128, (ri + 1) * 128)
                    for i, (qt, kt) in enumerate((
                            (qh_hi, kh_hi), (qh_hi, kh_lo), (qh_lo, kh_hi))):
                        nc.tensor.matmul(
                            sps[:, 0:w],
                            qt[po:po + 64, rsl],
                            kt[po:po + 64, 0:w],
                            start=(i == 0), stop=(i == 2))
                    spss.append(sps)
                for hh in range(2):
                    e = epool.tile([128, S], F32, name="e", tag="e")
                    nc.scalar.activation(e[:, 0:w], spss[hh][:, 0:w], Exp)
                    # strict-causal mask applied post-exp on the diagonal
                    # block: e *= LT (0/1) on the otherwise-idle Pool engine
                    # (exp of unmasked scores is finite; x*0 == 0 exactly)
                    nc.gpsimd.tensor_tensor(
                        e[:, ri * 128:(ri + 1) * 128],
                        e[:, ri * 128:(ri + 1) * 128], lt_sb, op=AO.mult)
                    if ri == 0:
                        zf = smallpool.tile(
                            [128, 1], F32, name=f"zfull{hh}", tag=f"zfull{hh}")
                        zfulls[hh] = zf
                        nc.vector.reduce_sum(
                            zf, e[:, 0:w], axis=mybir.AxisListType.X)
                    nc.vector.max(
                        out=top8s[hh][:, ri * 8:(ri + 1) * 8], in_=e[:, 0:w])
                    etiles[hh][ri] = e
            ptrows = [[None] * RT, [None] * RT]
            rtiless, pnss = [], []
            for hh in range(2):
                top8 = top8s[hh]
                zk = smallpool.tile([128, RT], F32, name="zk", tag="zk")
                nc.vector.reduce_sum(
                    zk, top8.rearrange("p (r e) -> p r e", e=8)[:, :, 0:k_index],
                    axis=mybir.AxisListType.X)
                nc.vector.tensor_copy(zk[0:k_index, 0:1], zfulls[hh][0:k_index, :])
                nc.vector.memset(zk[0:1, 0:1], 1.0)
                # rows < k keep every valid entry: tau := 0
                nc.vector.memset(top8[0:k_index, k_index - 1:k_index], 0.0)
                rz = smallpool.tile([128, RT], F32, name="rz", tag="rz")
                nc.vector.reciprocal(rz, zk)

                # R[ri] = diag(rz[:, ri]) in f16: ACT copy-with-scale of
                # the identity
                rtiles = []
                for ri in range(RT):
                    R = rpool.tile([128, 128], F16, name="rdiag", tag="rdiag")
                    nc.gpsimd.tensor_scalar(
                        R, identp_sb, rz[:, ri:ri + 1], None, op0=AO.mult)
                    rtiles.append(R)
                if True:
                    rtiless.append(rtiles)

                # masked (unnormalized) probs, f16
                pns = []
                for ri in range(RT):
                    w = (ri + 1) * 128
                    e = etiles[hh][ri]
                    tau = top8[:, ri * 8 + k_index - 1: ri * 8 + k_index]
                    pn = pnpool.tile([128, S], F16, name="pn", tag="pn")
                    nc.vector.scalar_tensor_tensor(
                        pn[:, 0:w], e[:, 0:w], tau, e[:, 0:w],
                        op0=AO.is_ge, op1=AO.mult)
                    pns.append(pn)
                pnss.append(pns)

            # normalized transpose: ptb[c, r-block] = pn[r-block, c]^T
            # @ diag(rz) -- regular matmul, transpose + 1/Z in one op.
            # ci-major with heads alternating so attnT's ci-ordered
            # accumulation can start after the first two groups, not five.
            for ci in range(RT):
                for hh in range(2):
                    rtiles = rtiless[hh]
                    pns = pnss[hh]
                    wv_ = (RT - ci) * 128
                    ptb = ps_pt.tile([128, S], F32, name="ptb", tag="ptb")
                    for ri in range(ci, RT):
                        nc.tensor.matmul(
                            ptb[:, (ri - ci) * 128:(ri - ci + 1) * 128],
                            pns[ri][:, ci * 128:(ci + 1) * 128],
                            rtiles[ri], start=True, stop=True)
                    ptrow = ptpool.tile([128, S], F16, name="ptrow", tag="ptrow")
                    if pt_dve:
                        nc.vector.tensor_copy(ptrow[:, 0:wv_], ptb[:, 0:wv_])
                    else:
                        nc.scalar.copy(ptrow[:, 0:wv_], ptb[:, 0:wv_])
                    ptrows[hh][ci] = ptrow

            def finish(vh):
                at_ps = ps_at.tile([128, S], F32, name="atps", tag="atps")
                for ci in range(RT):
                    wv_ = (RT - ci) * 128
                    for hh in range(2):
                        h = 2 * hp + hh
                        po = hh * 64
                        nc.tensor.matmul(
                            at_ps[po:po + 64, ci * 128:S],
                            vh[ci][:, h * DK:(h + 1) * DK],
                            ptrows[hh][ci][:, 0:wv_],
                            start=(ci == 0), stop=(ci == RT - 1),
                            skip_group_check=True)
                at = atpool.tile([128, S], F16, name=f"at{hp}", tag=f"at{hp}")
                if at_dve:
                    nc.vector.tensor_copy(at, at_ps)
                else:
                    nc.scalar.copy(at, at_ps)
                return at
            if vh is None:
                return finish
            return finish(vh)

        def emit_y(b, attnT_sb, y_dve=False):
            for ri in range(RT):
                yps = ps_y.tile([128, D], F32, name="yps", tag="yps")
                nbias = "bo" in bias_sb
                for hp in range(FT):
                    nc.tensor.matmul(
                        yps, attnT_sb[hp][:, ri * 128:(ri + 1) * 128],
                        wo_sb[:, hp * D:(hp + 1) * D],
                        start=(hp == 0), stop=(hp == FT - 1 and not nbias))
                if nbias:
                    nc.tensor.matmul(
                        yps, ones_sb[0:1, 0:128], bias_sb["bo"],
                        start=False, stop=True)
                y = ypool.tile([128, D], F32, name="y", tag="y")
                if y_dve:
                    nc.vector.tensor_copy(y, yps)
                else:
                    nc.scalar.copy(y, yps)
                nc.scalar.dma_start(out[b, ri * 128:(ri + 1) * 128, :], y)

        # proj for batch b+1 is emitted between hp1 and hp2 of batch b
        # (latency-critical DVE lo-subtracts enqueue ahead of later head
        # pairs' DVE chains).  The LAST batch's head pairs are interleaved
        # into batch BC-2's stream so only two chains drain at the tail.
        projs = {0: emit_proj(0)}
        ats = {b: [] for b in range(BC)}
        for b in range(BC - 1):
            qhT, khT, vh = projs.pop(b)
            if b < BC - 2:
                for hp in range(FT):
                    ats[b].append(emit_headpair(hp, qhT, khT, vh))
                    if hp == 1:
                        projs[b + 1] = emit_proj(b + 1)
                emit_y(b, ats[b])
            else:
                # interleave tail: b2.hp0 b2.hp1 [proj3] b2.hp2 b3.hp0
                # b2.hp3 b3.hp1 y2 b3.hp2 b3.hp3 y3
                ats[b].append(emit_headpair(0, qhT, khT, vh))
                ats[b].append(emit_headpair(1, qhT, khT, vh))
                projs[b + 1] = emit_proj(b + 1)
                qhT3, khT3, vh3 = projs.pop(b + 1)
                ats[b].append(emit_headpair(2, qhT, khT, vh))
                ats[b + 1].append(emit_headpair(0, qhT3, khT3, vh3))
                ats[b].append(emit_headpair(3, qhT, khT, vh))
                ats[b + 1].append(emit_headpair(1, qhT3, khT3, vh3, pt_dve=True))
                emit_y(b, ats[b])
                ats[b + 1].append(emit_headpair(2, qhT3, khT3, vh3,
                                                 pt_dve=True, at_dve=True))
                ats[b + 1].append(emit_headpair(3, qhT3, khT3, vh3,
                                                 pt_dve=True, at_dve=True))
                emit_y(b + 1, ats[b + 1], y_dve=True)

    nc.compile()
    return nc


def _split16(x):
    """Split fp32 array into (hi, lo) f16 pair with hi + lo ~= x."""
    hi = x.astype(np.float16)
    lo = (x - hi.astype(np.float32)).astype(np.float16)
    return hi, lo


def kernel(**inputs):
    q = np.asarray(inputs["q"], np.float32)
    k = np.asarray(inputs["k"], np.float32)
    v = np.asarray(inputs["v"], np.float32)
    w_q = np.asarray(inputs["w_q"], np.float32)
    w_k = np.asarray(inputs["w_k"], np.float32)
    w_v = np.asarray(inputs["w_v"], np.float32)
    w_o = np.asarray(inputs["w_o"], np.float32)
    b_q = np.asarray(inputs["b_q"], np.float32)
    b_k = np.asarray(inputs["b_k"], np.float32)
    b_v = np.asarray(inputs["b_v"], np.float32)
    b_o = np.asarray(inputs["b_o"], np.float32)
    k_index = int(np.asarray(inputs["k_index"]))
    assert 1 <= k_index <= 8, f"kernel supports k_index<=8, got {k_index}"

    # fold the 1/sqrt(DK) score scaling into the q projection (exact: 2^-3)
    scale = np.float32(1.0 / math.sqrt(DK))
    w_qs = (w_q * scale).astype(np.float32)
    b_qs = (b_q * scale).astype(np.float32)

    has_bias = {
        "bq": bool(np.any(b_qs)),
        "bk": bool(np.any(b_k)),
        "bv": bool(np.any(b_v)),
        "bo": bool(np.any(b_o)),
    }

    nc = _build_program(k_index, has_bias)
    global _last_nc
    _last_nc = nc

    def _wide_w(w16):
        # [D, D] -> [128, FT*D]: ft-blocks of 128 rows laid side by side
        return np.ascontiguousarray(
            w16.reshape(FT, 128, D).transpose(1, 0, 2).reshape(128, FT * D))

    def _wide_x(x16):
        # [B', D, S] -> [B', 128, FT*S]
        bb = x16.shape[0]
        return np.ascontiguousarray(
            x16.reshape(bb, FT, 128, S).transpose(0, 2, 1, 3)
            .reshape(bb, 128, FT * S))

    wqhi, wqlo = _split16(w_qs)
    wkhi, wklo = _split16(w_k)
    shared = {
        "wqhi": _wide_w(wqhi),
        "wqlo": _wide_w(wqlo),
        "wkhi": _wide_w(wkhi),
        "wklo": _wide_w(wklo),
        "wv": _wide_w(w_v.astype(np.float16)),
        "wo": _wide_w(w_o.astype(np.float16)),
    }
    for nm, arr in (("bq", b_qs), ("bk", b_k), ("bv", b_v), ("bo", b_o)):
        if has_bias[nm]:
            shared[nm] = np.ascontiguousarray(arr.reshape(1, D).astype(np.float32))

    qT = q.transpose(0, 2, 1)
    kT = k.transpose(0, 2, 1)
    vTf = v.transpose(0, 2, 1).astype(np.float16)
    qhiT, qloT = _split16(qT)
    khiT, kloT = _split16(kT)

    in_maps = []
    for c in range(NCORES):
        sl = slice(c * BC, (c + 1) * BC)
        in_maps.append(dict(
            shared,
            qhiT=_wide_x(qhiT[sl]),
            qloT=_wide_x(qloT[sl]),
            khiT=_wide_x(khiT[sl]),
            kloT=_wide_x(kloT[sl]),
            vT=_wide_x(vTf[sl]),
        ))

    res = run_bass_kernel_spmd(
        nc, in_maps, core_ids=list(range(NCORES)), trace=CFG["trace"]
    )
    out = np.concatenate([r["out"] for r in res.results], axis=0)
    kernel.last_result = res
    return out


# revision 73
# speedup vs baseline: 1.1614x; 1.0109x over previous
"""Trainium2 Bass kernel for sparse (top-k) multi-head causal attention.

Problem (hardcoded shapes, from the reference):
  B=32, S=512, D=512, H=8, DK=64, k_index=5 (any k<=8 supported)
  out = TopKCausalAttention(q, k, v; w_q..w_o, b_q..b_o)

Sharding: data-parallel over batch across 8 NeuronCores (4 batches/core).

Numerics: the top-k selection is discontinuous, so scores need ~2^-16
relative accuracy vs the fp32 reference.  fp32 matmuls run at 4 cyc/row
on the PE; instead the q/k path uses f16 hi/lo PAIR arithmetic (3
matmuls at 1 cyc/row, ~2^-22 effective):
  q = qhi + qlo (host-split f16), w_q = whi + wlo (host-split f16)
  qh = qhi*whi + qhi*wlo + qlo*whi        (dropped qlo*wlo ~ 2^-22)
  qh -> (hi, lo) f16 evac split; scores = qhh*khh + qhh*khl + qhl*khh
Measured end-to-end rel err vs fp32 reference: ~2.7e-3 (gate 2e-2).
(float32r at 1 cyc/row was measured: its DMA/weight path quantizes to
11 mantissa bits -> rel err 1.6e-2, too close to the gate; and
engine-written f32r tiles load garbage as PE weights.)

Per-core algorithm (per batch b, head pair hp, heads hh=0,1):
  scores_psum[r-tile, 0:w] = 3 pair matmuls per head (upper tiles
      skipped; no mask matmul -- see below)
  e = exp(scores)                 (ACT, PSUM->SBUF)
  e[diag block] *= LT             (strict-causal mask applied POST-exp
                                   as a 0/1 lower-triangular multiply on
                                   the otherwise-idle Pool engine; exp of
                                   unmasked scores is finite and x*0 == 0,
                                   so the math is exact; frees 6.8us of
                                   PE identity-matmul mask work)
  zfull = row-sum of masked e at ri=0 (DVE reduce, for rows < k)
  top8 = vector.max(e)            (top-8 per row, one DVE op)
  tau = top8[:, k-1]; rows < k get tau := 0; Z = sum(top8[:, :k]) or
      full-row sum for rows < k; row 0: Z := 1
  pu = (e >= tau) * e             (DVE stt, f16 out; exact-by-value
                                   threshold, matching reference)
  R[ri] = diag(1/Z)               (f16, tensor_scalar identity * rz)
  ptb[c, r] = pu[r, c]^T @ R      (regular PE matmul: transpose AND
                                   1/Z normalization in one 1cyc/row op)
  attnT[d, r] += vh_ci^T @ ptrow_ci   (f16, triangular)
  y[r, :] = sum_hp attnT^T @ w_o (+ b_o) -> DRAM

Scheduling (vs the per-instruction cost model): PE is the bottleneck
(~202.5us busy of ~249us total).  Batch b+1's projections are emitted
between hp1/hp2 of batch b so their PSUM evacuations (ACT hi-copy +
DVE lo-subtract; GPSIMD cannot touch PSUM) never queue behind head-pair
DVE chains.  The last batch's head pairs interleave into batch BC-2's
stream, and its pt/y evacuations move ACT->DVE, to shorten the
pipeline-drain tail.  Batch-0 q/wq loads are issued in halves so the
first projection starts before the full 2MB lands.
"""

import math
import os

os.environ.setdefault("MYCRO_LOCAL_CACHE", "1")

from contextlib import ExitStack

import numpy as np

import concourse.bass as bass
import concourse.bacc as bacc
import concourse.mybir as mybir
import concourse.tile as tile
from concourse.bass_utils import run_bass_kernel_spmd

B, S, D, H = 32, 512, 512, 8
DK = D // H  # 64
NCORES = 8
BC = B // NCORES  # batches per core
RT = S // 128  # row tiles per sequence
FT = D // 128  # feature tiles
NEG = -1.0e32

F32 = mybir.dt.float32
BF16 = mybir.dt.bfloat16
F16 = mybir.dt.float16

_last_nc = None

CFG = {
    "trace": False,
    "mask_on_pe": True,   # bf16 identity-matmul mask vs DVE tensor add
}


def _build_program(k_index: int, has_bias: dict):
    """Builds the per-core Bass program."""
    nc = bacc.Bacc(
        "TRN2", target_bir_lowering=False, debug=False, num_devices=NCORES
    )

    # --- DRAM I/O -------------------------------------------------------
    # q/k in transposed layout, host-split into f16 hi/lo pairs and
    # host-pre-arranged as [128, FT*S] (ft-blocks side by side) so each
    # tensor loads with ONE wide DMA instead of FT strided ones.
    qhiT = nc.dram_tensor("qhiT", (BC, 128, FT * S), F16, kind="ExternalInput").ap()
    qloT = nc.dram_tensor("qloT", (BC, 128, FT * S), F16, kind="ExternalInput").ap()
    khiT = nc.dram_tensor("khiT", (BC, 128, FT * S), F16, kind="ExternalInput").ap()
    kloT = nc.dram_tensor("kloT", (BC, 128, FT * S), F16, kind="ExternalInput").ap()
    vT = nc.dram_tensor("vT", (BC, 128, FT * S), F16, kind="ExternalInput").ap()
    wqhi = nc.dram_tensor("wqhi", (128, FT * D), F16, kind="ExternalInput").ap()
    wqlo = nc.dram_tensor("wqlo", (128, FT * D), F16, kind="ExternalInput").ap()
    wkhi = nc.dram_tensor("wkhi", (128, FT * D), F16, kind="ExternalInput").ap()
    wklo = nc.dram_tensor("wklo", (128, FT * D), F16, kind="ExternalInput").ap()
    wv = nc.dram_tensor("wv", (128, FT * D), F16, kind="ExternalInput").ap()
    wo = nc.dram_tensor("wo", (128, FT * D), F16, kind="ExternalInput").ap()
    bias_aps = {}
    for name in ("bq", "bk", "bv", "bo"):
        if has_bias[name]:
            bias_aps[name] = nc.dram_tensor(
                name, (1, D), F32, kind="ExternalInput"
            ).ap()
    out = nc.dram_tensor("out", (BC, S, D), F32, kind="ExternalOutput").ap()

    # --- inline constants ----------------------------------------------
    ident_np = np.eye(128, dtype=np.float32)
    ident_p = nc.inline_tensor(
        ident_np.astype(mybir.dt.np(F16)), name="identp"
    ).ap()
    lt_np = (np.arange(128)[None, :] < np.arange(128)[:, None]).astype(
        mybir.dt.np(F16))
    lt_tri = nc.inline_tensor(lt_np, name="lttri").ap()
    ones_row = nc.inline_tensor(
        np.ones((1, S), dtype=np.float32), name="onesrow"
    ).ap()

    with tile.TileContext(nc) as tc, ExitStack() as ctx:
        # ---------------- pools ----------------
        consts = ctx.enter_context(tc.tile_pool(name="consts", bufs=1))
        xpool = ctx.enter_context(tc.tile_pool(name="xpool", bufs=2))
        projpool = ctx.enter_context(tc.tile_pool(name="projpool", bufs=2))
        epool = ctx.enter_context(tc.tile_pool(name="epool", bufs=20))
        pnpool = ctx.enter_context(tc.tile_pool(name="pnpool", bufs=12))
        rpool = ctx.enter_context(tc.tile_pool(name="rpool", bufs=10))
        ptpool = ctx.enter_context(tc.tile_pool(name="ptpool", bufs=12))
        smallpool = ctx.enter_context(tc.tile_pool(name="smallpool", bufs=4))
        atpool = ctx.enter_context(tc.tile_pool(name="atpool", bufs=3))
        ypool = ctx.enter_context(tc.tile_pool(name="ypool", bufs=3))

        ps_proj = ctx.enter_context(tc.tile_pool(name="ps_proj", bufs=2, space="PSUM"))
        ps_sc = ctx.enter_context(tc.tile_pool(name="ps_sc", bufs=3, space="PSUM"))
        ps_pt = ctx.enter_context(tc.tile_pool(name="ps_pt", bufs=1, space="PSUM"))
        ps_at = ctx.enter_context(tc.tile_pool(name="ps_at", bufs=1, space="PSUM"))
        ps_y = ctx.enter_context(tc.tile_pool(name="ps_y", bufs=1, space="PSUM"))

        # ---------------- resident constants ----------------
        # combined [128, FT*S] tiles: one wide DMA per tensor; q weights +
        # batch 0's q first so the first projection matmuls start earliest.
        HW = FT * D // 2
        wqh_sb = consts.tile([128, FT * D], F16, name="wqh")
        nc.sync.dma_start(wqh_sb[:, 0:HW], wqhi[:, 0:HW])
        _xq0h = xpool.tile([128, FT * S], F16, name="xqh", tag="xqh")
        nc.sync.dma_start(_xq0h[:, 0:HW], qhiT[0, :, 0:HW])
        wql_sb = consts.tile([128, FT * D], F16, name="wql")
        nc.sync.dma_start(wql_sb[:, 0:HW], wqlo[:, 0:HW])
        _xq0l = xpool.tile([128, FT * S], F16, name="xql", tag="xql")
        nc.sync.dma_start(_xq0l[:, 0:HW], qloT[0, :, 0:HW])
        nc.sync.dma_start(wqh_sb[:, HW:], wqhi[:, HW:])
        nc.sync.dma_start(_xq0h[:, HW:], qhiT[0, :, HW:])
        nc.sync.dma_start(wql_sb[:, HW:], wqlo[:, HW:])
        nc.sync.dma_start(_xq0l[:, HW:], qloT[0, :, HW:])
        _xq0 = (_xq0h, _xq0l)
        wkh_sb = consts.tile_from(wkhi, name="wkh")
        _xk0h = xpool.tile_from(khiT[0], name="xkh")
        wkl_sb = consts.tile_from(wklo, name="wkl")
        preloaded = {}
        preloaded[0] = (
            _xq0,
            (_xk0h, xpool.tile_from(kloT[0], name="xkl")),
            xpool.tile_from(vT[0], name="xv"),
        )
        wv_sb = consts.tile_from(wv, name="wv")
        wo_sb = consts.tile_from(wo, name="wo")
        identp_sb = consts.tile_from(ident_p, name="identp_sb")
        lt_sb = consts.tile_from(lt_tri, name="lt_sb")
        ones_sb = consts.tile_from(ones_row, name="ones_sb")
        bias_sb = {
            nm: consts.tile_from(ap, name=f"{nm}_sb") for nm, ap in bias_aps.items()
        }

        Exp = mybir.ActivationFunctionType.Exp
        AO = mybir.AluOpType

        def emit_proj(b, defer_v=False):
            """Loads + q/k/v projections for batch b.

            q/k: f16 pair-product accumulation (12 matmuls per output
            tile), evacuated as an f16 hi/lo split: hi via ACT copy,
            lo = psum - hi via DVE/Pool tensor_tensor subtract.
            """
            if b in preloaded:
                (xqh, xql), (xkh, xkl), xv = preloaded.pop(b)
            else:
                xqh = xpool.tile_from(qhiT[b], name="xqh")
                xql = xpool.tile_from(qloT[b], name="xql")
                xkh = xpool.tile_from(khiT[b], name="xkh")
                xkl = xpool.tile_from(kloT[b], name="xkl")
                xv = xpool.tile_from(vT[b], name="xv")
            qhT, khT, vh = [], [], []  # qhT/khT: list of (hi, lo) per dt
            # which-major: all q groups then all k groups, matching the DMA
            # arrival order (PE executes its queue in order; dt-major would
            # stall matmul #13 on the k loads, which land 4 transfers later)
            for which, whi_sb, wlo_sb, xh, xl, bkey, outl in (
                    ("q", wqh_sb, wql_sb, xqh, xql, "bq", qhT),
                    ("k", wkh_sb, wkl_sb, xkh, xkl, "bk", khT)):
                for dt in range(FT):
                    ps = ps_proj.tile([128, S], F32, name="psq", tag="psproj")
                    nbias = bkey in bias_sb
                    nmm = 3 * FT
                    i = 0
                    # term-major: the 4 hi*hi matmuls come first and only
                    # need the hi loads (startup: lo tensors still in flight)
                    for w_sb, xs in ((whi_sb, xh), (wlo_sb, xh),
                                     (whi_sb, xl)):
                        for ft in range(FT):
                            wsl = slice(ft * D + dt * 128,
                                        ft * D + (dt + 1) * 128)
                            xsl = slice(ft * S, (ft + 1) * S)
                            i += 1
                            nc.tensor.matmul(
                                ps, w_sb[:, wsl], xs[:, xsl],
                                start=(i == 1),
                                stop=(i == nmm and not nbias))
                    if nbias:
                        nc.tensor.matmul(
                            ps, bias_sb[bkey][0:1, dt * 128:(dt + 1) * 128],
                            ones_sb, start=False, stop=True)
                    thi = projpool.tile([128, S], F16, name=f"{which}hT{dt}h",
                                        tag=f"{which}hT{dt}h")
                    nc.scalar.copy(thi, ps)
                    tlo = projpool.tile([128, S], F16, name=f"{which}hT{dt}l",
                                        tag=f"{which}hT{dt}l")
                    # GPSIMD cannot access PSUM (walrus constraint): the
                    # latency-critical lo evac goes on DVE
                    nc.vector.tensor_tensor(tlo, ps, thi, op=AO.subtract)
                    outl.append((thi, tlo))

            def do_vproj(rts=range(RT)):
                for rt in rts:
                    ps = ps_proj.tile([128, D], F32, name="psv", tag="psproj")
                    nbias = "bv" in bias_sb
                    for ft in range(FT):
                        nc.tensor.matmul(
                            ps, xv[:, ft * S + rt * 128:ft * S + (rt + 1) * 128],
                            wv_sb[:, ft * D:(ft + 1) * D],
                            start=(ft == 0), stop=(ft == FT - 1 and not nbias))
                    if nbias:
                        nc.tensor.matmul(
                            ps, ones_sb[0:1, 0:128], bias_sb["bv"],
                            start=False, stop=True)
                    t = projpool.tile([128, D], F16, name=f"vh{rt}", tag=f"vh{rt}")
                    nc.scalar.copy(t, ps)
                    vh.append(t)
                return vh
            if defer_v:
                return qhT, khT, do_vproj
            return qhT, khT, do_vproj()

        def emit_headpair(hp, qhT, khT, vh, pt_dve=False, at_dve=False):
            """Scores / top-k softmax / normalized transpose / attnT for one
            head pair (partition halves 0:64 / 64:128 of the proj tiles)."""
            etiles = [[None] * RT, [None] * RT]
            zfulls = [None, None]
            top8s = []
            for hh in range(2):
                top8s.append(smallpool.tile(
                    [128, RT * 8], F32, name=f"top8{hh}", tag=f"top8{hh}"))
            qh_hi, qh_lo = qhT[hp]
            kh_hi, kh_lo = khT[hp]
            for ri in range(RT):
                w = (ri + 1) * 128
                spss = []
                for hh in range(2):
                    po = hh * 64
                    sps = ps_sc.tile([128, S], F32, name="sps", tag="sps")
                    rsl = slice(ri * 128, (ri + 1) * 128)
                    for i, (qt, kt) in enumerate((
                            (qh_hi, kh_hi), (qh_hi, kh_lo), (qh_lo, kh_hi))):
                        nc.tensor.matmul(
                            sps[:, 0:w],
                            qt[po:po + 64, rsl],
                            kt[po:po + 64, 0:w],
                            start=(i == 0), stop=(i == 2))
                    spss.append(sps)
                for hh in range(2):
                    e = epool.tile([128, S], F32, name="e", tag="e")
                    nc.scalar.activation(e[:, 0:w], spss[hh][:, 0:w], Exp)
                    # strict-causal mask applied post-exp on the diagonal
                    # block: e *= LT (0/1) on the otherwise-idle Pool engine
                    # (exp of unmasked scores is finite; x*0 == 0 exactly)
                    nc.gpsimd.tensor_tensor(
                        e[:, ri * 128:(ri + 1) * 128],
                        e[:, ri * 128:(ri + 1) * 128], lt_sb, op=AO.mult)
                    if ri == 0:
                        zf = smallpool.tile(
                            [128, 1], F32, name=f"zfull{hh}", tag=f"zfull{hh}")
                        zfulls[hh] = zf
                        nc.vector.reduce_sum(
                            zf, e[:, 0:w], axis=mybir.AxisListType.X)
                    nc.vector.max(
                        out=top8s[hh][:, ri * 8:(ri + 1) * 8], in_=e[:, 0:w])
                    etiles[hh][ri] = e
            ptrows = [[None] * RT, [None] * RT]
            rtiless, pnss = [], []
            for hh in range(2):
                top8 = top8s[hh]
                zk = smallpool.tile([128, RT], F32, name="zk", tag="zk")
                nc.vector.reduce_sum(
                    zk, top8.rearrange("p (r e) -> p r e", e=8)[:, :, 0:k_index],
                    axis=mybir.AxisListType.X)
                nc.vector.tensor_copy(zk[0:k_index, 0:1], zfulls[hh][0:k_index, :])
                nc.vector.memset(zk[0:1, 0:1], 1.0)
                # rows < k keep every valid entry: tau := 0
                nc.vector.memset(top8[0:k_index, k_index - 1:k_index], 0.0)
                rz = smallpool.tile([128, RT], F32, name="rz", tag="rz")
                nc.vector.reciprocal(rz, zk)

                # R[ri] = diag(rz[:, ri]) in f16: ACT copy-with-scale of
                # the identity
                rtiles = []
                for ri in range(RT):
                    R = rpool.tile([128, 128], F16, name="rdiag", tag="rdiag")
                    nc.gpsimd.tensor_scalar(
                        R, identp_sb, rz[:, ri:ri + 1], None, op0=AO.mult)
                    rtiles.append(R)
                if True:
                    rtiless.append(rtiles)

                # masked (unnormalized) probs, f16
                pns = []
                for ri in range(RT):
                    w = (ri + 1) * 128
                    e = etiles[hh][ri]
                    tau = top8[:, ri * 8 + k_index - 1: ri * 8 + k_index]
                    pn = pnpool.tile([128, S], F16, name="pn", tag="pn")
                    nc.vector.scalar_tensor_tensor(
                        pn[:, 0:w], e[:, 0:w], tau, e[:, 0:w],
                        op0=AO.is_ge, op1=AO.mult)
                    pns.append(pn)
                pnss.append(pns)

            # normalized transpose: ptb[c, r-block] = pn[r-block, c]^T
            # @ diag(rz) -- regular matmul, transpose + 1/Z in one op.
            # ci-major with heads alternating so attnT's ci-ordered
            # accumulation can start after the first two groups, not five.
            for ci in range(RT):
                for hh in range(2):
                    rtiles = rtiless[hh]
                    pns = pnss[hh]
                    wv_ = (RT - ci) * 128
                    ptb = ps_pt.tile([128, S], F32, name="ptb", tag="ptb")
                    for ri in range(ci, RT):
                        nc.tensor.matmul(
                            ptb[:, (ri - ci) * 128:(ri - ci + 1) * 128],
                            pns[ri][:, ci * 128:(ci + 1) * 128],
                            rtiles[ri], start=True, stop=True)
                    ptrow = ptpool.tile([128, S], F16, name="ptrow", tag="ptrow")
                    if pt_dve:
                        nc.vector.tensor_copy(ptrow[:, 0:wv_], ptb[:, 0:wv_])
                    else:
                        nc.scalar.copy(ptrow[:, 0:wv_], ptb[:, 0:wv_])
                    ptrows[hh][ci] = ptrow

            def finish(vh):
                at_ps = ps_at.tile([128, S], F32, name="atps", tag="atps")
                for ci in range(RT):
                    wv_ = (RT - ci) * 128
                    for hh in range(2):
                        h = 2 * hp + hh
                        po = hh * 64
                        nc.tensor.matmul(
                            at_ps[po:po + 64, ci * 128:S],
                            vh[ci][:, h * DK:(h + 1) * DK],
                            ptrows[hh][ci][:, 0:wv_],
                            start=(ci == 0), stop=(ci == RT - 1),
                            skip_group_check=True)
                at = atpool.tile([128, S], F16, name=f"at{hp}", tag=f"at{hp}")
                if at_dve:
                    nc.vector.tensor_copy(at, at_ps)
                else:
                    nc.scalar.copy(at, at_ps)
                return at
            if vh is None:
                return finish
            return finish(vh)

        def emit_y(b, attnT_sb, y_dve=False):
            for ri in range(RT):
                yps = ps_y.tile([128, D], F32, name="yps", tag="yps")
                nbias = "bo" in bias_sb
                for hp in range(FT):
                    nc.tensor.matmul(
                        yps, attnT_sb[hp][:, ri * 128:(ri + 1) * 128],
                        wo_sb[:, hp * D:(hp + 1) * D],
                        start=(hp == 0), stop=(hp == FT - 1 and not nbias))
                if nbias:
                    nc.tensor.matmul(
                        yps, ones_sb[0:1, 0:128], bias_sb["bo"],
                        start=False, stop=True)
                y = ypool.tile([128, D], F32, name="y", tag="y")
                if y_dve:
                    nc.vector.tensor_copy(y, yps)
                else:
                    nc.scalar.copy(y, yps)
                nc.scalar.dma_start(out[b, ri * 128:(ri + 1) * 128, :], y)

        # proj for batch b+1 is emitted between hp1 and hp2 of batch b
        # (latency-critical DVE lo-subtracts enqueue ahead of later head
        # pairs' DVE chains).  The LAST batch's head pairs are interleaved
        # into batch BC-2's stream so only two chains drain at the tail.
        projs = {0: emit_proj(0)}
        ats = {b: [] for b in range(BC)}
        for b in range(BC - 1):
            qhT, khT, vh = projs.pop(b)
            if b < BC - 2:
                for hp in range(FT):
                    ats[b].append(emit_headpair(hp, qhT, khT, vh))
                    if hp == 1:
                        projs[b + 1] = emit_proj(b + 1)
                emit_y(b, ats[b])
            else:
                # interleave tail: b2.hp0 b2.hp1 [proj3] b2.hp2 b3.hp0
                # b2.hp3 b3.hp1 y2 b3.hp2 b3.hp3 y3
                ats[b].append(emit_headpair(0, qhT, khT, vh))
                ats[b].append(emit_headpair(1, qhT, khT, vh))
                projs[b + 1] = emit_proj(b + 1)
                qhT3, khT3, vh3 = projs.pop(b + 1)
                ats[b].append(emit_headpair(2, qhT, khT, vh))
                ats[b + 1].append(emit_headpair(0, qhT3, khT3, vh3))
                ats[b].append(emit_headpair(3, qhT, khT, vh))
                ats[b + 1].append(emit_headpair(1, qhT3, khT3, vh3, pt_dve=True))
                emit_y(b, ats[b])
                ats[b + 1].append(emit_headpair(2, qhT3, khT3, vh3,
                                                 pt_dve=True, at_dve=True))
                ats[b + 1].append(emit_headpair(3, qhT3, khT3, vh3,
                                                 pt_dve=True, at_dve=True))
                emit_y(b + 1, ats[b + 1], y_dve=True)

    nc.compile()
    return nc


def _split16(x):
    """Split fp32 array into (hi, lo) f16 pair with hi + lo ~= x."""
    hi = x.astype(np.float16)
    lo = (x - hi.astype(np.float32)).astype(np.float16)
    return hi, lo


def kernel(**inputs):
    q = np.asarray(inputs["q"], np.float32)
    k = np.asarray(inputs["k"], np.float32)
    v = np.asarray(inputs["v"], np.float32)
    w_q = np.asarray(inputs["w_q"], np.float32)
    w_k = np.asarray(inputs["w_k"], np.float32)
    w_v = np.asarray(inputs["w_v"], np.float32)
    w_o = np.asarray(inputs["w_o"], np.float32)
    b_q = np.asarray(inputs["b_q"], np.float32)
    b_k = np.asarray(inputs["b_k"], np.float32)
    b_v = np.asarray(inputs["b_v"], np.float32)
    b_o = np.asarray(inputs["b_o"], np.float32)
    k_index = int(np.asarray(inputs["k_index"]))
    assert 1 <= k_index <= 8, f"kernel supports k_index<=8, got {k_index}"

    # fold the 1/sqrt(DK) score scaling into the q projection (exact: 2^-3)
    scale = np.float32(1.0 / math.sqrt(DK))
    w_qs = (w_q * scale).astype(np.float32)
    b_qs = (b_q * scale).astype(np.float32)

    has_bias = {
        "bq": bool(np.any(b_qs)),
        "bk": bool(np.any(b_k)),
        "bv": bool(np.any(b_v)),
        "bo": bool(np.any(b_o)),
    }

    nc = _build_program(k_index, has_bias)
    global _last_nc
    _last_nc = nc

    def _wide_w(w16):
        # [D, D] -> [128, FT*D]: ft-blocks of 128 rows laid side by side
        return np.ascontiguousarray(
            w16.reshape(FT, 128, D).transpose(1, 0, 2).reshape(128, FT * D))

    def _wide_x(x16):
        # [B', D, S] -> [B', 128, FT*S]
        bb = x16.shape[0]
        return np.ascontiguousarray(
            x16.reshape(bb, FT, 128, S).transpose(0, 2, 1, 3)
            .reshape(bb, 128, FT * S))

    wqhi, wqlo = _split16(w_qs)
    wkhi, wklo = _split16(w_k)
    shared = {
        "wqhi": _wide_w(wqhi),
        "wqlo": _wide_w(wqlo),
        "wkhi": _wide_w(wkhi),
        "wklo": _wide_w(wklo),
        "wv": _wide_w(w_v.astype(np.float16)),
        "wo": _wide_w(w_o.astype(np.float16)),
    }
    for nm, arr in (("bq", b_qs), ("bk", b_k), ("bv", b_v), ("bo", b_o)):
        if has_bias[nm]:
            shared[nm] = np.ascontiguousarray(arr.reshape(1, D).astype(np.float32))

    qT = q.transpose(0, 2, 1)
    kT = k.transpose(0, 2, 1)
    vTf = v.transpose(0, 2, 1).astype(np.float16)
    qhiT, qloT = _split16(qT)
    khiT, kloT = _split16(kT)

    in_maps = []
    for c in range(NCORES):
        sl = slice(c * BC, (c + 1) * BC)
        in_maps.append(dict(
            shared,
            qhiT=_wide_x(qhiT[sl]),
            qloT=_wide_x(qloT[sl]),
            khiT=_wide_x(khiT[sl]),
            kloT=_wide_x(kloT[sl]),
            vT=_wide_x(vTf[sl]),
        ))

    res = run_bass_kernel_spmd(
        nc, in_maps, core_ids=list(range(NCORES)), trace=CFG["trace"]
    )
    out = np.concatenate([r["out"] for r in res.results], axis=0)
    kernel.last_result = res
    return out


# revision 82
# speedup vs baseline: 1.1628x; 1.0012x over previous
"""Trainium2 Bass kernel for sparse (top-k) multi-head causal attention.

Problem (hardcoded shapes, from the reference):
  B=32, S=512, D=512, H=8, DK=64, k_index=5 (any k<=8 supported)
  out = TopKCausalAttention(q, k, v; w_q..w_o, b_q..b_o)

Sharding: data-parallel over batch across 8 NeuronCores (4 batches/core).

Numerics: the top-k selection is discontinuous, so scores need ~2^-16
relative accuracy vs the fp32 reference.  fp32 matmuls run at 4 cyc/row
on the PE; instead the q/k path uses f16 hi/lo PAIR arithmetic (3
matmuls at 1 cyc/row, ~2^-22 effective):
  q = qhi + qlo (host-split f16), w_q = whi + wlo (host-split f16)
  qh = qhi*whi + qhi*wlo + qlo*whi        (dropped qlo*wlo ~ 2^-22)
  qh -> (hi, lo) f16 evac split; scores = qhh*khh + qhh*khl + qhl*khh
Measured end-to-end rel err vs fp32 reference: ~2.7e-3 (gate 2e-2).
(float32r at 1 cyc/row was measured: its DMA/weight path quantizes to
11 mantissa bits -> rel err 1.6e-2, too close to the gate; and
engine-written f32r tiles load garbage as PE weights.)

Per-core algorithm (per batch b, head pair hp, heads hh=0,1):
  scores_psum[r-tile, 0:w] = 3 pair matmuls per head (upper tiles
      skipped; no mask matmul -- see below)
  e = exp(scores)                 (ACT, PSUM->SBUF)
  e[diag block] *= LT             (strict-causal mask applied POST-exp
                                   as a 0/1 lower-triangular multiply on
                                   the otherwise-idle Pool engine; exp of
                                   unmasked scores is finite and x*0 == 0,
                                   so the math is exact; frees 6.8us of
                                   PE identity-matmul mask work)
  zfull = row-sum of masked e at ri=0 (DVE reduce, for rows < k)
  top8 = vector.max(e)            (top-8 per row, one DVE op)
  tau = top8[:, k-1]; rows < k get tau := 0; Z = sum(top8[:, :k]) or
      full-row sum for rows < k; row 0: Z := 1
  pu = (e >= tau) * e             (DVE stt, f16 out; exact-by-value
                                   threshold, matching reference)
  R[ri] = diag(1/Z)               (f16, tensor_scalar identity * rz)
  ptb[c, r] = pu[r, c]^T @ R      (regular PE matmul: transpose AND
                                   1/Z normalization in one 1cyc/row op)
  attnT[d, r] += vh_ci^T @ ptrow_ci   (f16, triangular)
  y[r, :] = sum_hp attnT^T @ w_o (+ b_o) -> DRAM

Scheduling (vs the per-instruction cost model): PE is the bottleneck
(~202.5us busy of ~249us total).  Batch b+1's projections are emitted
between hp1/hp2 of batch b so their PSUM evacuations (ACT hi-copy +
DVE lo-subtract; GPSIMD cannot touch PSUM) never queue behind head-pair
DVE chains.  The last batch's head pairs interleave into batch BC-2's
stream, and its pt/y evacuations move ACT->DVE, to shorten the
pipeline-drain tail.  Batch-0 q/wq loads are issued in halves so the
first projection starts before the full 2MB lands.
"""

import math
import os

os.environ.setdefault("MYCRO_LOCAL_CACHE", "1")

from contextlib import ExitStack

import numpy as np

import concourse.bass as bass
import concourse.bacc as bacc
import concourse.mybir as mybir
import concourse.tile as tile
from concourse.bass_utils import run_bass_kernel_spmd

B, S, D, H = 32, 512, 512, 8
DK = D // H  # 64
NCORES = 8
BC = B // NCORES  # batches per core
RT = S // 128  # row tiles per sequence
FT = D // 128  # feature tiles
NEG = -1.0e32

F32 = mybir.dt.float32
BF16 = mybir.dt.bfloat16
F16 = mybir.dt.float16

_last_nc = None

CFG = {
    "trace": False,
    "mask_on_pe": True,   # bf16 identity-matmul mask vs DVE tensor add
}


def _build_program(k_index: int, has_bias: dict):
    """Builds the per-core Bass program."""
    nc = bacc.Bacc(
        "TRN2", target_bir_lowering=False, debug=False, num_devices=NCORES
    )

    # --- DRAM I/O -------------------------------------------------------
    # q/k in transposed layout, host-split into f16 hi/lo pairs and
    # host-pre-arranged as [128, FT*S] (ft-blocks side by side) so each
    # tensor loads with ONE wide DMA instead of FT strided ones.
    qhiT = nc.dram_tensor("qhiT", (BC, 128, FT * S), F16, kind="ExternalInput").ap()
    qloT = nc.dram_tensor("qloT", (BC, 128, FT * S), F16, kind="ExternalInput").ap()
    khiT = nc.dram_tensor("khiT", (BC, 128, FT * S), F16, kind="ExternalInput").ap()
    kloT = nc.dram_tensor("kloT", (BC, 128, FT * S), F16, kind="ExternalInput").ap()
    vT = nc.dram_tensor("vT", (BC, 128, FT * S), F16, kind="ExternalInput").ap()
    wqhi = nc.dram_tensor("wqhi", (128, FT * D), F16, kind="ExternalInput").ap()
    wqlo = nc.dram_tensor("wqlo", (128, FT * D), F16, kind="ExternalInput").ap()
    wkhi = nc.dram_tensor("wkhi", (128, FT * D), F16, kind="ExternalInput").ap()
    wklo = nc.dram_tensor("wklo", (128, FT * D), F16, kind="ExternalInput").ap()
    wv = nc.dram_tensor("wv", (128, FT * D), F16, kind="ExternalInput").ap()
    wo = nc.dram_tensor("wo", (128, FT * D), F16, kind="ExternalInput").ap()
    bias_aps = {}
    for name in ("bq", "bk", "bv", "bo"):
        if has_bias[name]:
            bias_aps[name] = nc.dram_tensor(
                name, (1, D), F32, kind="ExternalInput"
            ).ap()
    out = nc.dram_tensor("out", (BC, S, D), F32, kind="ExternalOutput").ap()

    # --- inline constants ----------------------------------------------
    ident_np = np.eye(128, dtype=np.float32)
    ident_p = nc.inline_tensor(
        ident_np.astype(mybir.dt.np(F16)), name="identp"
    ).ap()
    lt_np = (np.arange(128)[None, :] < np.arange(128)[:, None]).astype(
        mybir.dt.np(F16))
    lt_tri = nc.inline_tensor(lt_np, name="lttri").ap()
    ones_row = nc.inline_tensor(
        np.ones((1, S), dtype=np.float32), name="onesrow"
    ).ap()

    with tile.TileContext(nc) as tc, ExitStack() as ctx:
        # ---------------- pools ----------------
        consts = ctx.enter_context(tc.tile_pool(name="consts", bufs=1))
        xpool = ctx.enter_context(tc.tile_pool(name="xpool", bufs=2))
        projpool = ctx.enter_context(tc.tile_pool(name="projpool", bufs=2))
        epool = ctx.enter_context(tc.tile_pool(name="epool", bufs=20))
        pnpool = ctx.enter_context(tc.tile_pool(name="pnpool", bufs=12))
        rpool = ctx.enter_context(tc.tile_pool(name="rpool", bufs=10))
        ptpool = ctx.enter_context(tc.tile_pool(name="ptpool", bufs=12))
        smallpool = ctx.enter_context(tc.tile_pool(name="smallpool", bufs=4))
        atpool = ctx.enter_context(tc.tile_pool(name="atpool", bufs=3))
        ypool = ctx.enter_context(tc.tile_pool(name="ypool", bufs=3))

        ps_proj = ctx.enter_context(tc.tile_pool(name="ps_proj", bufs=2, space="PSUM"))
        ps_sc = ctx.enter_context(tc.tile_pool(name="ps_sc", bufs=3, space="PSUM"))
        ps_pt = ctx.enter_context(tc.tile_pool(name="ps_pt", bufs=1, space="PSUM"))
        ps_at = ctx.enter_context(tc.tile_pool(name="ps_at", bufs=1, space="PSUM"))
        ps_y = ctx.enter_context(tc.tile_pool(name="ps_y", bufs=1, space="PSUM"))

        # ---------------- resident constants ----------------
        # combined [128, FT*S] tiles: one wide DMA per tensor; q weights +
        # batch 0's q first so the first projection matmuls start earliest.
        HW = FT * D // 2
        wqh_sb = consts.tile([128, FT * D], F16, name="wqh")
        nc.sync.dma_start(wqh_sb[:, 0:HW], wqhi[:, 0:HW])
        _xq0h = xpool.tile([128, FT * S], F16, name="xqh", tag="xqh")
        nc.sync.dma_start(_xq0h[:, 0:HW], qhiT[0, :, 0:HW])
        wql_sb = consts.tile([128, FT * D], F16, name="wql")
        nc.sync.dma_start(wql_sb[:, 0:HW], wqlo[:, 0:HW])
        _xq0l = xpool.tile([128, FT * S], F16, name="xql", tag="xql")
        nc.sync.dma_start(_xq0l[:, 0:HW], qloT[0, :, 0:HW])
        nc.sync.dma_start(wqh_sb[:, HW:], wqhi[:, HW:])
        nc.sync.dma_start(_xq0h[:, HW:], qhiT[0, :, HW:])
        nc.sync.dma_start(wql_sb[:, HW:], wqlo[:, HW:])
        nc.sync.dma_start(_xq0l[:, HW:], qloT[0, :, HW:])
        _xq0 = (_xq0h, _xq0l)
        wkh_sb = consts.tile_from(wkhi, name="wkh")
        _xk0h = xpool.tile_from(khiT[0], name="xkh")
        wkl_sb = consts.tile_from(wklo, name="wkl")
        preloaded = {}
        preloaded[0] = (
            _xq0,
            (_xk0h, xpool.tile_from(kloT[0], name="xkl")),
            xpool.tile_from(vT[0], name="xv"),
        )
        wv_sb = consts.tile_from(wv, name="wv")
        wo_sb = consts.tile_from(wo, name="wo")
        identp_sb = consts.tile_from(ident_p, name="identp_sb")
        lt_sb = consts.tile_from(lt_tri, name="lt_sb")
        ones_sb = consts.tile_from(ones_row, name="ones_sb")
        bias_sb = {
            nm: consts.tile_from(ap, name=f"{nm}_sb") for nm, ap in bias_aps.items()
        }

        Exp = mybir.ActivationFunctionType.Exp
        AO = mybir.AluOpType

        def emit_proj(b, defer_v=False):
            """Loads + q/k/v projections for batch b.

            q/k: f16 pair-product accumulation (12 matmuls per output
            tile), evacuated as an f16 hi/lo split: hi via ACT copy,
            lo = psum - hi via DVE/Pool tensor_tensor subtract.
            """
            if b in preloaded:
                (xqh, xql), (xkh, xkl), xv = preloaded.pop(b)
            else:
                xqh = xpool.tile_from(qhiT[b], name="xqh")
                xql = xpool.tile_from(qloT[b], name="xql")
                xkh = xpool.tile_from(khiT[b], name="xkh")
                xkl = xpool.tile_from(kloT[b], name="xkl")
                xv = xpool.tile_from(vT[b], name="xv")
            qhT, khT, vh = [], [], []  # qhT/khT: list of (hi, lo) per dt
            # which-major: all q groups then all k groups, matching the DMA
            # arrival order (PE executes its queue in order; dt-major would
            # stall matmul #13 on the k loads, which land 4 transfers later)
            for which, whi_sb, wlo_sb, xh, xl, bkey, outl in (
                    ("q", wqh_sb, wql_sb, xqh, xql, "bq", qhT),
                    ("k", wkh_sb, wkl_sb, xkh, xkl, "bk", khT)):
                for dt in range(FT):
                    ps = ps_proj.tile([128, S], F32, name="psq", tag="psproj")
                    nbias = bkey in bias_sb
                    nmm = 3 * FT
                    i = 0
                    # term-major: the 4 hi*hi matmuls come first and only
                    # need the hi loads (startup: lo tensors still in flight)
                    for w_sb, xs in ((whi_sb, xh), (wlo_sb, xh),
                                     (whi_sb, xl)):
                        for ft in range(FT):
                            wsl = slice(ft * D + dt * 128,
                                        ft * D + (dt + 1) * 128)
                            xsl = slice(ft * S, (ft + 1) * S)
                            i += 1
                            nc.tensor.matmul(
                                ps, w_sb[:, wsl], xs[:, xsl],
                                start=(i == 1),
                                stop=(i == nmm and not nbias))
                    if nbias:
                        nc.tensor.matmul(
                            ps, bias_sb[bkey][0:1, dt * 128:(dt + 1) * 128],
                            ones_sb, start=False, stop=True)
                    thi = projpool.tile([128, S], F16, name=f"{which}hT{dt}h",
                                        tag=f"{which}hT{dt}h")
                    nc.scalar.copy(thi, ps)
                    tlo = projpool.tile([128, S], F16, name=f"{which}hT{dt}l",
                                        tag=f"{which}hT{dt}l")
                    # GPSIMD cannot access PSUM (walrus constraint): the
                    # latency-critical lo evac goes on DVE
                    nc.vector.tensor_tensor(tlo, ps, thi, op=AO.subtract)
                    outl.append((thi, tlo))

            def do_vproj(rts=range(RT)):
                for rt in rts:
                    ps = ps_proj.tile([128, D], F32, name="psv", tag="psproj")
                    nbias = "bv" in bias_sb
                    for ft in range(FT):
                        nc.tensor.matmul(
                            ps, xv[:, ft * S + rt * 128:ft * S + (rt + 1) * 128],
                            wv_sb[:, ft * D:(ft + 1) * D],
                            start=(ft == 0), stop=(ft == FT - 1 and not nbias))
                    if nbias:
                        nc.tensor.matmul(
                            ps, ones_sb[0:1, 0:128], bias_sb["bv"],
                            start=False, stop=True)
                    t = projpool.tile([128, D], F16, name=f"vh{rt}", tag=f"vh{rt}")
                    nc.scalar.copy(t, ps)
                    vh.append(t)
                return vh
            if defer_v:
                return qhT, khT, do_vproj
            return qhT, khT, do_vproj()

        def emit_headpair(hp, qhT, khT, vh, pt_dve=False, at_dve=False):
            """Scores / top-k softmax / normalized transpose / attnT for one
            head pair (partition halves 0:64 / 64:128 of the proj tiles)."""
            etiles = [[None] * RT, [None] * RT]
            zfulls = [None, None]
            top8s = []
            for hh in range(2):
                top8s.append(smallpool.tile(
                    [128, RT * 8], F32, name=f"top8{hh}", tag=f"top8{hh}"))
            qh_hi, qh_lo = qhT[hp]
            kh_hi, kh_lo = khT[hp]
            for ri in range(RT):
                w = (ri + 1) * 128
                spss = []
                for hh in range(2):
                    po = hh * 64
                    sps = ps_sc.tile([128, S], F32, name="sps", tag="sps")
                    rsl = slice(ri * 128, (ri + 1) * 128)
                    for i, (qt, kt) in enumerate((
                            (qh_hi, kh_hi), (qh_hi, kh_lo), (qh_lo, kh_hi))):
                        nc.tensor.matmul(
                            sps[:, 0:w],
                            qt[po:po + 64, rsl],
                            kt[po:po + 64, 0:w],
                            start=(i == 0), stop=(i == 2))
                    spss.append(sps)
                for hh in range(2):
                    e = epool.tile([128, S], F32, name="e", tag="e")
                    nc.scalar.activation(e[:, 0:w], spss[hh][:, 0:w], Exp)
                    # strict-causal mask applied post-exp on the diagonal
                    # block: e *= LT (0/1) on the otherwise-idle Pool engine
                    # (exp of unmasked scores is finite; x*0 == 0 exactly)
                    nc.gpsimd.tensor_tensor(
                        e[:, ri * 128:(ri + 1) * 128],
                        e[:, ri * 128:(ri + 1) * 128], lt_sb, op=AO.mult)
                    if ri == 0:
                        zf = smallpool.tile(
                            [128, 1], F32, name=f"zfull{hh}", tag=f"zfull{hh}")
                        zfulls[hh] = zf
                        nc.vector.reduce_sum(
                            zf, e[:, 0:w], axis=mybir.AxisListType.X)
                    nc.vector.max(
                        out=top8s[hh][:, ri * 8:(ri + 1) * 8], in_=e[:, 0:w])
                    etiles[hh][ri] = e
            ptrows = [[None] * RT, [None] * RT]
            rtiless, pnss = [], []
            for hh in range(2):
                top8 = top8s[hh]
                zk = smallpool.tile([128, RT], F32, name="zk", tag="zk")
                nc.vector.reduce_sum(
                    zk, top8.rearrange("p (r e) -> p r e", e=8)[:, :, 0:k_index],
                    axis=mybir.AxisListType.X)
                nc.vector.tensor_copy(zk[0:k_index, 0:1], zfulls[hh][0:k_index, :])
                nc.vector.memset(zk[0:1, 0:1], 1.0)
                # rows < k keep every valid entry: tau := 0
                nc.vector.memset(top8[0:k_index, k_index - 1:k_index], 0.0)
                rz = smallpool.tile([128, RT], F32, name="rz", tag="rz")
                nc.vector.reciprocal(rz, zk)

                # R[ri] = diag(rz[:, ri]) in f16: ACT copy-with-scale of
                # the identity
                rtiles = []
                for ri in range(RT):
                    R = rpool.tile([128, 128], F16, name="rdiag", tag="rdiag")
                    nc.gpsimd.tensor_scalar(
                        R, identp_sb, rz[:, ri:ri + 1], None, op0=AO.mult)
                    rtiles.append(R)
                if True:
                    rtiless.append(rtiles)

                # masked (unnormalized) probs, f16
                pns = []
                for ri in range(RT):
                    w = (ri + 1) * 128
                    e = etiles[hh][ri]
                    tau = top8[:, ri * 8 + k_index - 1: ri * 8 + k_index]
                    pn = pnpool.tile([128, S], F16, name="pn", tag="pn")
                    nc.vector.scalar_tensor_tensor(
                        pn[:, 0:w], e[:, 0:w], tau, e[:, 0:w],
                        op0=AO.is_ge, op1=AO.mult)
                    pns.append(pn)
                pnss.append(pns)

            # normalized transpose: ptb[c, r-block] = pn[r-block, c]^T
            # @ diag(rz) -- regular matmul, transpose + 1/Z in one op.
            # ci-major with heads alternating so attnT's ci-ordered
            # accumulation can start after the first two groups, not five.
            for ci in range(RT):
                for hh in range(2):
                    rtiles = rtiless[hh]
                    pns = pnss[hh]
                    wv_ = (RT - ci) * 128
                    ptb = ps_pt.tile([128, S], F32, name="ptb", tag="ptb")
                    for ri in range(ci, RT):
                        nc.tensor.matmul(
                            ptb[:, (ri - ci) * 128:(ri - ci + 1) * 128],
                            pns[ri][:, ci * 128:(ci + 1) * 128],
                            rtiles[ri], start=True, stop=True)
                    ptrow = ptpool.tile([128, S], F16, name="ptrow", tag="ptrow")
                    if pt_dve:
                        nc.vector.tensor_copy(ptrow[:, 0:wv_], ptb[:, 0:wv_])
                    else:
                        nc.scalar.copy(ptrow[:, 0:wv_], ptb[:, 0:wv_])
                    ptrows[hh][ci] = ptrow

            def finish(vh):
                at_ps = ps_at.tile([128, S], F32, name="atps", tag="atps")
                for ci in range(RT):
                    wv_ = (RT - ci) * 128
                    for hh in range(2):
                        h = 2 * hp + hh
                        po = hh * 64
                        nc.tensor.matmul(
                            at_ps[po:po + 64, ci * 128:S],
                            vh[ci][:, h * DK:(h + 1) * DK],
                            ptrows[hh][ci][:, 0:wv_],
                            start=(ci == 0), stop=(ci == RT - 1),
                            skip_group_check=True)
                at = atpool.tile([128, S], F16, name=f"at{hp}", tag=f"at{hp}")
                if at_dve:
                    nc.vector.tensor_copy(at, at_ps)
                else:
                    nc.scalar.copy(at, at_ps)
                return at
            if vh is None:
                return finish
            return finish(vh)

        def emit_y(b, attnT_sb, y_dve=False):
            for ri in range(RT):
                # alternate psum banks with ps_at (idle during y: all attnT
                # accumulations precede y) -> ri+1 accumulates while ri
                # evacuates
                if ri % 2 == 0:
                    yps = ps_y.tile([128, D], F32, name="yps", tag="yps")
                else:
                    yps = ps_at.tile([128, D], F32, name="yps", tag="atps")
                nbias = "bo" in bias_sb
                for hp in range(FT):
                    nc.tensor.matmul(
                        yps, attnT_sb[hp][:, ri * 128:(ri + 1) * 128],
                        wo_sb[:, hp * D:(hp + 1) * D],
                        start=(hp == 0), stop=(hp == FT - 1 and not nbias))
                if nbias:
                    nc.tensor.matmul(
                        yps, ones_sb[0:1, 0:128], bias_sb["bo"],
                        start=False, stop=True)
                y = ypool.tile([128, D], F32, name="y", tag="y")
                if y_dve:
                    nc.vector.tensor_copy(y, yps)
                else:
                    nc.scalar.copy(y, yps)
                nc.scalar.dma_start(out[b, ri * 128:(ri + 1) * 128, :], y)

        # proj for batch b+1 is emitted between hp1 and hp2 of batch b
        # (latency-critical DVE lo-subtracts enqueue ahead of later head
        # pairs' DVE chains).  The LAST batch's head pairs are interleaved
        # into batch BC-2's stream so only two chains drain at the tail.
        projs = {0: emit_proj(0)}
        ats = {b: [] for b in range(BC)}
        for b in range(BC - 1):
            qhT, khT, vh = projs.pop(b)
            if b < BC - 2:
                for hp in range(FT):
                    ats[b].append(emit_headpair(hp, qhT, khT, vh))
                    if hp == 1:
                        projs[b + 1] = emit_proj(b + 1)
                emit_y(b, ats[b])
            else:
                # interleave tail: b2.hp0 b2.hp1 [proj3] b2.hp2 b3.hp0
                # b2.hp3 b3.hp1 y2 b3.hp2 b3.hp3 y3
                ats[b].append(emit_headpair(0, qhT, khT, vh))
                ats[b].append(emit_headpair(1, qhT, khT, vh))
                projs[b + 1] = emit_proj(b + 1)
                qhT3, khT3, vh3 = projs.pop(b + 1)
                ats[b].append(emit_headpair(2, qhT, khT, vh))
                ats[b + 1].append(emit_headpair(0, qhT3, khT3, vh3))
                ats[b].append(emit_headpair(3, qhT, khT, vh))
                ats[b + 1].append(emit_headpair(1, qhT3, khT3, vh3, pt_dve=True))
                emit_y(b, ats[b])
                ats[b + 1].append(emit_headpair(2, qhT3, khT3, vh3,
                                                 pt_dve=True, at_dve=True))
                ats[b + 1].append(emit_headpair(3, qhT3, khT3, vh3,
                                                 pt_dve=True, at_dve=True))
                emit_y(b + 1, ats[b + 1], y_dve=True)

    nc.compile()
    return nc


def _split16(x):
    """Split fp32 array into (hi, lo) f16 pair with hi + lo ~= x."""
    hi = x.astype(np.float16)
    lo = (x - hi.astype(np.float32)).astype(np.float16)
    return hi, lo


def kernel(**inputs):
    q = np.asarray(inputs["q"], np.float32)
    k = np.asarray(inputs["k"], np.float32)
    v = np.asarray(inputs["v"], np.float32)
    w_q = np.asarray(inputs["w_q"], np.float32)
    w_k = np.asarray(inputs["w_k"], np.float32)
    w_v = np.asarray(inputs["w_v"], np.float32)
    w_o = np.asarray(inputs["w_o"], np.float32)
    b_q = np.asarray(inputs["b_q"], np.float32)
    b_k = np.asarray(inputs["b_k"], np.float32)
    b_v = np.asarray(inputs["b_v"], np.float32)
    b_o = np.asarray(inputs["b_o"], np.float32)
    k_index = int(np.asarray(inputs["k_index"]))
    assert 1 <= k_index <= 8, f"kernel supports k_index<=8, got {k_index}"

    # fold the 1/sqrt(DK) score scaling into the q projection (exact: 2^-3)
    scale = np.float32(1.0 / math.sqrt(DK))
    w_qs = (w_q * scale).astype(np.float32)
    b_qs = (b_q * scale).astype(np.float32)

    has_bias = {
        "bq": bool(np.any(b_qs)),
        "bk": bool(np.any(b_k)),
        "bv": bool(np.any(b_v)),
        "bo": bool(np.any(b_o)),
    }

    nc = _build_program(k_index, has_bias)
    global _last_nc
    _last_nc = nc

    def _wide_w(w16):
        # [D, D] -> [128, FT*D]: ft-blocks of 128 rows laid side by side
        return np.ascontiguousarray(
            w16.reshape(FT, 128, D).transpose(1, 0, 2).reshape(128, FT * D))

    def _wide_x(x16):
        # [B', D, S] -> [B', 128, FT*S]
        bb = x16.shape[0]
        return np.ascontiguousarray(
            x16.reshape(bb, FT, 128, S).transpose(0, 2, 1, 3)
            .reshape(bb, 128, FT * S))

    wqhi, wqlo = _split16(w_qs)
    wkhi, wklo = _split16(w_k)
    shared = {
        "wqhi": _wide_w(wqhi),
        "wqlo": _wide_w(wqlo),
        "wkhi": _wide_w(wkhi),
        "wklo": _wide_w(wklo),
        "wv": _wide_w(w_v.astype(np.float16)),
        "wo": _wide_w(w_o.astype(np.float16)),
    }
    for nm, arr in (("bq", b_qs), ("bk", b_k), ("bv", b_v), ("bo", b_o)):
        if has_bias[nm]:
            shared[nm] = np.ascontiguousarray(arr.reshape(1, D).astype(np.float32))

    qT = q.transpose(0, 2, 1)
    kT = k.transpose(0, 2, 1)
    vTf = v.transpose(0, 2, 1).astype(np.float16)
    qhiT, qloT = _split16(qT)
    khiT, kloT = _split16(kT)

    in_maps = []
    for c in range(NCORES):
        sl = slice(c * BC, (c + 1) * BC)
        in_maps.append(dict(
            shared,
            qhiT=_wide_x(qhiT[sl]),
            qloT=_wide_x(qloT[sl]),
            khiT=_wide_x(khiT[sl]),
            kloT=_wide_x(kloT[sl]),
            vT=_wide_x(vTf[sl]),
        ))

    res = run_bass_kernel_spmd(
        nc, in_maps, core_ids=list(range(NCORES)), trace=CFG["trace"]
    )
    out = np.concatenate([r["out"] for r in res.results], axis=0)
    kernel.last_result = res
    return out


# revision 89
# speedup vs baseline: 1.1641x; 1.0011x over previous
"""Trainium2 Bass kernel for sparse (top-k) multi-head causal attention.

Problem (hardcoded shapes, from the reference):
  B=32, S=512, D=512, H=8, DK=64, k_index=5 (any k<=8 supported)
  out = TopKCausalAttention(q, k, v; w_q..w_o, b_q..b_o)

Sharding: data-parallel over batch across 8 NeuronCores (4 batches/core).

Numerics: the top-k selection is discontinuous, so scores need ~2^-16
relative accuracy vs the fp32 reference.  fp32 matmuls run at 4 cyc/row
on the PE; instead the q/k path uses f16 hi/lo PAIR arithmetic (3
matmuls at 1 cyc/row, ~2^-22 effective):
  q = qhi + qlo (host-split f16), w_q = whi + wlo (host-split f16)
  qh = qhi*whi + qhi*wlo + qlo*whi        (dropped qlo*wlo ~ 2^-22)
  qh -> (hi, lo) f16 evac split; scores = qhh*khh + qhh*khl + qhl*khh
Measured end-to-end rel err vs fp32 reference: ~2.7e-3 (gate 2e-2).
(float32r at 1 cyc/row was measured: its DMA/weight path quantizes to
11 mantissa bits -> rel err 1.6e-2, too close to the gate; and
engine-written f32r tiles load garbage as PE weights.)

Per-core algorithm (per batch b, head pair hp, heads hh=0,1):
  scores_psum[r-tile, 0:w] = 3 pair matmuls per head (upper tiles
      skipped; no mask matmul -- see below)
  e = exp(scores)                 (ACT, PSUM->SBUF)
  e[diag block] *= LT             (strict-causal mask applied POST-exp
                                   as a 0/1 lower-triangular multiply on
                                   the otherwise-idle Pool engine; exp of
                                   unmasked scores is finite and x*0 == 0,
                                   so the math is exact; frees 6.8us of
                                   PE identity-matmul mask work)
  zfull = row-sum of masked e at ri=0 (DVE reduce, for rows < k)
  top8 = vector.max(e)            (top-8 per row, one DVE op)
  tau = top8[:, k-1]; rows < k get tau := 0; Z = sum(top8[:, :k]) or
      full-row sum for rows < k; row 0: Z := 1
  pu = (e >= tau) * e             (DVE stt, f16 out; exact-by-value
                                   threshold, matching reference)
  R[ri] = diag(1/Z)               (f16, tensor_scalar identity * rz)
  ptb[c, r] = pu[r, c]^T @ R      (regular PE matmul: transpose AND
                                   1/Z normalization in one 1cyc/row op)
  attnT[d, r] += vh_ci^T @ ptrow_ci   (f16, triangular)
  y[r, :] = sum_hp attnT^T @ w_o (+ b_o) -> DRAM

Scheduling (vs the per-instruction cost model): PE is the bottleneck
(~202.5us busy of ~249us total).  Batch b+1's projections are emitted
between hp1/hp2 of batch b so their PSUM evacuations (ACT hi-copy +
DVE lo-subtract; GPSIMD cannot touch PSUM) never queue behind head-pair
DVE chains.  The last batch's head pairs interleave into batch BC-2's
stream, and its pt/y evacuations move ACT->DVE, to shorten the
pipeline-drain tail.  Batch-0 q/wq loads are issued in halves so the
first projection starts before the full 2MB lands.
"""

import math
import os

os.environ.setdefault("MYCRO_LOCAL_CACHE", "1")

from contextlib import ExitStack

import numpy as np

import concourse.bass as bass
import concourse.bacc as bacc
import concourse.mybir as mybir
import concourse.tile as tile
from concourse.bass_utils import run_bass_kernel_spmd

B, S, D, H = 32, 512, 512, 8
DK = D // H  # 64
NCORES = 8
BC = B // NCORES  # batches per core
RT = S // 128  # row tiles per sequence
FT = D // 128  # feature tiles
NEG = -1.0e32

F32 = mybir.dt.float32
BF16 = mybir.dt.bfloat16
F16 = mybir.dt.float16

_last_nc = None

CFG = {
    "trace": False,
    "mask_on_pe": True,   # bf16 identity-matmul mask vs DVE tensor add
}


def _build_program(k_index: int, has_bias: dict):
    """Builds the per-core Bass program."""
    nc = bacc.Bacc(
        "TRN2", target_bir_lowering=False, debug=False, num_devices=NCORES
    )

    # --- DRAM I/O -------------------------------------------------------
    # q/k in transposed layout, host-split into f16 hi/lo pairs and
    # host-pre-arranged as [128, FT*S] (ft-blocks side by side) so each
    # tensor loads with ONE wide DMA instead of FT strided ones.
    qhiT = nc.dram_tensor("qhiT", (BC, 128, FT * S), F16, kind="ExternalInput").ap()
    qloT = nc.dram_tensor("qloT", (BC, 128, FT * S), F16, kind="ExternalInput").ap()
    khiT = nc.dram_tensor("khiT", (BC, 128, FT * S), F16, kind="ExternalInput").ap()
    kloT = nc.dram_tensor("kloT", (BC, 128, FT * S), F16, kind="ExternalInput").ap()
    vT = nc.dram_tensor("vT", (BC, 128, FT * S), F16, kind="ExternalInput").ap()
    wqhi = nc.dram_tensor("wqhi", (128, FT * D), F16, kind="ExternalInput").ap()
    wqlo = nc.dram_tensor("wqlo", (128, FT * D), F16, kind="ExternalInput").ap()
    wkhi = nc.dram_tensor("wkhi", (128, FT * D), F16, kind="ExternalInput").ap()
    wklo = nc.dram_tensor("wklo", (128, FT * D), F16, kind="ExternalInput").ap()
    wv = nc.dram_tensor("wv", (128, FT * D), F16, kind="ExternalInput").ap()
    wo = nc.dram_tensor("wo", (128, FT * D), F16, kind="ExternalInput").ap()
    bias_aps = {}
    for name in ("bq", "bk", "bv", "bo"):
        if has_bias[name]:
            bias_aps[name] = nc.dram_tensor(
                name, (1, D), F32, kind="ExternalInput"
            ).ap()
    out = nc.dram_tensor("out", (BC, S, D), F32, kind="ExternalOutput").ap()

    # --- inline constants ----------------------------------------------
    ident_np = np.eye(128, dtype=np.float32)
    ident_p = nc.inline_tensor(
        ident_np.astype(mybir.dt.np(F16)), name="identp"
    ).ap()
    lt_np = (np.arange(128)[None, :] < np.arange(128)[:, None]).astype(
        mybir.dt.np(F16))
    lt_tri = nc.inline_tensor(lt_np, name="lttri").ap()
    ones_row = nc.inline_tensor(
        np.ones((1, S), dtype=np.float32), name="onesrow"
    ).ap()

    with tile.TileContext(nc) as tc, ExitStack() as ctx:
        # ---------------- pools ----------------
        consts = ctx.enter_context(tc.tile_pool(name="consts", bufs=1))
        xpool = ctx.enter_context(tc.tile_pool(name="xpool", bufs=2))
        projpool = ctx.enter_context(tc.tile_pool(name="projpool", bufs=2))
        epool = ctx.enter_context(tc.tile_pool(name="epool", bufs=20))
        pnpool = ctx.enter_context(tc.tile_pool(name="pnpool", bufs=13))
        rpool = ctx.enter_context(tc.tile_pool(name="rpool", bufs=10))
        ptpool = ctx.enter_context(tc.tile_pool(name="ptpool", bufs=12))
        smallpool = ctx.enter_context(tc.tile_pool(name="smallpool", bufs=4))
        atpool = ctx.enter_context(tc.tile_pool(name="atpool", bufs=3))
        ypool = ctx.enter_context(tc.tile_pool(name="ypool", bufs=3))

        ps_proj = ctx.enter_context(tc.tile_pool(name="ps_proj", bufs=2, space="PSUM"))
        ps_sc = ctx.enter_context(tc.tile_pool(name="ps_sc", bufs=3, space="PSUM"))
        ps_pt = ctx.enter_context(tc.tile_pool(name="ps_pt", bufs=1, space="PSUM"))
        ps_at = ctx.enter_context(tc.tile_pool(name="ps_at", bufs=1, space="PSUM"))
        ps_y = ctx.enter_context(tc.tile_pool(name="ps_y", bufs=1, space="PSUM"))

        # ---------------- resident constants ----------------
        # combined [128, FT*S] tiles: one wide DMA per tensor; q weights +
        # batch 0's q first so the first projection matmuls start earliest.
        HW = FT * D // 2
        wqh_sb = consts.tile([128, FT * D], F16, name="wqh")
        nc.sync.dma_start(wqh_sb[:, 0:HW], wqhi[:, 0:HW])
        _xq0h = xpool.tile([128, FT * S], F16, name="xqh", tag="xqh")
        nc.sync.dma_start(_xq0h[:, 0:HW], qhiT[0, :, 0:HW])
        wql_sb = consts.tile([128, FT * D], F16, name="wql")
        nc.sync.dma_start(wql_sb[:, 0:HW], wqlo[:, 0:HW])
        _xq0l = xpool.tile([128, FT * S], F16, name="xql", tag="xql")
        nc.sync.dma_start(_xq0l[:, 0:HW], qloT[0, :, 0:HW])
        nc.sync.dma_start(wqh_sb[:, HW:], wqhi[:, HW:])
        nc.sync.dma_start(_xq0h[:, HW:], qhiT[0, :, HW:])
        nc.sync.dma_start(wql_sb[:, HW:], wqlo[:, HW:])
        nc.sync.dma_start(_xq0l[:, HW:], qloT[0, :, HW:])
        _xq0 = (_xq0h, _xq0l)
        wkh_sb = consts.tile_from(wkhi, name="wkh")
        _xk0h = xpool.tile_from(khiT[0], name="xkh")
        wkl_sb = consts.tile_from(wklo, name="wkl")
        preloaded = {}
        preloaded[0] = (
            _xq0,
            (_xk0h, xpool.tile_from(kloT[0], name="xkl")),
            xpool.tile_from(vT[0], name="xv"),
        )
        wv_sb = consts.tile_from(wv, name="wv")
        wo_sb = consts.tile_from(wo, name="wo")
        identp_sb = consts.tile_from(ident_p, name="identp_sb")
        lt_sb = consts.tile_from(lt_tri, name="lt_sb")
        ones_sb = consts.tile_from(ones_row, name="ones_sb")
        bias_sb = {
            nm: consts.tile_from(ap, name=f"{nm}_sb") for nm, ap in bias_aps.items()
        }

        Exp = mybir.ActivationFunctionType.Exp
        AO = mybir.AluOpType

        def emit_proj(b, defer_v=False):
            """Loads + q/k/v projections for batch b.

            q/k: f16 pair-product accumulation (12 matmuls per output
            tile), evacuated as an f16 hi/lo split: hi via ACT copy,
            lo = psum - hi via DVE/Pool tensor_tensor subtract.
            """
            if b in preloaded:
                (xqh, xql), (xkh, xkl), xv = preloaded.pop(b)
            else:
                xqh = xpool.tile_from(qhiT[b], name="xqh")
                xql = xpool.tile_from(qloT[b], name="xql")
                xkh = xpool.tile_from(khiT[b], name="xkh")
                xkl = xpool.tile_from(kloT[b], name="xkl")
                xv = xpool.tile_from(vT[b], name="xv")
            qhT, khT, vh = [], [], []  # qhT/khT: list of (hi, lo) per dt
            # which-major: all q groups then all k groups, matching the DMA
            # arrival order (PE executes its queue in order; dt-major would
            # stall matmul #13 on the k loads, which land 4 transfers later)
            for which, whi_sb, wlo_sb, xh, xl, bkey, outl in (
                    ("q", wqh_sb, wql_sb, xqh, xql, "bq", qhT),
                    ("k", wkh_sb, wkl_sb, xkh, xkl, "bk", khT)):
                for dt in range(FT):
                    ps = ps_proj.tile([128, S], F32, name="psq", tag="psproj")
                    nbias = bkey in bias_sb
                    nmm = 3 * FT
                    i = 0
                    # term-major: the 4 hi*hi matmuls come first and only
                    # need the hi loads (startup: lo tensors still in flight)
                    for w_sb, xs in ((whi_sb, xh), (wlo_sb, xh),
                                     (whi_sb, xl)):
                        for ft in range(FT):
                            wsl = slice(ft * D + dt * 128,
                                        ft * D + (dt + 1) * 128)
                            xsl = slice(ft * S, (ft + 1) * S)
                            i += 1
                            nc.tensor.matmul(
                                ps, w_sb[:, wsl], xs[:, xsl],
                                start=(i == 1),
                                stop=(i == nmm and not nbias))
                    if nbias:
                        nc.tensor.matmul(
                            ps, bias_sb[bkey][0:1, dt * 128:(dt + 1) * 128],
                            ones_sb, start=False, stop=True)
                    thi = projpool.tile([128, S], F16, name=f"{which}hT{dt}h",
                                        tag=f"{which}hT{dt}h")
                    nc.scalar.copy(thi, ps)
                    tlo = projpool.tile([128, S], F16, name=f"{which}hT{dt}l",
                                        tag=f"{which}hT{dt}l")
                    # GPSIMD cannot access PSUM (walrus constraint): the
                    # latency-critical lo evac goes on DVE
                    nc.vector.tensor_tensor(tlo, ps, thi, op=AO.subtract)
                    outl.append((thi, tlo))

            def do_vproj(rts=range(RT)):
                for rt in rts:
                    ps = ps_proj.tile([128, D], F32, name="psv", tag="psproj")
                    nbias = "bv" in bias_sb
                    for ft in range(FT):
                        nc.tensor.matmul(
                            ps, xv[:, ft * S + rt * 128:ft * S + (rt + 1) * 128],
                            wv_sb[:, ft * D:(ft + 1) * D],
                            start=(ft == 0), stop=(ft == FT - 1 and not nbias))
                    if nbias:
                        nc.tensor.matmul(
                            ps, ones_sb[0:1, 0:128], bias_sb["bv"],
                            start=False, stop=True)
                    t = projpool.tile([128, D], F16, name=f"vh{rt}", tag=f"vh{rt}")
                    nc.scalar.copy(t, ps)
                    vh.append(t)
                return vh
            if defer_v:
                return qhT, khT, do_vproj
            return qhT, khT, do_vproj()

        def emit_headpair(hp, qhT, khT, vh, pt_dve=False, at_dve=False):
            """Scores / top-k softmax / normalized transpose / attnT for one
            head pair (partition halves 0:64 / 64:128 of the proj tiles)."""
            etiles = [[None] * RT, [None] * RT]
            zfulls = [None, None]
            top8s = []
            for hh in range(2):
                top8s.append(smallpool.tile(
                    [128, RT * 8], F32, name=f"top8{hh}", tag=f"top8{hh}"))
            qh_hi, qh_lo = qhT[hp]
            kh_hi, kh_lo = khT[hp]
            for ri in range(RT):
                w = (ri + 1) * 128
                spss = []
                for hh in range(2):
                    po = hh * 64
                    sps = ps_sc.tile([128, S], F32, name="sps", tag="sps")
                    rsl = slice(ri * 128, (ri + 1) * 128)
                    for i, (qt, kt) in enumerate((
                            (qh_hi, kh_hi), (qh_hi, kh_lo), (qh_lo, kh_hi))):
                        nc.tensor.matmul(
                            sps[:, 0:w],
                            qt[po:po + 64, rsl],
                            kt[po:po + 64, 0:w],
                            start=(i == 0), stop=(i == 2))
                    spss.append(sps)
                for hh in range(2):
                    e = epool.tile([128, S], F32, name="e", tag="e")
                    nc.scalar.activation(e[:, 0:w], spss[hh][:, 0:w], Exp)
                    # strict-causal mask applied post-exp on the diagonal
                    # block: e *= LT (0/1) on the otherwise-idle Pool engine
                    # (exp of unmasked scores is finite; x*0 == 0 exactly)
                    nc.gpsimd.tensor_tensor(
                        e[:, ri * 128:(ri + 1) * 128],
                        e[:, ri * 128:(ri + 1) * 128], lt_sb, op=AO.mult)
                    if ri == 0:
                        zf = smallpool.tile(
                            [128, 1], F32, name=f"zfull{hh}", tag=f"zfull{hh}")
                        zfulls[hh] = zf
                        nc.vector.reduce_sum(
                            zf, e[:, 0:w], axis=mybir.AxisListType.X)
                    nc.vector.max(
                        out=top8s[hh][:, ri * 8:(ri + 1) * 8], in_=e[:, 0:w])
                    etiles[hh][ri] = e
            ptrows = [[None] * RT, [None] * RT]
            rtiless, pnss = [], []
            for hh in range(2):
                top8 = top8s[hh]
                zk = smallpool.tile([128, RT], F32, name="zk", tag="zk")
                nc.vector.reduce_sum(
                    zk, top8.rearrange("p (r e) -> p r e", e=8)[:, :, 0:k_index],
                    axis=mybir.AxisListType.X)
                nc.vector.tensor_copy(zk[0:k_index, 0:1], zfulls[hh][0:k_index, :])
                nc.vector.memset(zk[0:1, 0:1], 1.0)
                # rows < k keep every valid entry: tau := 0
                nc.vector.memset(top8[0:k_index, k_index - 1:k_index], 0.0)
                rz = smallpool.tile([128, RT], F32, name="rz", tag="rz")
                nc.vector.reciprocal(rz, zk)

                # R[ri] = diag(rz[:, ri]) in f16: ACT copy-with-scale of
                # the identity
                rtiles = []
                for ri in range(RT):
                    R = rpool.tile([128, 128], F16, name="rdiag", tag="rdiag")
                    nc.gpsimd.tensor_scalar(
                        R, identp_sb, rz[:, ri:ri + 1], None, op0=AO.mult)
                    rtiles.append(R)
                if True:
                    rtiless.append(rtiles)

                # masked (unnormalized) probs, f16
                pns = []
                for ri in range(RT):
                    w = (ri + 1) * 128
                    e = etiles[hh][ri]
                    tau = top8[:, ri * 8 + k_index - 1: ri * 8 + k_index]
                    pn = pnpool.tile([128, S], F16, name="pn", tag="pn")
                    nc.vector.scalar_tensor_tensor(
                        pn[:, 0:w], e[:, 0:w], tau, e[:, 0:w],
                        op0=AO.is_ge, op1=AO.mult)
                    pns.append(pn)
                pnss.append(pns)

            # normalized transpose: ptb[c, r-block] = pn[r-block, c]^T
            # @ diag(rz) -- regular matmul, transpose + 1/Z in one op.
            # ci-major with heads alternating so attnT's ci-ordered
            # accumulation can start after the first two groups, not five.
            for ci in range(RT):
                for hh in range(2):
                    rtiles = rtiless[hh]
                    pns = pnss[hh]
                    wv_ = (RT - ci) * 128
                    ptb = ps_pt.tile([128, S], F32, name="ptb", tag="ptb")
                    for ri in range(ci, RT):
                        nc.tensor.matmul(
                            ptb[:, (ri - ci) * 128:(ri - ci + 1) * 128],
                            pns[ri][:, ci * 128:(ci + 1) * 128],
                            rtiles[ri], start=True, stop=True)
                    ptrow = ptpool.tile([128, S], F16, name="ptrow", tag="ptrow")
                    if pt_dve:
                        nc.vector.tensor_copy(ptrow[:, 0:wv_], ptb[:, 0:wv_])
                    else:
                        nc.scalar.copy(ptrow[:, 0:wv_], ptb[:, 0:wv_])
                    ptrows[hh][ci] = ptrow

            def finish(vh):
                at_ps = ps_at.tile([128, S], F32, name="atps", tag="atps")
                for ci in range(RT):
                    wv_ = (RT - ci) * 128
                    for hh in range(2):
                        h = 2 * hp + hh
                        po = hh * 64
                        nc.tensor.matmul(
                            at_ps[po:po + 64, ci * 128:S],
                            vh[ci][:, h * DK:(h + 1) * DK],
                            ptrows[hh][ci][:, 0:wv_],
                            start=(ci == 0), stop=(ci == RT - 1),
                            skip_group_check=True)
                at = atpool.tile([128, S], F16, name=f"at{hp}", tag=f"at{hp}")
                if at_dve:
                    nc.vector.tensor_copy(at, at_ps)
                else:
                    nc.scalar.copy(at, at_ps)
                return at
            if vh is None:
                return finish
            return finish(vh)

        def emit_y(b, attnT_sb, y_dve=False):
            for ri in range(RT):
                # alternate psum banks with ps_at (idle during y: all attnT
                # accumulations precede y) -> ri+1 accumulates while ri
                # evacuates
                if ri % 2 == 0:
                    yps = ps_y.tile([128, D], F32, name="yps", tag="yps")
                else:
                    yps = ps_at.tile([128, D], F32, name="yps", tag="atps")
                nbias = "bo" in bias_sb
                for hp in range(FT):
                    nc.tensor.matmul(
                        yps, attnT_sb[hp][:, ri * 128:(ri + 1) * 128],
                        wo_sb[:, hp * D:(hp + 1) * D],
                        start=(hp == 0), stop=(hp == FT - 1 and not nbias))
                if nbias:
                    nc.tensor.matmul(
                        yps, ones_sb[0:1, 0:128], bias_sb["bo"],
                        start=False, stop=True)
                y = ypool.tile([128, D], F32, name="y", tag="y")
                if y_dve:
                    nc.vector.tensor_copy(y, yps)
                else:
                    nc.scalar.copy(y, yps)
                nc.scalar.dma_start(out[b, ri * 128:(ri + 1) * 128, :], y)

        # proj for batch b+1 is emitted between hp1 and hp2 of batch b
        # (latency-critical DVE lo-subtracts enqueue ahead of later head
        # pairs' DVE chains).  The LAST batch's head pairs are interleaved
        # into batch BC-2's stream so only two chains drain at the tail.
        projs = {0: emit_proj(0)}
        ats = {b: [] for b in range(BC)}
        for b in range(BC - 1):
            qhT, khT, vh = projs.pop(b)
            if b < BC - 2:
                for hp in range(FT):
                    ats[b].append(emit_headpair(hp, qhT, khT, vh))
                    if hp == 1:
                        projs[b + 1] = emit_proj(b + 1)
                emit_y(b, ats[b])
            else:
                # interleave tail: b2.hp0 b2.hp1 [proj3] b2.hp2 b3.hp0
                # b2.hp3 b3.hp1 y2 b3.hp2 b3.hp3 y3
                ats[b].append(emit_headpair(0, qhT, khT, vh))
                ats[b].append(emit_headpair(1, qhT, khT, vh))
                projs[b + 1] = emit_proj(b + 1)
                qhT3, khT3, vh3 = projs.pop(b + 1)
                ats[b].append(emit_headpair(2, qhT, khT, vh))
                ats[b + 1].append(emit_headpair(0, qhT3, khT3, vh3))
                ats[b].append(emit_headpair(3, qhT, khT, vh))
                ats[b + 1].append(emit_headpair(1, qhT3, khT3, vh3, pt_dve=True))
                emit_y(b, ats[b])
                ats[b + 1].append(emit_headpair(2, qhT3, khT3, vh3,
                                                 pt_dve=True, at_dve=True))
                ats[b + 1].append(emit_headpair(3, qhT3, khT3, vh3,
                                                 pt_dve=True, at_dve=True))
                emit_y(b + 1, ats[b + 1], y_dve=True)

    nc.compile()
    return nc


def _split16(x):
    """Split fp32 array into (hi, lo) f16 pair with hi + lo ~= x."""
    hi = x.astype(np.float16)
    lo = (x - hi.astype(np.float32)).astype(np.float16)
    return hi, lo


def kernel(**inputs):
    q = np.asarray(inputs["q"], np.float32)
    k = np.asarray(inputs["k"], np.float32)
    v = np.asarray(inputs["v"], np.float32)
    w_q = np.asarray(inputs["w_q"], np.float32)
    w_k = np.asarray(inputs["w_k"], np.float32)
    w_v = np.asarray(inputs["w_v"], np.float32)
    w_o = np.asarray(inputs["w_o"], np.float32)
    b_q = np.asarray(inputs["b_q"], np.float32)
    b_k = np.asarray(inputs["b_k"], np.float32)
    b_v = np.asarray(inputs["b_v"], np.float32)
    b_o = np.asarray(inputs["b_o"], np.float32)
    k_index = int(np.asarray(inputs["k_index"]))
    assert 1 <= k_index <= 8, f"kernel supports k_index<=8, got {k_index}"

    # fold the 1/sqrt(DK) score scaling into the q projection (exact: 2^-3)
    scale = np.float32(1.0 / math.sqrt(DK))
    w_qs = (w_q * scale).astype(np.float32)
    b_qs = (b_q * scale).astype(np.float32)

    has_bias = {
        "bq": bool(np.any(b_qs)),
        "bk": bool(np.any(b_k)),
        "bv": bool(np.any(b_v)),
        "bo": bool(np.any(b_o)),
    }

    nc = _build_program(k_index, has_bias)
    global _last_nc
    _last_nc = nc

    def _wide_w(w16):
        # [D, D] -> [128, FT*D]: ft-blocks of 128 rows laid side by side
        return np.ascontiguousarray(
            w16.reshape(FT, 128, D).transpose(1, 0, 2).reshape(128, FT * D))

    def _wide_x(x16):
        # [B', D, S] -> [B', 128, FT*S]
        bb = x16.shape[0]
        return np.ascontiguousarray(
            x16.reshape(bb, FT, 128, S).transpose(0, 2, 1, 3)
            .reshape(bb, 128, FT * S))

    wqhi, wqlo = _split16(w_qs)
    wkhi, wklo = _split16(w_k)
    shared = {
        "wqhi": _wide_w(wqhi),
        "wqlo": _wide_w(wqlo),
        "wkhi": _wide_w(wkhi),
        "wklo": _wide_w(wklo),
        "wv": _wide_w(w_v.astype(np.float16)),
        "wo": _wide_w(w_o.astype(np.float16)),
    }
    for nm, arr in (("bq", b_qs), ("bk", b_k), ("bv", b_v), ("bo", b_o)):
        if has_bias[nm]:
            shared[nm] = np.ascontiguousarray(arr.reshape(1, D).astype(np.float32))

    qT = q.transpose(0, 2, 1)
    kT = k.transpose(0, 2, 1)
    vTf = v.transpose(0, 2, 1).astype(np.float16)
    qhiT, qloT = _split16(qT)
    khiT, kloT = _split16(kT)

    in_maps = []
    for c in range(NCORES):
        sl = slice(c * BC, (c + 1) * BC)
        in_maps.append(dict(
            shared,
            qhiT=_wide_x(qhiT[sl]),
            qloT=_wide_x(qloT[sl]),
            khiT=_wide_x(khiT[sl]),
            kloT=_wide_x(kloT[sl]),
            vT=_wide_x(vTf[sl]),
        ))

    res = run_bass_kernel_spmd(
        nc, in_maps, core_ids=list(range(NCORES)), trace=CFG["trace"]
    )
    out = np.concatenate([r["out"] for r in res.results], axis=0)
    kernel.last_result = res
    return out
